# revision 1
# baseline (speedup 1.0000x reference)
"""Trainium2 Bass kernel for nn_ClassicalQuantumAttention.

Data-parallel over batch: 128 batch elems -> 16 per NeuronCore x 8 cores.

Per-core pipeline (per batch elem b):
  scores path : hpreT[j,T] = [Wfold;bfold]^T @ [x;1]  (weights folded on host)
                tanh (ACT) -> scoresT[1,T] = att_w2^T @ tanhT  (PE)
                DMA-scatter scores -> [nc, 16] layout, softmax on DVE/ACT
  chunk path  : xw[nc,C] = sum_k w[nc,k] * xperm[nc, k, :]   (STT chain)
                xwT = PE-transpose;  chunksT = [emb_w;emb_b]^T @ [xwT;1]
                params = sigmoid(chunks @ proj_w + proj_b)   (PE + ACT)
  quantum     : 6-qubit statevector per (b, chunk): 128 states on partitions,
                state = [64 re | 64 im] on free dim.  Gates via fused
                scalar_tensor_tensor with per-partition cos/sin scalars.
  LCU         : mixed = sum_nc coeffs[nc] * evolved  (PE, coeffs from host)
  tail        : normalize, qff ansatz (shared params), expvals (TTR),
                out head + layernorm + classifier (PE + small ops)
"""

import numpy as np
import sys

for _p in ("/opt/trn_rl_repo",):
    if _p not in sys.path:
        sys.path.insert(0, _p)

import concourse.bass as bass
import concourse.tile as tile
from concourse import mybir
from concourse.bass_utils import run_bass_kernel_spmd

F32 = mybir.dt.float32
ALU = mybir.AluOpType
AF = mybir.ActivationFunctionType
AX = mybir.AxisListType

N_CORES = 8
B_TOT = 128
BPC = B_TOT // N_CORES  # 16 batch elems per core
C_IN = 64
T = 2048
D = 256
CH = 16
NC = T // CH  # 128 chunks
NQ = 6
DIM = 64  # 2**6 amplitudes
STF = 2 * DIM  # 128 floats per state ([64 re | 64 im])


# ---------------------------------------------------------------- gate list
def ansatz_gates(n_layers):
    """[(kind, wire-or-(ctrl,tgt), param_idx)] matching reference _ansatz."""
    gates = []
    idx = 0
    for _ in range(n_layers):
        for i in range(NQ):
            gates.append(("rx", i, idx))
            gates.append(("ry", i, idx + 1))
            gates.append(("rz", i, idx + 2))
            idx += 3
        for i in range(NQ):
            gates.append(("crx", (i, (i + 1) % NQ), idx))
            idx += 1
        for i in range(NQ - 1, -1, -1):
            gates.append(("crx", (i, (i - 1) % NQ), idx))
            idx += 1
    return gates


# ------------------------------------------------------------- AP helpers
def amp_view(t, ri, fixed, swap_p=None, split_ps=()):
    """Strided view of a statevector AP t ([P, 128] = [P, (ri, amp6bits)]).

    ri: 0 (re), 1 (im), or None (both -> extra leading free dim).
    fixed: {bit_pos: 0/1} fixes amplitude bits (wire w <-> bit 5-w).
    swap_p: bit position iterated in order (1, 0) via negative step.
    split_ps: bit positions forced into their own [step, 2] dim (to shape-match
              a swap view on another tensor).
    """
    part = t.ap[0]  # partition dim
    offset = t.offset
    dims = []
    if ri is None:
        dims.append([DIM, 2])
    else:
        offset += ri * DIM
    run = None  # [step, count]
    for p in range(5, -1, -1):
        if p in fixed:
            if run is not None:
                dims.append(run)
                run = None
            offset += fixed[p] << p
        elif swap_p == p:
            if run is not None:
                dims.append(run)
                run = None
            dims.append([-(1 << p), 2])
            offset += 1 << p
        elif p in split_ps:
            if run is not None:
                dims.append(run)
                run = None
            dims.append([1 << p, 2])
        else:
            if run is None:
                run = [1 << p, 2]
            else:
                run = [1 << p, run[1] * 2]
    if run is not None:
        dims.append(run)
    if not dims:
        dims.append([1, 1])
    # walrus compute ops accept at most 3 total dims (partition + 2 free)
    assert len(dims) <= 2, f"too many free dims: {dims}"
    return bass.AP(tensor=t.tensor, offset=offset, ap=[list(part)] + dims)


# ------------------------------------------------------------ gate emitters
def g_rx_first(eng, st, c, s, ns, pq, sup):
    """RX on bit pq when all bits <= pq are zero (sparse start)."""
    eng.tensor_scalar_mul(
        amp_view(st, 0, {**sup, pq: 1}), amp_view(st, 1, {**sup, pq: 0}), s
    )
    eng.tensor_scalar_mul(
        amp_view(st, 1, {**sup, pq: 1}), amp_view(st, 0, {**sup, pq: 0}), ns
    )
    v0 = amp_view(st, None, {**sup, pq: 0})
    eng.tensor_scalar_mul(v0, v0, c)


def g_rx(eng, st, B, c, s, pq):
    eng.tensor_scalar_mul(B, st, s)
    for k in (0, 1):
        o = amp_view(st, 0, {pq: k})
        eng.scalar_tensor_tensor(
            o, o, c, amp_view(B, 1, {pq: 1 - k}), ALU.mult, ALU.add
        )
        o = amp_view(st, 1, {pq: k})
        eng.scalar_tensor_tensor(
            o, o, c, amp_view(B, 0, {pq: 1 - k}), ALU.mult, ALU.subtract
        )


def g_ry(eng, st, B, c, s, pq, sup):
    eng.tensor_scalar_mul(amp_view(B, None, sup), amp_view(st, None, sup), s)
    for ri in (0, 1):
        o = amp_view(st, ri, {**sup, pq: 0})
        eng.scalar_tensor_tensor(
            o, o, c, amp_view(B, ri, {**sup, pq: 1}), ALU.mult, ALU.subtract
        )
    for ri in (0, 1):
        o = amp_view(st, ri, {**sup, pq: 1})
        eng.scalar_tensor_tensor(
            o, o, c, amp_view(B, ri, {**sup, pq: 0}), ALU.mult, ALU.add
        )


def g_rz(eng, st, B, c, s, pq, sup):
    eng.tensor_scalar_mul(amp_view(B, None, sup), amp_view(st, None, sup), s)
    for ri, k, op in (
        (0, 0, ALU.add),
        (1, 0, ALU.subtract),
        (0, 1, ALU.subtract),
        (1, 1, ALU.add),
    ):
        o = amp_view(st, ri, {**sup, pq: k})
        eng.scalar_tensor_tensor(
            o, o, c, amp_view(B, 1 - ri, {**sup, pq: k}), ALU.mult, op
        )


def g_crx(eng, st, B, c, s, pc, pt):
    for ri in (0, 1):
        eng.tensor_scalar_mul(
            amp_view(B, ri, {pc: 1}), amp_view(st, ri, {pc: 1}), s
        )
    for kt in (0, 1):
        o = amp_view(st, 0, {pc: 1, pt: kt})
        eng.scalar_tensor_tensor(
            o, o, c, amp_view(B, 1, {pc: 1, pt: 1 - kt}), ALU.mult, ALU.add
        )
        o = amp_view(st, 1, {pc: 1, pt: kt})
        eng.scalar_tensor_tensor(
            o, o, c, amp_view(B, 0, {pc: 1, pt: 1 - kt}), ALU.mult, ALU.subtract
        )


def emit_ansatz(eng, st, B, col, n_layers, sparse):
    """col(j, kind) -> [P,1] AP of cos/sin/negsin for param j."""
    gates = ansatz_gates(n_layers)
    for gi, (kind, loc, j) in enumerate(gates):
        c = col(j, "c")
        s = col(j, "s")
        if kind == "crx":
            wc, wt = loc
            g_crx(eng, st, B, c, s, 5 - wc, 5 - wt)
        else:
            pq = 5 - loc
            in_l0 = sparse and gi < 3 * NQ
            sup = {p: 0 for p in range(pq)} if in_l0 else {}
            if kind == "rx":
                if in_l0:
                    g_rx_first(eng, st, c, s, col(j, "n"), pq, sup)
                else:
                    g_rx(eng, st, B, c, s, pq)
            elif kind == "ry":
                g_ry(eng, st, B, c, s, pq, sup)
            else:
                g_rz(eng, st, B, c, s, pq, sup)


def _split_multi_waits(nc):
    """This walrus build allows at most ONE sync-wait per instruction.

    Hoist extra waits onto same-engine NoOps inserted immediately before the
    offending instruction (engine stalls on the nops first - semantically
    identical).
    """
    ctr = [0]
    for f in nc.m.functions:
        for b in f.blocks:
            new = []
            for inst in b.instructions:
                si = inst.sync_info
                if si is not None and len(si.on_wait) > 1:
                    waits = list(si.on_wait)
                    for w in waits[:-1]:
                        ctr[0] += 1
                        nop = mybir.InstNoOp(
                            name=f"wsplit-{ctr[0]}",
                            ins=[],
                            outs=[],
                            engine=inst.engine,
                            sync_info=mybir.SyncInfo(on_wait=[w], on_update=[]),
                        )
                        new.append(nop)
                    inst.sync_info = mybir.SyncInfo(
                        on_wait=[waits[-1]], on_update=list(si.on_update)
                    )
                new.append(inst)
            b.instructions = new


# ---------------------------------------------------------------- program
def build_program(split_waits=True):
    nc = bass.Bass()

    # register extra activation-bias constants (pi/2 for cos-via-sin, ln eps)
    for v in (float(np.pi / 2), 1e-5):
        t = nc.alloc_sbuf_tensor(f"const-f32-{v}", [128, 1], F32)
        nc.gpsimd.memset(t.ap(), v)
        nc.const_aps.aps[(F32, v)] = t.ap()
    nc.all_engine_barrier()

    # ---- dram I/O (per core) ----
    xs = nc.declare_dram_parameter("xs", [BPC, C_IN, T], F32, isOutput=False)
    xp = nc.declare_dram_parameter("xp", [BPC, NC, CH * C_IN], F32, isOutput=False)
    wfb = nc.declare_dram_parameter("wfb", [C_IN + 1, 128], F32, isOutput=False)
    aw2 = nc.declare_dram_parameter("aw2", [128, 1], F32, isOutput=False)
    ewb = nc.declare_dram_parameter("ewb", [C_IN + 1, D], F32, isOutput=False)
    pjw = nc.declare_dram_parameter("pjw", [128, 120], F32, isOutput=False)
    pjb = nc.declare_dram_parameter("pjb", [1, 60], F32, isOutput=False)
    cf2 = nc.declare_dram_parameter("cf2", [NC, 2], F32, isOutput=False)
    qfc = nc.declare_dram_parameter("qfc", [BPC, 30], F32, isOutput=False)
    qfs = nc.declare_dram_parameter("qfs", [BPC, 30], F32, isOutput=False)
    qfn = nc.declare_dram_parameter("qfn", [BPC, 30], F32, isOutput=False)
    owb = nc.declare_dram_parameter("owb", [19, D], F32, isOutput=False)
    lng = nc.declare_dram_parameter("lng", [BPC, D], F32, isOutput=False)
    lnb = nc.declare_dram_parameter("lnb", [BPC, D], F32, isOutput=False)
    cw1 = nc.declare_dram_parameter("cw1", [128, 2 * D], F32, isOutput=False)
    cb1 = nc.declare_dram_parameter("cb1", [1, D], F32, isOutput=False)
    cw2 = nc.declare_dram_parameter("cw2", [128, 4], F32, isOutput=False)
    cb2 = nc.declare_dram_parameter("cb2", [1, 2], F32, isOutput=False)
    idn = nc.declare_dram_parameter("idn", [128, 128], F32, isOutput=False)
    out = nc.declare_dram_parameter("out", [BPC, 2], F32, isOutput=True)

    with tile.TileContext(nc) as tc:
        with (
            tc.tile_pool(name="const", bufs=1) as cp,
            tc.tile_pool(name="xbuf", bufs=2) as xpool,
            tc.tile_pool(name="xpbuf", bufs=2) as xppool,
            tc.tile_pool(name="tanh", bufs=2) as thpool,
            tc.tile_pool(name="perb", bufs=1) as pb,
            tc.tile_pool(name="stp", bufs=1) as stp,
            tc.tile_pool(name="small", bufs=4) as sm,
            tc.tile_pool(name="ps_h", bufs=2, space="PSUM") as ps_h,
            tc.tile_pool(name="ps_s", bufs=2, space="PSUM") as ps_s,
            tc.tile_pool(name="ps_m", bufs=2, space="PSUM") as ps_m,
            tc.tile_pool(name="ps_t", bufs=2, space="PSUM") as ps_t,
        ):
            # ---------------- constants into SBUF ----------------
            def cload(name, dram, shape):
                t = cp.tile(shape, F32, tag=name, name=name)
                nc.sync.dma_start(out=t, in_=dram[:, :])
                return t

            wfb_s = cload("wfb", wfb, [C_IN + 1, 128])
            aw2_s = cload("aw2", aw2, [128, 1])
            ewb_s = cload("ewb", ewb, [C_IN + 1, D])
            pjw_s = cload("pjw", pjw, [128, 120])
            pjb_s = cload("pjb", pjb, [1, 60])
            cf2_s = cload("cf2", cf2, [NC, 2])
            qfc_s = cload("qfc", qfc, [BPC, 30])
            qfs_s = cload("qfs", qfs, [BPC, 30])
            qfn_s = cload("qfn", qfn, [BPC, 30])
            owb_s = cload("owb", owb, [19, D])
            lng_s = cload("lng", lng, [BPC, D])
            lnb_s = cload("lnb", lnb, [BPC, D])
            cw1_s = cload("cw1", cw1, [128, 2 * D])
            cb1_s = cload("cb1", cb1, [1, D])
            cw2_s = cload("cw2", cw2, [128, 4])
            cb2_s = cload("cb2", cb2, [1, 2])
            idn_s = cload("idn", idn, [128, 128])

            ones = cp.tile([1, 128], F32, tag="ones")
            nc.vector.memset(ones, 1.0)

            # persistent per-group score tiles
            sc_g = [cp.tile([NC, 8 * CH], F32, tag=f"scg{g}", name=f"scg{g}") for g in range(2)]
            esc_g = [cp.tile([NC, 8 * CH], F32, tag=f"escg{g}", name=f"escg{g}") for g in range(2)]
            w_g = [cp.tile([NC, 8 * CH], F32, tag=f"wg{g}", name=f"wg{g}") for g in range(2)]

            B_dve = cp.tile([128, STF], F32, tag="Bdve")
            B_dve2 = cp.tile([128, STF], F32, tag="Bdve2")

            # per-b double buffers
            x_sb = [xpool.tile([C_IN + 1, T], F32, tag="x", name=f"xsb{i}") for i in range(2)]
            xp_sb = [xppool.tile([NC, CH * C_IN], F32, tag="xp", name=f"xpsb{i}") for i in range(2)]
            xwt_sb = [xppool.tile([C_IN + 1, NC], F32, tag="xwt", name=f"xwtsb{i}") for i in range(2)]
            for i in range(2):
                nc.vector.memset(x_sb[i][C_IN : C_IN + 1, :], 1.0)
                nc.vector.memset(xwt_sb[i][C_IN : C_IN + 1, :], 1.0)

            cs_b = []  # per-b (cos, sin, nsin) [128, 64]
            st_b = []  # per-b state tiles
            for b in range(BPC):
                cs_b.append(
                    (
                        pb.tile([NC, 64], F32, tag=f"cos{b}", name=f"cos{b}"),
                        pb.tile([NC, 64], F32, tag=f"sin{b}", name=f"sin{b}"),
                        pb.tile([NC, 64], F32, tag=f"nsin{b}", name=f"nsin{b}"),
                    )
                )
                st_b.append(stp.tile([128, STF], F32, tag=f"st{b}", name=f"st{b}"))

            lq_all = cp.tile([BPC, 2 * STF], F32, tag="lqall")
            mix = cp.tile([BPC, STF], F32, tag="mix")
            B_q = cp.tile([BPC, STF], F32, tag="Bq")
            qfeat = cp.tile([BPC, 19], F32, tag="qfeat")
            nc.vector.memset(qfeat[:, 18:19], 1.0)

            # ================= classical per-b =================
            for b in range(BPC):
                xb = x_sb[b % 2]
                nc.sync.dma_start(out=xb[0:C_IN, :], in_=xs[b, :, :])

                th = thpool.tile([128, T], F32, tag="th")
                for blk in range(4):
                    hp = ps_h.tile([128, 512], F32, tag="hp")
                    nc.tensor.matmul(
                        hp,
                        wfb_s,
                        xb[:, blk * 512 : (blk + 1) * 512],
                        start=True,
                        stop=True,
                    )
                    nc.scalar.activation(
                        th[:, blk * 512 : (blk + 1) * 512], hp, AF.Tanh
                    )
                    sc = ps_s.tile([1, 512], F32, tag="sc")
                    nc.tensor.matmul(
                        sc,
                        aw2_s,
                        th[:, blk * 512 : (blk + 1) * 512],
                        start=True,
                        stop=True,
                    )
                    # PSUM -> SBUF staging (DMA cannot read PSUM), then
                    # DMA-scatter [1, (32 nc, 16)] -> [32 partitions, 16]
                    ssc = sm.tile([1, 512], F32, tag="ssc", name="ssc")
                    if blk % 2 == 0:
                        nc.vector.tensor_copy(ssc, sc)
                    else:
                        nc.scalar.copy(ssc, sc)
                    g, bb = b // 8, b % 8
                    src = ssc.rearrange("p (n k) -> p n k", n=32, k=CH)
                    dst = sc_g[g][blk * 32 : (blk + 1) * 32, bb * CH : (bb + 1) * CH]
                    nc.sync.dma_start(out=dst, in_=src)

                # ---- group softmax + per-b chunk path, after each group of 8
                if b % 8 == 7:
                    g = b // 8
                    nc.scalar.activation(esc_g[g], sc_g[g], AF.Exp)
                    ssum = sm.tile([NC, 8], F32, tag="ssum")
                    nc.vector.tensor_reduce(
                        ssum,
                        esc_g[g].rearrange("p (n k) -> p n k", n=8, k=CH),
                        AX.X,
                        ALU.add,
                    )
                    rsum = sm.tile([NC, 8], F32, tag="rsum")
                    nc.vector.reciprocal(rsum, ssum)
                    for bb in range(8):
                        nc.vector.tensor_scalar_mul(
                            w_g[g][:, bb * CH : (bb + 1) * CH],
                            esc_g[g][:, bb * CH : (bb + 1) * CH],
                            rsum[:, bb : bb + 1],
                        )

                    for bb in range(8):
                        bfull = g * 8 + bb
                        xpb = xp_sb[bfull % 2]
                        nc.sync.dma_start(out=xpb, in_=xp[bfull, :, :])
                        # xw[nc, c] = sum_k w[nc, k] * xperm[nc, k*64: k*64+64]
                        xw = sm.tile([NC, C_IN], F32, tag="xw")
                        nc.vector.tensor_scalar_mul(
                            xw,
                            xpb[:, 0:C_IN],
                            w_g[g][:, bb * CH : bb * CH + 1],
                        )
                        for k in range(1, CH):
                            nc.vector.scalar_tensor_tensor(
                                xw,
                                xpb[:, k * C_IN : (k + 1) * C_IN],
                                w_g[g][:, bb * CH + k : bb * CH + k + 1],
                                xw,
                                ALU.mult,
                                ALU.add,
                            )
                        # transpose -> [64, 128]
                        xwt_ps = ps_m.tile([C_IN, NC], F32, tag="m")
                        nc.tensor.transpose(xwt_ps, xw, idn_s)
                        xwt = xwt_sb[bfull % 2]
                        nc.vector.tensor_copy(xwt[0:C_IN, :], xwt_ps)
                        # chunksT halves + params
                        cht = [None, None]
                        for h in range(2):
                            chp = ps_m.tile([128, NC], F32, tag="m")
                            nc.tensor.matmul(
                                chp,
                                ewb_s[:, h * 128 : (h + 1) * 128],
                                xwt,
                                start=True,
                                stop=True,
                            )
                            cht[h] = sm.tile([128, NC], F32, tag=f"cht{h}", name=f"cht{h}")
                            nc.vector.tensor_copy(cht[h], chp)
                        par = ps_t.tile([NC, 60], F32, tag="t")
                        nc.tensor.matmul(
                            par, cht[0], pjw_s[:, 0:60], start=True, stop=False
                        )
                        nc.tensor.matmul(
                            par, cht[1], pjw_s[:, 60:120], start=False, stop=False
                        )
                        nc.tensor.matmul(
                            par, ones, pjb_s, start=False, stop=True
                        )
                        co, si, ns = cs_b[bfull]
                        # theta = sigmoid(z); cos(theta/2) = sin(theta/2 + pi/2)
                        nc.scalar.activation(par, par, AF.Sigmoid)
                        nc.scalar.activation(
                            co[:, 0:60], par, AF.Sin,
                            bias=float(np.pi / 2), scale=0.5,
                        )
                        nc.scalar.activation(
                            si[:, 0:60], par, AF.Sin, bias=0.0, scale=0.5
                        )
                        nc.scalar.activation(
                            ns[:, 0:60], par, AF.Sin, bias=0.0, scale=-0.5
                        )

            # ================= quantum stage 1 =================
            for b in range(BPC):
                st = st_b[b]
                nc.vector.memset(st, 0.0)
                nc.vector.memset(st[:, 0:1], 1.0)
                co, si, ns = cs_b[b]

                def col(j, kind, co=co, si=si, ns=ns):
                    t = {"c": co, "s": si, "n": ns}[kind]
                    return t[:, j : j + 1]

                Bsc = B_dve if b % 2 == 0 else B_dve2
                emit_ansatz(nc.vector, st, Bsc, col, 2, sparse=True)

                # ---- LCU: 3 matmuls [K=128 nc] ----
                r0 = ps_t.tile([1, STF], F32, tag="t")
                nc.tensor.matmul(r0, cf2_s[:, 0:1], st, start=True, stop=True)
                m2a = ps_t.tile([1, DIM], F32, tag="t")
                nc.tensor.matmul(
                    m2a, cf2_s[:, 1:2], st[:, DIM:STF], start=True, stop=True
                )
                m2b = ps_t.tile([1, DIM], F32, tag="t")
                nc.tensor.matmul(
                    m2b, cf2_s[:, 1:2], st[:, 0:DIM], start=True, stop=True
                )
                # PSUM -> SBUF staging row [1, 256] = [r0 | m2a | m2b]
                lst = sm.tile([1, 2 * STF], F32, tag="lst", name="lst")
                nc.vector.tensor_copy(lst[:, 0:STF], r0)
                nc.vector.tensor_copy(lst[:, STF : STF + DIM], m2a)
                nc.vector.tensor_copy(lst[:, STF + DIM : 2 * STF], m2b)
                nc.sync.dma_start(out=lq_all[b : b + 1, :], in_=lst)

            # mixed = row0 -/+ m2 ; normalize
            nc.vector.tensor_tensor(
                mix[:, 0:DIM], lq_all[:, 0:DIM], lq_all[:, STF : STF + DIM],
                ALU.subtract,
            )
            nc.vector.tensor_tensor(
                mix[:, DIM:STF], lq_all[:, DIM:STF],
                lq_all[:, STF + DIM : 2 * STF], ALU.add,
            )
            sqs = sm.tile([BPC, STF], F32, tag="sqs")
            ss = sm.tile([BPC, 1], F32, tag="ss")
            nc.vector.tensor_tensor(sqs, mix, mix, ALU.mult)
            nc.vector.tensor_reduce(ss, sqs, AX.X, ALU.add)
            sd = sm.tile([BPC, 1], F32, tag="sd")
            nc.scalar.activation(sd, ss, AF.Sqrt)
            rn = sm.tile([BPC, 1], F32, tag="rn")
            nc.vector.reciprocal(rn, sd)
            nc.vector.tensor_scalar_mul(mix, mix, rn)

            # ================= qff ansatz (shared params) =================
            def qcol(j, kind):
                t = {"c": qfc_s, "s": qfs_s, "n": qfn_s}[kind]
                return t[:, j : j + 1]

            emit_ansatz(nc.vector, mix, B_q, qcol, 1, sparse=False)

            # ================= expvals -> qfeat [BPC, 18] =================
            scr = sm.tile([BPC, DIM], F32, tag="scr")
            scr2 = sm.tile([BPC, DIM], F32, tag="scr2")
            tmp1 = sm.tile([BPC, 1], F32, tag="tmp1")
            tmp2 = sm.tile([BPC, 1], F32, tag="tmp2")
            yr2 = sm.tile([BPC, 2], F32, tag="yr2")

            def clike(dst, off, ref):
                """contiguous view of dst at elem offset off, shaped like ref's
                free dims"""
                counts = [d[1] for d in ref.ap[1:]]
                dims = []
                stride = 1
                for c in reversed(counts):
                    dims.insert(0, [stride, c])
                    stride *= c
                return bass.AP(
                    tensor=dst.tensor, offset=dst.offset + off,
                    ap=[list(dst.ap[0])] + dims,
                )

            def prod(dst, off, a, b):
                nc.vector.tensor_tensor(clike(dst, off, a), a, b, ALU.mult)

            for i in range(NQ):
                p = 5 - i
                v = lambda ri, k: amp_view(mix, ri, {p: k})
                # X_i = 2 * sum(s0*s1) over re+im  (x2 applied at the end)
                prod(scr, 0, v(0, 0), v(0, 1))
                prod(scr, 32, v(1, 0), v(1, 1))
                nc.vector.tensor_reduce(qfeat[:, i : i + 1], scr, AX.X, ALU.add)
                # Y_i = 2 * sum(r0*i1 - i0*r1)
                prod(scr, 0, v(0, 0), v(1, 1))
                prod(scr, 32, v(1, 0), v(0, 1))
                nc.vector.tensor_reduce(
                    yr2,
                    scr.rearrange("p (h q) -> p h q", h=2, q=32),
                    AX.X,
                    ALU.add,
                )
                nc.vector.tensor_tensor(
                    qfeat[:, 6 + i : 7 + i], yr2[:, 0:1], yr2[:, 1:2],
                    ALU.subtract,
                )
                # Z_i = sum|bit0|^2 - sum|bit1|^2
                prod(scr, 0, v(0, 0), v(0, 0))
                prod(scr, 32, v(1, 0), v(1, 0))
                prod(scr2, 0, v(0, 1), v(0, 1))
                prod(scr2, 32, v(1, 1), v(1, 1))
                nc.vector.tensor_reduce(tmp1, scr, AX.X, ALU.add)
                nc.vector.tensor_reduce(tmp2, scr2, AX.X, ALU.add)
                nc.vector.tensor_tensor(
                    qfeat[:, 12 + i : 13 + i], tmp1, tmp2, ALU.subtract
                )
            # x2 for the X and Y blocks
            nc.vector.tensor_scalar_mul(qfeat[:, 0:12], qfeat[:, 0:12], 2.0)

            # ================= tail =================
            qfT_ps = ps_m.tile([19, BPC], F32, tag="m")
            nc.tensor.transpose(qfT_ps, qfeat, idn_s[0:BPC, 0:BPC])
            qfT = sm.tile([19, BPC], F32, tag="qfTs")
            nc.vector.tensor_copy(qfT, qfT_ps)
            o1 = ps_t.tile([BPC, D], F32, tag="t")
            nc.tensor.matmul(o1, qfT, owb_s, start=True, stop=True)

            stats = sm.tile([BPC, 6], F32, tag="stats")
            nc.vector.bn_stats(stats, o1)
            mv = sm.tile([BPC, 2], F32, tag="mv")
            nc.vector.bn_aggr(mv, stats)
            sdv = sm.tile([BPC, 1], F32, tag="sdv")
            nc.scalar.activation(sdv, mv[:, 1:2], AF.Sqrt, bias=1e-5)
            rstd = sm.tile([BPC, 1], F32, tag="rstd")
            nc.vector.reciprocal(rstd, sdv)
            ln1 = sm.tile([BPC, D], F32, tag="ln1")
            nc.vector.tensor_scalar(
                ln1, o1, mv[:, 0:1], rstd, ALU.subtract, ALU.mult
            )
            ln2 = sm.tile([BPC, D], F32, tag="ln2")
            nc.vector.tensor_tensor(ln2, ln1, lng_s, ALU.mult)
            nc.vector.tensor_tensor(ln2, ln2, lnb_s, ALU.add)

            # cls layer 1
            lnT = [None, None]
            for h in range(2):
                lnT_ps = ps_m.tile([128, BPC], F32, tag="m")
                nc.tensor.transpose(
                    lnT_ps, ln2[:, h * 128 : (h + 1) * 128], idn_s[0:BPC, 0:BPC]
                )
                lnT[h] = sm.tile([128, BPC], F32, tag=f"lnT{h}", name=f"lnT{h}")
                nc.vector.tensor_copy(lnT[h], lnT_ps)
            h2p = ps_t.tile([BPC, D], F32, tag="t")
            nc.tensor.matmul(h2p, lnT[0], cw1_s[:, 0:D], start=True, stop=False)
            nc.tensor.matmul(
                h2p, lnT[1], cw1_s[:, D : 2 * D], start=False, stop=False
            )
            nc.tensor.matmul(
                h2p, ones[:, 0:BPC], cb1_s, start=False, stop=True
            )
            h2 = sm.tile([BPC, D], F32, tag="h2")
            nc.scalar.activation(h2, h2p, AF.Relu)

            # cls layer 2
            h2T = [None, None]
            for h in range(2):
                h2T_ps = ps_m.tile([128, BPC], F32, tag="m")
                nc.tensor.transpose(
                    h2T_ps, h2[:, h * 128 : (h + 1) * 128], idn_s[0:BPC, 0:BPC]
                )
                h2T[h] = sm.tile([128, BPC], F32, tag=f"h2T{h}", name=f"h2T{h}")
                nc.vector.tensor_copy(h2T[h], h2T_ps)
            lg = ps_t.tile([BPC, 2], F32, tag="t")
            nc.tensor.matmul(lg, h2T[0], cw2_s[:, 0:2], start=True, stop=False)
            nc.tensor.matmul(lg, h2T[1], cw2_s[:, 2:4], start=False, stop=False)
            nc.tensor.matmul(lg, ones[:, 0:BPC], cb2_s, start=False, stop=True)
            lgs = sm.tile([BPC, 2], F32, tag="lgs")
            nc.vector.tensor_copy(lgs, lg)
            nc.sync.dma_start(out=out[:, :], in_=lgs)

    if split_waits:
        _split_multi_waits(nc)
    return nc


_NC_CACHE = {}


def _get_program():
    if "nc" not in _NC_CACHE:
        _NC_CACHE["nc"] = build_program()
    return _NC_CACHE["nc"]


def host_prep(inputs):
    """Host-side parameter folding -> per-core input maps."""
    f32 = np.float32
    x = np.asarray(inputs["x"], f32)
    emb_w = np.asarray(inputs["emb_w"], np.float64)
    emb_b = np.asarray(inputs["emb_b"], np.float64)
    att_w1 = np.asarray(inputs["att_w1"], np.float64)
    att_b1 = np.asarray(inputs["att_b1"], np.float64)

    wfold = (emb_w @ att_w1).astype(f32)  # [64, 128]
    bfold = (emb_b @ att_w1 + att_b1).astype(f32)  # [128]
    wfb = np.concatenate([wfold, bfold[None, :]], 0)  # [65, 128]

    ewb = np.concatenate(
        [emb_w.astype(f32), emb_b.astype(f32)[None, :]], 0
    )  # [65, 256]

    pw = np.asarray(inputs["proj_w"], f32)  # [256, 60]
    pjw = np.concatenate([pw[0:128, :], pw[128:256, :]], 1)  # [128, 120]

    cr = np.asarray(inputs["mix_re"], np.float64)
    ci = np.asarray(inputs["mix_im"], np.float64)
    den = np.sqrt(cr * cr + ci * ci).sum() + 1e-8
    cf2 = np.stack([cr / den, ci / den], 1).astype(f32)  # [128, 2]

    qp = np.asarray(inputs["qff_params"], np.float64) * 0.5
    qfc = np.broadcast_to(np.cos(qp).astype(f32), (BPC, 30)).copy()
    qfs = np.broadcast_to(np.sin(qp).astype(f32), (BPC, 30)).copy()
    qfn = (-qfs).copy()

    owb = np.concatenate(
        [np.asarray(inputs["out_w"], f32), np.asarray(inputs["out_b"], f32)[None, :]],
        0,
    )  # [19, 256]
    lng = np.broadcast_to(np.asarray(inputs["ln_g"], f32), (BPC, D)).copy()
    lnb = np.broadcast_to(np.asarray(inputs["ln_b"], f32), (BPC, D)).copy()
    w1 = np.asarray(inputs["cls_w1"], f32)
    cw1 = np.concatenate([w1[0:128, :], w1[128:256, :]], 1)  # [128, 512]
    cb1 = np.asarray(inputs["cls_b1"], f32)[None, :]
    w2 = np.asarray(inputs["cls_w2"], f32)
    cw2 = np.concatenate([w2[0:128, :], w2[128:256, :]], 1)  # [128, 4]
    cb2 = np.asarray(inputs["cls_b2"], f32)[None, :]
    idn = np.eye(128, dtype=f32)
    pjb = np.asarray(inputs["proj_b"], f32)[None, :]

    shared = dict(
        wfb=wfb, aw2=np.asarray(inputs["att_w2"], f32), ewb=ewb, pjw=pjw,
        pjb=pjb, cf2=cf2, qfc=qfc, qfs=qfs, qfn=qfn, owb=owb, lng=lng,
        lnb=lnb, cw1=cw1, cb1=cb1, cw2=cw2, cb2=cb2, idn=idn,
    )

    in_maps = []
    for c in range(N_CORES):
        xc = x[c * BPC : (c + 1) * BPC]  # [16, 64, 2048]
        # xperm[b, nc, k*64+c] = x[b, c, nc*16+k]
        xp_c = np.ascontiguousarray(
            xc.reshape(BPC, C_IN, NC, CH).transpose(0, 2, 3, 1).reshape(
                BPC, NC, CH * C_IN
            )
        )
        m = dict(shared)
        m["xs"] = np.ascontiguousarray(xc)
        m["xp"] = xp_c
        in_maps.append(m)
    return in_maps


def kernel(**inputs):
    nc = _get_program()
    in_maps = host_prep(inputs)
    res = run_bass_kernel_spmd(nc, in_maps, core_ids=list(range(N_CORES)))
    outs = [res.results[c]["out"] for c in range(N_CORES)]
    return np.concatenate(outs, 0).astype(np.float32)


if __name__ == "__main__":
    nc = build_program()
    print("program built ok")



# revision 3
# speedup vs baseline: 2.0652x; 2.0652x over previous
"""Trainium2 Bass kernel for nn_ClassicalQuantumAttention.

Data-parallel over batch: 128 batch elems -> 16 per NeuronCore x 8 cores.

Per-core pipeline:
  classical   : scores path (PE matmuls + ACT tanh + softmax) and chunk path
                (weighted chunk sums, emb/proj matmuls) as in the baseline;
                circuit params sigmoid+sin/cos written as fp16 into SHARED
                coefficient tiles co/si/ns [128 chunks, 60 params x 16 b].
  quantum     : ALL 16 batch elems' statevectors in ONE fp16 tile
                ST [128 part = chunk, free = ri(2) x amp(64) x b(16)],
                b innermost.  Each gate = ~5 large tensor_tensor ops
                (FD 512-2048, fp16 2x mode) with per-(chunk,b) cos/sin
                applied via stride-0 broadcast views.  Layer-1 rotations
                use sparse (support-restricted) views.
  LCU         : per-b matmuls over chunk partitions (as baseline), then
                normalize on [16, 128].
  qff ansatz  : the 30 shared-parameter gates are ONE host-precomputed
                128x128 real matrix; applied by PE transpose + matmul.
  tail        : expvals (DVE quadratic forms), out head + layernorm +
                classifier (PE + small ops), as baseline.
"""

import numpy as np
import sys

for _p in ("/opt/trn_rl_repo",):
    if _p not in sys.path:
        sys.path.insert(0, _p)

import concourse.bass as bass
import concourse.tile as tile
from concourse import mybir
from concourse.bass_utils import run_bass_kernel_spmd

F32 = mybir.dt.float32
F16 = mybir.dt.float16
ALU = mybir.AluOpType
AF = mybir.ActivationFunctionType
AX = mybir.AxisListType

N_CORES = 8
B_TOT = 128
BPC = B_TOT // N_CORES  # 16 batch elems per core
C_IN = 64
T = 2048
D = 256
CH = 16
NC = T // CH  # 128 chunks
NQ = 6
DIM = 64  # 2**6 amplitudes
STF = 2 * DIM  # 128 floats per state ([64 re | 64 im])

# big-state free layout: idx = ri*1024 + amp*16 + b
SB = BPC          # 16 (b inner)
SAMP = DIM * SB   # 1024 (one ri slab)
SFREE = 2 * SAMP  # 2048


# ---------------------------------------------------------------- gate list
def ansatz_gates(n_layers):
    """[(kind, wire-or-(ctrl,tgt), param_idx)] matching reference _ansatz."""
    gates = []
    idx = 0
    for _ in range(n_layers):
        for i in range(NQ):
            gates.append(("rx", i, idx))
            gates.append(("ry", i, idx + 1))
            gates.append(("rz", i, idx + 2))
            idx += 3
        for i in range(NQ):
            gates.append(("crx", (i, (i + 1) % NQ), idx))
            idx += 1
        for i in range(NQ - 1, -1, -1):
            gates.append(("crx", (i, (i - 1) % NQ), idx))
            idx += 1
    return gates


# ------------------------------------------------------------- AP helpers
def fview(t, dims, off):
    return bass.AP(tensor=t.tensor, offset=t.offset + off, ap=[list(t.ap[0])] + dims)


def v_full(t, ri=None, w=6):
    """All involved amps (support width w: amps {k*2^(6-w)}), b inner.

    ri None: both ri slabs merged into the outer dim."""
    p = 6 - w
    step = (1 << p) * SB
    n = 1 << w
    if ri is None:
        return fview(t, [[step, 2 * n], [1, SB]], 0)
    return fview(t, [[step, n], [1, SB]], ri * SAMP)


def v_bit(t, p, val, ri=None, w=6):
    """Amps with bit p fixed to val; support width w (w<6 implies p == 6-w,
    lower bits all zero)."""
    off = val * (1 << p) * SB + (0 if ri is None else ri * SAMP)
    if w == 6:
        step_hi = (1 << (p + 1)) * SB
        n_hi = 1 << (5 - p)
        inner = (1 << p) * SB
        if ri is None:
            return fview(t, [[step_hi, 2 * n_hi], [1, inner]], off)
        return fview(t, [[step_hi, n_hi], [1, inner]], off)
    assert p == 6 - w
    step = (1 << (p + 1)) * SB
    n = 1 << (w - 1)
    if ri is None:
        return fview(t, [[step, 2 * n], [1, SB]], off)
    return fview(t, [[step, n], [1, SB]], off)


def v_2bit(t, ph, pl, vh, vl):
    """Both-ri view fixing adjacent amp bits ph = pl+1."""
    assert ph == pl + 1
    step_hi = (1 << (ph + 1)) * SB
    n_hi = 1 << (5 - ph)
    inner = (1 << pl) * SB
    off = (vh * (1 << ph) + vl * (1 << pl)) * SB
    return fview(t, [[step_hi, 2 * n_hi], [1, inner]], off)


def v_2bit_wrap(t, v5, v0, ri):
    """Per-ri view fixing amp bits 5 and 0 (the non-adjacent wrap case)."""
    off = ri * SAMP + (v5 * 32 + v0) * SB
    return fview(t, [[2 * SB, 16], [1, SB]], off)


def cview(ct, j, n):
    """Coefficient view for param j: [128, [0,n],[1,16]] (b inner)."""
    return bass.AP(
        tensor=ct.tensor, offset=ct.offset + SB * j,
        ap=[list(ct.ap[0]), [0, n], [1, SB]],
    )


# ------------------------------------------------------------ gate emitters
def emit_big_ansatz(nc, ST, B, B2, co, si, ns, cm1, gates, sparse_first):
    tt = nc.vector.tensor_tensor

    def rot(kind, p, j, w):
        n1 = 1 << w        # outer count of per-ri involved view
        n2 = 2 * n1        # both-ri
        if kind == "ry":
            # B = s*ST (no ri swap); ST *= c; ST[p0] -= B[p1]; ST[p1] += B[p0]
            tt(v_full(B, None, w), v_full(ST, None, w), cview(si, j, n2), ALU.mult)
            tt(v_full(ST, None, w), v_full(ST, None, w), cview(co, j, n2), ALU.mult)
            tt(v_bit(ST, p, 0, None, w), v_bit(ST, p, 0, None, w),
               v_bit(B, p, 1, None, w), ALU.subtract)
            tt(v_bit(ST, p, 1, None, w), v_bit(ST, p, 1, None, w),
               v_bit(B, p, 0, None, w), ALU.add)
            return
        # rx / rz: B[re] = s*ST[im]; B[im] = -s*ST[re]
        tt(v_full(B, 0, w), v_full(ST, 1, w), cview(si, j, n1), ALU.mult)
        tt(v_full(B, 1, w), v_full(ST, 0, w), cview(ns, j, n1), ALU.mult)
        tt(v_full(ST, None, w), v_full(ST, None, w), cview(co, j, n2), ALU.mult)
        if kind == "rx":
            # ST[p0] += B[p1]; ST[p1] += B[p0]
            tt(v_bit(ST, p, 0, None, w), v_bit(ST, p, 0, None, w),
               v_bit(B, p, 1, None, w), ALU.add)
            tt(v_bit(ST, p, 1, None, w), v_bit(ST, p, 1, None, w),
               v_bit(B, p, 0, None, w), ALU.add)
        else:  # rz: ST[p0] += B[p0]; ST[p1] -= B[p1]
            tt(v_bit(ST, p, 0, None, w), v_bit(ST, p, 0, None, w),
               v_bit(B, p, 0, None, w), ALU.add)
            tt(v_bit(ST, p, 1, None, w), v_bit(ST, p, 1, None, w),
               v_bit(B, p, 1, None, w), ALU.subtract)

    def crx(pc, pt, j):
        # B[re] = s*ST[im]; B[im] = -s*ST[re]; B2 = (c-1)*ST
        tt(v_full(B, 0), v_full(ST, 1), cview(si, j, 64), ALU.mult)
        tt(v_full(B, 1), v_full(ST, 0), cview(ns, j, 64), ALU.mult)
        tt(v_full(B2, None), v_full(ST, None), cview(cm1, j, 128), ALU.mult)
        # ST[pc=1] += B2[pc=1]   (-> c*ST on the control-1 half)
        tt(v_bit(ST, pc, 1), v_bit(ST, pc, 1), v_bit(B2, pc, 1), ALU.add)
        # ST[pc=1, pt=k] += B[pc=1, pt=1-k]
        if abs(pc - pt) == 1:
            ph, pl = max(pc, pt), min(pc, pt)
            cv = lambda b: 1 if pc == ph else b  # value of bit ph for (pc=1, pt=b)
            tv = lambda b: 1 if pt == ph else b  # helper: not used; explicit below
            for k in (0, 1):
                if pc == ph:
                    o = v_2bit(ST, ph, pl, 1, k)
                    i1 = v_2bit(B, ph, pl, 1, 1 - k)
                else:
                    o = v_2bit(ST, ph, pl, k, 1)
                    i1 = v_2bit(B, ph, pl, 1 - k, 1)
                tt(o, o, i1, ALU.add)
        else:
            # wrap: bits {5, 0}
            for k in (0, 1):
                for ri in (0, 1):
                    if pc == 0:  # control bit0, target bit5
                        o = v_2bit_wrap(ST, k, 1, ri)
                        i1 = v_2bit_wrap(B, 1 - k, 1, ri)
                    else:  # control bit5, target bit0
                        o = v_2bit_wrap(ST, 1, k, ri)
                        i1 = v_2bit_wrap(B, 1, 1 - k, ri)
                    tt(o, o, i1, ALU.add)

    for gi, (kind, loc, j) in enumerate(gates):
        if kind == "crx":
            crx(5 - loc[0], 5 - loc[1], j)
        else:
            w = (loc + 1) if (sparse_first and gi < 3 * NQ) else 6
            rot(kind, 5 - loc, j, w)


# --------------------------------------------- baseline amp_view (tail use)
def amp_view(t, ri, fixed, swap_p=None, split_ps=()):
    """Strided view of a statevector AP t ([P, 128] = [P, (ri, amp6bits)])."""
    part = t.ap[0]
    offset = t.offset
    dims = []
    if ri is None:
        dims.append([DIM, 2])
    else:
        offset += ri * DIM
    run = None
    for p in range(5, -1, -1):
        if p in fixed:
            if run is not None:
                dims.append(run)
                run = None
            offset += fixed[p] << p
        elif swap_p == p:
            if run is not None:
                dims.append(run)
                run = None
            dims.append([-(1 << p), 2])
            offset += 1 << p
        elif p in split_ps:
            if run is not None:
                dims.append(run)
                run = None
            dims.append([1 << p, 2])
        else:
            if run is None:
                run = [1 << p, 2]
            else:
                run = [1 << p, run[1] * 2]
    if run is not None:
        dims.append(run)
    if not dims:
        dims.append([1, 1])
    assert len(dims) <= 2, f"too many free dims: {dims}"
    return bass.AP(tensor=t.tensor, offset=offset, ap=[list(part)] + dims)


def _split_multi_waits(nc):
    """This walrus build allows at most ONE sync-wait per instruction."""
    ctr = [0]
    for f in nc.m.functions:
        for b in f.blocks:
            new = []
            for inst in b.instructions:
                si = inst.sync_info
                if si is not None and len(si.on_wait) > 1:
                    waits = list(si.on_wait)
                    for w in waits[:-1]:
                        ctr[0] += 1
                        nop = mybir.InstNoOp(
                            name=f"wsplit-{ctr[0]}",
                            ins=[],
                            outs=[],
                            engine=inst.engine,
                            sync_info=mybir.SyncInfo(on_wait=[w], on_update=[]),
                        )
                        new.append(nop)
                    inst.sync_info = mybir.SyncInfo(
                        on_wait=[waits[-1]], on_update=list(si.on_update)
                    )
                new.append(inst)
            b.instructions = new


# ---------------------------------------------------------------- program
def build_program(split_waits=True):
    nc = bass.Bass()

    for v in (float(np.pi / 2), 1e-5, -1.0):
        t = nc.alloc_sbuf_tensor(f"const-f32-{v}", [128, 1], F32)
        nc.gpsimd.memset(t.ap(), v)
        nc.const_aps.aps[(F32, v)] = t.ap()
    nc.all_engine_barrier()

    # ---- dram I/O (per core) ----
    xs = nc.declare_dram_parameter("xs", [BPC, C_IN, T], F32, isOutput=False)
    xp = nc.declare_dram_parameter("xp", [BPC, NC, CH * C_IN], F32, isOutput=False)
    wfb = nc.declare_dram_parameter("wfb", [C_IN + 1, 128], F32, isOutput=False)
    aw2 = nc.declare_dram_parameter("aw2", [128, 1], F32, isOutput=False)
    ewb = nc.declare_dram_parameter("ewb", [C_IN + 1, D], F32, isOutput=False)
    pjw = nc.declare_dram_parameter("pjw", [128, 120], F32, isOutput=False)
    pjb = nc.declare_dram_parameter("pjb", [1, 60], F32, isOutput=False)
    cf2 = nc.declare_dram_parameter("cf2", [NC, 2], F16, isOutput=False)
    mqt = nc.declare_dram_parameter("mqt", [STF, STF], F16, isOutput=False)
    owb = nc.declare_dram_parameter("owb", [19, D], F32, isOutput=False)
    lng = nc.declare_dram_parameter("lng", [BPC, D], F32, isOutput=False)
    lnb = nc.declare_dram_parameter("lnb", [BPC, D], F32, isOutput=False)
    cw1 = nc.declare_dram_parameter("cw1", [128, 2 * D], F32, isOutput=False)
    cb1 = nc.declare_dram_parameter("cb1", [1, D], F32, isOutput=False)
    cw2 = nc.declare_dram_parameter("cw2", [128, 4], F32, isOutput=False)
    cb2 = nc.declare_dram_parameter("cb2", [1, 2], F32, isOutput=False)
    idn = nc.declare_dram_parameter("idn", [128, 128], F32, isOutput=False)
    out = nc.declare_dram_parameter("out", [BPC, 2], F32, isOutput=True)

    with tile.TileContext(nc) as tc:
        with (
            tc.tile_pool(name="const", bufs=1) as cp,
            tc.tile_pool(name="xbuf", bufs=2) as xpool,
            tc.tile_pool(name="xpbuf", bufs=2) as xppool,
            tc.tile_pool(name="tanh", bufs=2) as thpool,
            tc.tile_pool(name="small", bufs=4) as sm,
            tc.tile_pool(name="ps_h", bufs=2, space="PSUM") as ps_h,
            tc.tile_pool(name="ps_s", bufs=2, space="PSUM") as ps_s,
            tc.tile_pool(name="ps_m", bufs=2, space="PSUM") as ps_m,
            tc.tile_pool(name="ps_t", bufs=2, space="PSUM") as ps_t,
        ):
            # ---------------- constants into SBUF ----------------
            def cload(name, dram, shape, dt=F32):
                t = cp.tile(shape, dt, tag=name, name=name)
                nc.sync.dma_start(out=t, in_=dram[:, :])
                return t

            wfb_s = cload("wfb", wfb, [C_IN + 1, 128])
            aw2_s = cload("aw2", aw2, [128, 1])
            ewb_s = cload("ewb", ewb, [C_IN + 1, D])
            pjw_s = cload("pjw", pjw, [128, 120])
            pjb_s = cload("pjb", pjb, [1, 60])
            cf2_s = cload("cf2", cf2, [NC, 2], F16)
            mqt_s = cload("mqt", mqt, [STF, STF], F16)
            owb_s = cload("owb", owb, [19, D])
            lng_s = cload("lng", lng, [BPC, D])
            lnb_s = cload("lnb", lnb, [BPC, D])
            cw1_s = cload("cw1", cw1, [128, 2 * D])
            cb1_s = cload("cb1", cb1, [1, D])
            cw2_s = cload("cw2", cw2, [128, 4])
            cb2_s = cload("cb2", cb2, [1, 2])
            idn_s = cload("idn", idn, [128, 128])

            ones = cp.tile([1, 128], F32, tag="ones")
            nc.vector.memset(ones, 1.0)

            # persistent per-group score tiles
            sc_g = [cp.tile([NC, 8 * CH], F32, tag=f"scg{g}", name=f"scg{g}") for g in range(2)]
            esc_g = [cp.tile([NC, 8 * CH], F32, tag=f"escg{g}", name=f"escg{g}") for g in range(2)]
            w_g = [cp.tile([NC, 8 * CH], F32, tag=f"wg{g}", name=f"wg{g}") for g in range(2)]

            # shared fp16 coefficient tiles: free = param_j*16 + b
            co_t = cp.tile([NC, 60 * SB], F16, tag="co", name="co")
            si_t = cp.tile([NC, 60 * SB], F16, tag="si", name="si")
            ns_t = cp.tile([NC, 60 * SB], F16, tag="ns", name="ns")
            cm1_t = cp.tile([NC, 60 * SB], F16, tag="cm1", name="cm1")

            # big state + scratch tiles
            ST = cp.tile([NC, SFREE], F16, tag="ST", name="ST")
            Bt = cp.tile([NC, SFREE], F16, tag="Bt", name="Bt")
            B2t = cp.tile([NC, SFREE], F16, tag="B2t", name="B2t")

            # per-b double buffers
            x_sb = [xpool.tile([C_IN + 1, T], F32, tag="x", name=f"xsb{i}") for i in range(2)]
            xp_sb = [xppool.tile([NC, CH * C_IN], F32, tag="xp", name=f"xpsb{i}") for i in range(2)]
            xwt_sb = [xppool.tile([C_IN + 1, NC], F32, tag="xwt", name=f"xwtsb{i}") for i in range(2)]
            for i in range(2):
                nc.vector.memset(x_sb[i][C_IN : C_IN + 1, :], 1.0)
                nc.vector.memset(xwt_sb[i][C_IN : C_IN + 1, :], 1.0)

            lq_all = cp.tile([BPC, 2 * STF], F32, tag="lqall")
            mix = cp.tile([BPC, STF], F32, tag="mix")
            qfeat = cp.tile([BPC, 19], F32, tag="qfeat")
            nc.vector.memset(qfeat[:, 18:19], 1.0)

            # ================= classical per-b =================
            for b in range(BPC):
                xb = x_sb[b % 2]
                nc.sync.dma_start(out=xb[0:C_IN, :], in_=xs[b, :, :])

                th = thpool.tile([128, T], F32, tag="th")
                for blk in range(4):
                    hp = ps_h.tile([128, 512], F32, tag="hp")
                    nc.tensor.matmul(
                        hp,
                        wfb_s,
                        xb[:, blk * 512 : (blk + 1) * 512],
                        start=True,
                        stop=True,
                    )
                    nc.scalar.activation(
                        th[:, blk * 512 : (blk + 1) * 512], hp, AF.Tanh
                    )
                    sc = ps_s.tile([1, 512], F32, tag="sc")
                    nc.tensor.matmul(
                        sc,
                        aw2_s,
                        th[:, blk * 512 : (blk + 1) * 512],
                        start=True,
                        stop=True,
                    )
                    ssc = sm.tile([1, 512], F32, tag="ssc", name="ssc")
                    if blk % 2 == 0:
                        nc.vector.tensor_copy(ssc, sc)
                    else:
                        nc.scalar.copy(ssc, sc)
                    g, bb = b // 8, b % 8
                    src = ssc.rearrange("p (n k) -> p n k", n=32, k=CH)
                    dst = sc_g[g][blk * 32 : (blk + 1) * 32, bb * CH : (bb + 1) * CH]
                    nc.sync.dma_start(out=dst, in_=src)

                # ---- group softmax + per-b chunk path, after each group of 8
                if b % 8 == 7:
                    g = b // 8
                    nc.scalar.activation(esc_g[g], sc_g[g], AF.Exp)
                    ssum = sm.tile([NC, 8], F32, tag="ssum")
                    nc.vector.tensor_reduce(
                        ssum,
                        esc_g[g].rearrange("p (n k) -> p n k", n=8, k=CH),
                        AX.X,
                        ALU.add,
                    )
                    rsum = sm.tile([NC, 8], F32, tag="rsum")
                    nc.vector.reciprocal(rsum, ssum)
                    for bb in range(8):
                        nc.vector.tensor_scalar_mul(
                            w_g[g][:, bb * CH : (bb + 1) * CH],
                            esc_g[g][:, bb * CH : (bb + 1) * CH],
                            rsum[:, bb : bb + 1],
                        )

                    for bb in range(8):
                        bfull = g * 8 + bb
                        xpb = xp_sb[bfull % 2]
                        nc.sync.dma_start(out=xpb, in_=xp[bfull, :, :])
                        xw = sm.tile([NC, C_IN], F32, tag="xw")
                        nc.vector.tensor_scalar_mul(
                            xw,
                            xpb[:, 0:C_IN],
                            w_g[g][:, bb * CH : bb * CH + 1],
                        )
                        for k in range(1, CH):
                            nc.vector.scalar_tensor_tensor(
                                xw,
                                xpb[:, k * C_IN : (k + 1) * C_IN],
                                w_g[g][:, bb * CH + k : bb * CH + k + 1],
                                xw,
                                ALU.mult,
                                ALU.add,
                            )
                        xwt_ps = ps_m.tile([C_IN, NC], F32, tag="m")
                        nc.tensor.transpose(xwt_ps, xw, idn_s)
                        xwt = xwt_sb[bfull % 2]
                        nc.vector.tensor_copy(xwt[0:C_IN, :], xwt_ps)
                        cht = [None, None]
                        for h in range(2):
                            chp = ps_m.tile([128, NC], F32, tag="m")
                            nc.tensor.matmul(
                                chp,
                                ewb_s[:, h * 128 : (h + 1) * 128],
                                xwt,
                                start=True,
                                stop=True,
                            )
                            cht[h] = sm.tile([128, NC], F32, tag=f"cht{h}", name=f"cht{h}")
                            nc.vector.tensor_copy(cht[h], chp)
                        par = ps_t.tile([NC, 60], F32, tag="t")
                        nc.tensor.matmul(
                            par, cht[0], pjw_s[:, 0:60], start=True, stop=False
                        )
                        nc.tensor.matmul(
                            par, cht[1], pjw_s[:, 60:120], start=False, stop=False
                        )
                        nc.tensor.matmul(
                            par, ones, pjb_s, start=False, stop=True
                        )
                        # theta = sigmoid(z); coefficient columns (stride 16)
                        nc.scalar.activation(par, par, AF.Sigmoid)
                        co_v = fview(co_t, [[SB, 60]], bfull)
                        si_v = fview(si_t, [[SB, 60]], bfull)
                        ns_v = fview(ns_t, [[SB, 60]], bfull)
                        nc.scalar.activation(
                            co_v, par, AF.Sin, bias=float(np.pi / 2), scale=0.5
                        )
                        nc.scalar.activation(si_v, par, AF.Sin, bias=0.0, scale=0.5)
                        nc.scalar.activation(ns_v, par, AF.Sin, bias=0.0, scale=-0.5)

            # cm1 = cos - 1 (for CRX), one shot
            nc.scalar.activation(cm1_t, co_t, AF.Copy, bias=-1.0)

            # ================= quantum stage 1 (b-batched) =================
            nc.vector.memset(ST, 0.0)
            nc.vector.memset(fview(ST, [[1, SB]], 0), 1.0)  # amp0, re, all b

            emit_big_ansatz(
                nc, ST, Bt, B2t, co_t, si_t, ns_t, cm1_t,
                ansatz_gates(2), sparse_first=True,
            )

            # ---- LCU: per-b matmuls over chunk partitions ----
            for b in range(BPC):
                rhs_all = fview(ST, [[SB, STF]], b)
                rhs_re = fview(ST, [[SB, DIM]], b)
                rhs_im = fview(ST, [[SB, DIM]], SAMP + b)
                r0 = ps_t.tile([1, STF], F32, tag="t")
                nc.tensor.matmul(r0, cf2_s[:, 0:1], rhs_all, start=True, stop=True)
                m2a = ps_t.tile([1, DIM], F32, tag="t")
                nc.tensor.matmul(m2a, cf2_s[:, 1:2], rhs_im, start=True, stop=True)
                m2b = ps_t.tile([1, DIM], F32, tag="t")
                nc.tensor.matmul(m2b, cf2_s[:, 1:2], rhs_re, start=True, stop=True)
                lst = sm.tile([1, 2 * STF], F32, tag="lst", name="lst")
                nc.vector.tensor_copy(lst[:, 0:STF], r0)
                nc.vector.tensor_copy(lst[:, STF : STF + DIM], m2a)
                nc.vector.tensor_copy(lst[:, STF + DIM : 2 * STF], m2b)
                nc.sync.dma_start(out=lq_all[b : b + 1, :], in_=lst)

            # mixed = row0 -/+ m2 ; normalize
            nc.vector.tensor_tensor(
                mix[:, 0:DIM], lq_all[:, 0:DIM], lq_all[:, STF : STF + DIM],
                ALU.subtract,
            )
            nc.vector.tensor_tensor(
                mix[:, DIM:STF], lq_all[:, DIM:STF],
                lq_all[:, STF + DIM : 2 * STF], ALU.add,
            )
            sqs = sm.tile([BPC, STF], F32, tag="sqs")
            ss = sm.tile([BPC, 1], F32, tag="ss")
            nc.vector.tensor_tensor(sqs, mix, mix, ALU.mult)
            nc.vector.tensor_reduce(ss, sqs, AX.X, ALU.add)
            sd = sm.tile([BPC, 1], F32, tag="sd")
            nc.scalar.activation(sd, ss, AF.Sqrt)
            rn = sm.tile([BPC, 1], F32, tag="rn")
            nc.vector.reciprocal(rn, sd)
            nc.vector.tensor_scalar_mul(mix, mix, rn)

            # ================= qff ansatz: one 128x128 matmul =================
            mT_ps = ps_m.tile([STF, BPC], F32, tag="m")
            nc.tensor.transpose(mT_ps, mix, idn_s[0:BPC, 0:BPC])
            mT = sm.tile([STF, BPC], F16, tag="mT")
            nc.vector.tensor_copy(mT, mT_ps)
            qT_ps = ps_m.tile([STF, BPC], F32, tag="m")
            nc.tensor.matmul(qT_ps, mqt_s, mT, start=True, stop=True)
            qTs = sm.tile([STF, BPC], F32, tag="qTs")
            nc.vector.tensor_copy(qTs, qT_ps)
            q2_ps = ps_m.tile([BPC, STF], F32, tag="m")
            nc.tensor.transpose(q2_ps, qTs, idn_s)
            nc.vector.tensor_copy(mix, q2_ps)

            # ================= expvals -> qfeat [BPC, 18] =================
            scr = sm.tile([BPC, DIM], F32, tag="scr")
            scr2 = sm.tile([BPC, DIM], F32, tag="scr2")
            tmp1 = sm.tile([BPC, 1], F32, tag="tmp1")
            tmp2 = sm.tile([BPC, 1], F32, tag="tmp2")
            yr2 = sm.tile([BPC, 2], F32, tag="yr2")

            def clike(dst, off, ref):
                counts = [d[1] for d in ref.ap[1:]]
                dims = []
                stride = 1
                for c in reversed(counts):
                    dims.insert(0, [stride, c])
                    stride *= c
                return bass.AP(
                    tensor=dst.tensor, offset=dst.offset + off,
                    ap=[list(dst.ap[0])] + dims,
                )

            def prod(dst, off, a, b):
                nc.vector.tensor_tensor(clike(dst, off, a), a, b, ALU.mult)

            for i in range(NQ):
                p = 5 - i
                v = lambda ri, k: amp_view(mix, ri, {p: k})
                prod(scr, 0, v(0, 0), v(0, 1))
                prod(scr, 32, v(1, 0), v(1, 1))
                nc.vector.tensor_reduce(qfeat[:, i : i + 1], scr, AX.X, ALU.add)
                prod(scr, 0, v(0, 0), v(1, 1))
                prod(scr, 32, v(1, 0), v(0, 1))
                nc.vector.tensor_reduce(
                    yr2,
                    scr.rearrange("p (h q) -> p h q", h=2, q=32),
                    AX.X,
                    ALU.add,
                )
                nc.vector.tensor_tensor(
                    qfeat[:, 6 + i : 7 + i], yr2[:, 0:1], yr2[:, 1:2],
                    ALU.subtract,
                )
                prod(scr, 0, v(0, 0), v(0, 0))
                prod(scr, 32, v(1, 0), v(1, 0))
                prod(scr2, 0, v(0, 1), v(0, 1))
                prod(scr2, 32, v(1, 1), v(1, 1))
                nc.vector.tensor_reduce(tmp1, scr, AX.X, ALU.add)
                nc.vector.tensor_reduce(tmp2, scr2, AX.X, ALU.add)
                nc.vector.tensor_tensor(
                    qfeat[:, 12 + i : 13 + i], tmp1, tmp2, ALU.subtract
                )
            nc.vector.tensor_scalar_mul(qfeat[:, 0:12], qfeat[:, 0:12], 2.0)

            # ================= tail =================
            qfT_ps = ps_m.tile([19, BPC], F32, tag="m")
            nc.tensor.transpose(qfT_ps, qfeat, idn_s[0:BPC, 0:BPC])
            qfT = sm.tile([19, BPC], F32, tag="qfTs")
            nc.vector.tensor_copy(qfT, qfT_ps)
            o1 = ps_t.tile([BPC, D], F32, tag="t")
            nc.tensor.matmul(o1, qfT, owb_s, start=True, stop=True)

            stats = sm.tile([BPC, 6], F32, tag="stats")
            nc.vector.bn_stats(stats, o1)
            mv = sm.tile([BPC, 2], F32, tag="mv")
            nc.vector.bn_aggr(mv, stats)
            sdv = sm.tile([BPC, 1], F32, tag="sdv")
            nc.scalar.activation(sdv, mv[:, 1:2], AF.Sqrt, bias=1e-5)
            rstd = sm.tile([BPC, 1], F32, tag="rstd")
            nc.vector.reciprocal(rstd, sdv)
            ln1 = sm.tile([BPC, D], F32, tag="ln1")
            nc.vector.tensor_scalar(
                ln1, o1, mv[:, 0:1], rstd, ALU.subtract, ALU.mult
            )
            ln2 = sm.tile([BPC, D], F32, tag="ln2")
            nc.vector.tensor_tensor(ln2, ln1, lng_s, ALU.mult)
            nc.vector.tensor_tensor(ln2, ln2, lnb_s, ALU.add)

            # cls layer 1
            lnT = [None, None]
            for h in range(2):
                lnT_ps = ps_m.tile([128, BPC], F32, tag="m")
                nc.tensor.transpose(
                    lnT_ps, ln2[:, h * 128 : (h + 1) * 128], idn_s[0:BPC, 0:BPC]
                )
                lnT[h] = sm.tile([128, BPC], F32, tag=f"lnT{h}", name=f"lnT{h}")
                nc.vector.tensor_copy(lnT[h], lnT_ps)
            h2p = ps_t.tile([BPC, D], F32, tag="t")
            nc.tensor.matmul(h2p, lnT[0], cw1_s[:, 0:D], start=True, stop=False)
            nc.tensor.matmul(
                h2p, lnT[1], cw1_s[:, D : 2 * D], start=False, stop=False
            )
            nc.tensor.matmul(
                h2p, ones[:, 0:BPC], cb1_s, start=False, stop=True
            )
            h2 = sm.tile([BPC, D], F32, tag="h2")
            nc.scalar.activation(h2, h2p, AF.Relu)

            # cls layer 2
            h2T = [None, None]
            for h in range(2):
                h2T_ps = ps_m.tile([128, BPC], F32, tag="m")
                nc.tensor.transpose(
                    h2T_ps, h2[:, h * 128 : (h + 1) * 128], idn_s[0:BPC, 0:BPC]
                )
                h2T[h] = sm.tile([128, BPC], F32, tag=f"h2T{h}", name=f"h2T{h}")
                nc.vector.tensor_copy(h2T[h], h2T_ps)
            lg = ps_t.tile([BPC, 2], F32, tag="t")
            nc.tensor.matmul(lg, h2T[0], cw2_s[:, 0:2], start=True, stop=False)
            nc.tensor.matmul(lg, h2T[1], cw2_s[:, 2:4], start=False, stop=False)
            nc.tensor.matmul(lg, ones[:, 0:BPC], cb2_s, start=False, stop=True)
            lgs = sm.tile([BPC, 2], F32, tag="lgs")
            nc.vector.tensor_copy(lgs, lg)
            nc.sync.dma_start(out=out[:, :], in_=lgs)

    if split_waits:
        _split_multi_waits(nc)
    return nc


_NC_CACHE = {}


def _get_program():
    if "nc" not in _NC_CACHE:
        _NC_CACHE["nc"] = build_program()
    return _NC_CACHE["nc"]


def _qff_matrix(qp):
    """Compose the 30 shared-parameter qff gates into one 64x64 complex matrix."""
    U = np.eye(DIM, dtype=np.complex128)
    for kind, loc, j in ansatz_gates(1):
        th = float(qp[j])
        c, s = np.cos(th / 2), np.sin(th / 2)
        G = np.zeros((DIM, DIM), np.complex128)
        if kind == "crx":
            wc, wt = loc
            bc, bt = 5 - wc, 5 - wt
            for k in range(DIM):
                if (k >> bc) & 1:
                    G[k, k] = c
                    G[k, k ^ (1 << bt)] = -1j * s
                else:
                    G[k, k] = 1.0
        else:
            bq = 5 - loc
            for k in range(DIM):
                kb = (k >> bq) & 1
                if kind == "rx":
                    G[k, k] = c
                    G[k, k ^ (1 << bq)] = -1j * s
                elif kind == "ry":
                    G[k, k] = c
                    G[k, k ^ (1 << bq)] = -s if kb == 0 else s
                else:  # rz
                    G[k, k] = np.exp(-0.5j * th) if kb == 0 else np.exp(0.5j * th)
        U = G @ U
    return U


def host_prep(inputs):
    """Host-side parameter folding -> per-core input maps."""
    f32 = np.float32
    x = np.asarray(inputs["x"], f32)
    emb_w = np.asarray(inputs["emb_w"], np.float64)
    emb_b = np.asarray(inputs["emb_b"], np.float64)
    att_w1 = np.asarray(inputs["att_w1"], np.float64)
    att_b1 = np.asarray(inputs["att_b1"], np.float64)

    wfold = (emb_w @ att_w1).astype(f32)
    bfold = (emb_b @ att_w1 + att_b1).astype(f32)
    wfb = np.concatenate([wfold, bfold[None, :]], 0)

    ewb = np.concatenate(
        [emb_w.astype(f32), emb_b.astype(f32)[None, :]], 0
    )

    pw = np.asarray(inputs["proj_w"], f32)
    pjw = np.concatenate([pw[0:128, :], pw[128:256, :]], 1)

    cr = np.asarray(inputs["mix_re"], np.float64)
    ci = np.asarray(inputs["mix_im"], np.float64)
    den = np.sqrt(cr * cr + ci * ci).sum() + 1e-8
    cf2 = np.stack([cr / den, ci / den], 1).astype(np.float16)

    qp = np.asarray(inputs["qff_params"], np.float64)
    U = _qff_matrix(qp)
    M = np.block([[U.real, -U.imag], [U.imag, U.real]])
    mqt = np.ascontiguousarray(M.T).astype(np.float16)

    owb = np.concatenate(
        [np.asarray(inputs["out_w"], f32), np.asarray(inputs["out_b"], f32)[None, :]],
        0,
    )
    lng = np.broadcast_to(np.asarray(inputs["ln_g"], f32), (BPC, D)).copy()
    lnb = np.broadcast_to(np.asarray(inputs["ln_b"], f32), (BPC, D)).copy()
    w1 = np.asarray(inputs["cls_w1"], f32)
    cw1 = np.concatenate([w1[0:128, :], w1[128:256, :]], 1)
    cb1 = np.asarray(inputs["cls_b1"], f32)[None, :]
    w2 = np.asarray(inputs["cls_w2"], f32)
    cw2 = np.concatenate([w2[0:128, :], w2[128:256, :]], 1)
    cb2 = np.asarray(inputs["cls_b2"], f32)[None, :]
    idn = np.eye(128, dtype=f32)
    pjb = np.asarray(inputs["proj_b"], f32)[None, :]

    shared = dict(
        wfb=wfb, aw2=np.asarray(inputs["att_w2"], f32), ewb=ewb, pjw=pjw,
        pjb=pjb, cf2=cf2, mqt=mqt, owb=owb, lng=lng,
        lnb=lnb, cw1=cw1, cb1=cb1, cw2=cw2, cb2=cb2, idn=idn,
    )

    in_maps = []
    for c in range(N_CORES):
        xc = x[c * BPC : (c + 1) * BPC]
        xp_c = np.ascontiguousarray(
            xc.reshape(BPC, C_IN, NC, CH).transpose(0, 2, 3, 1).reshape(
                BPC, NC, CH * C_IN
            )
        )
        m = dict(shared)
        m["xs"] = np.ascontiguousarray(xc)
        m["xp"] = xp_c
        in_maps.append(m)
    return in_maps


def kernel(**inputs):
    nc = _get_program()
    in_maps = host_prep(inputs)
    res = run_bass_kernel_spmd(nc, in_maps, core_ids=list(range(N_CORES)))
    outs = [res.results[c]["out"] for c in range(N_CORES)]
    return np.concatenate(outs, 0).astype(np.float32)


if __name__ == "__main__":
    nc = build_program()
    print("program built ok")


# revision 15
# speedup vs baseline: 2.5364x; 1.2282x over previous
"""Trainium2 Bass kernel for nn_ClassicalQuantumAttention.

Data-parallel over batch: 128 batch elems -> 16 per NeuronCore x 8 cores.

Per-core pipeline:
  classical   : scores path (PE matmuls + ACT tanh + softmax) and chunk path
                (weighted chunk sums, emb/proj matmuls) as in the baseline;
                circuit params sigmoid+sin/cos written as fp16 into SHARED
                coefficient tiles co/si/ns [128 chunks, 60 params x 16 b].
  quantum     : ALL 16 batch elems' statevectors in ONE fp16 tile
                ST [128 part = chunk, free = ri(2) x amp(64) x b(16)],
                b innermost.  Each gate = ~5 large tensor_tensor ops
                (FD 512-2048, fp16 2x mode) with per-(chunk,b) cos/sin
                applied via stride-0 broadcast views.  Layer-1 rotations
                use sparse (support-restricted) views.
  LCU         : per-b matmuls over chunk partitions (as baseline), then
                normalize on [16, 128].
  qff ansatz  : the 30 shared-parameter gates are ONE host-precomputed
                128x128 real matrix; applied by PE transpose + matmul.
  tail        : expvals (DVE quadratic forms), out head + layernorm +
                classifier (PE + small ops), as baseline.
"""

import numpy as np
import sys

for _p in ("/opt/trn_rl_repo",):
    if _p not in sys.path:
        sys.path.insert(0, _p)

import concourse.bass as bass
import concourse.tile as tile
from concourse import mybir
from concourse.bass_utils import run_bass_kernel_spmd

F32 = mybir.dt.float32
F16 = mybir.dt.float16
ALU = mybir.AluOpType
AF = mybir.ActivationFunctionType
AX = mybir.AxisListType

N_CORES = 8
B_TOT = 128
BPC = B_TOT // N_CORES  # 16 batch elems per core
C_IN = 64
T = 2048
D = 256
CH = 16
NC = T // CH  # 128 chunks
NQ = 6
DIM = 64  # 2**6 amplitudes
STF = 2 * DIM  # 128 floats per state ([64 re | 64 im])

# big-state free layout: idx = ri*1024 + amp*16 + b
SB = BPC          # 16 (b inner)
SAMP = DIM * SB   # 1024 (one ri slab)
SFREE = 2 * SAMP  # 2048


# ---------------------------------------------------------------- gate list
def ansatz_gates(n_layers):
    """[(kind, wire-or-(ctrl,tgt), param_idx)] matching reference _ansatz."""
    gates = []
    idx = 0
    for _ in range(n_layers):
        for i in range(NQ):
            gates.append(("rx", i, idx))
            gates.append(("ry", i, idx + 1))
            gates.append(("rz", i, idx + 2))
            idx += 3
        for i in range(NQ):
            gates.append(("crx", (i, (i + 1) % NQ), idx))
            idx += 1
        for i in range(NQ - 1, -1, -1):
            gates.append(("crx", (i, (i - 1) % NQ), idx))
            idx += 1
    return gates


# ------------------------------------------------------------- AP helpers
def fview(t, dims, off):
    return bass.AP(tensor=t.tensor, offset=t.offset + off, ap=[list(t.ap[0])] + dims)


def v_full(t, ri=None, w=6):
    """All involved amps (support width w: amps {k*2^(6-w)}), b inner.

    ri None: both ri slabs merged into the outer dim."""
    p = 6 - w
    step = (1 << p) * SB
    n = 1 << w
    if ri is None:
        return fview(t, [[step, 2 * n], [1, SB]], 0)
    return fview(t, [[step, n], [1, SB]], ri * SAMP)


def v_bit(t, p, val, ri=None, w=6):
    """Amps with bit p fixed to val; support width w (w<6 implies p == 6-w,
    lower bits all zero)."""
    off = val * (1 << p) * SB + (0 if ri is None else ri * SAMP)
    if w == 6:
        step_hi = (1 << (p + 1)) * SB
        n_hi = 1 << (5 - p)
        inner = (1 << p) * SB
        if ri is None:
            return fview(t, [[step_hi, 2 * n_hi], [1, inner]], off)
        return fview(t, [[step_hi, n_hi], [1, inner]], off)
    assert p == 6 - w
    step = (1 << (p + 1)) * SB
    n = 1 << (w - 1)
    if ri is None:
        return fview(t, [[step, 2 * n], [1, SB]], off)
    return fview(t, [[step, n], [1, SB]], off)


def v_2bit(t, ph, pl, vh, vl):
    """Both-ri view fixing adjacent amp bits ph = pl+1."""
    assert ph == pl + 1
    step_hi = (1 << (ph + 1)) * SB
    n_hi = 1 << (5 - ph)
    inner = (1 << pl) * SB
    off = (vh * (1 << ph) + vl * (1 << pl)) * SB
    return fview(t, [[step_hi, 2 * n_hi], [1, inner]], off)


def v_2bit_wrap(t, v5, v0, ri):
    """Per-ri view fixing amp bits 5 and 0 (the non-adjacent wrap case)."""
    off = ri * SAMP + (v5 * 32 + v0) * SB
    return fview(t, [[2 * SB, 16], [1, SB]], off)


def cview(ct, j, n):
    """Coefficient view for param j: [128, [0,n],[1,16]] (b inner)."""
    return bass.AP(
        tensor=ct.tensor, offset=ct.offset + SB * j,
        ap=[list(ct.ap[0]), [0, n], [1, SB]],
    )


# ------------------------------------------------------------ gate emitters
def emit_big_ansatz(nc, ST, B, B2, co, si, ns, cm1, gates, sparse_first):
    tt = nc.vector.tensor_tensor

    def rot(kind, p, j, w):
        n1 = 1 << w        # outer count of per-ri involved view
        n2 = 2 * n1        # both-ri
        if kind == "ry":
            # B = s*ST (no ri swap); ST *= c; ST[p0] -= B[p1]; ST[p1] += B[p0]
            tt(v_full(B, None, w), v_full(ST, None, w), cview(si, j, n2), ALU.mult)
            tt(v_full(ST, None, w), v_full(ST, None, w), cview(co, j, n2), ALU.mult)
            tt(v_bit(ST, p, 0, None, w), v_bit(ST, p, 0, None, w),
               v_bit(B, p, 1, None, w), ALU.subtract)
            tt(v_bit(ST, p, 1, None, w), v_bit(ST, p, 1, None, w),
               v_bit(B, p, 0, None, w), ALU.add)
            return
        # rx / rz: B[re] = s*ST[im]; B[im] = -s*ST[re]
        tt(v_full(B, 0, w), v_full(ST, 1, w), cview(si, j, n1), ALU.mult)
        tt(v_full(B, 1, w), v_full(ST, 0, w), cview(ns, j, n1), ALU.mult)
        tt(v_full(ST, None, w), v_full(ST, None, w), cview(co, j, n2), ALU.mult)
        if kind == "rx":
            # ST[p0] += B[p1]; ST[p1] += B[p0]
            tt(v_bit(ST, p, 0, None, w), v_bit(ST, p, 0, None, w),
               v_bit(B, p, 1, None, w), ALU.add)
            tt(v_bit(ST, p, 1, None, w), v_bit(ST, p, 1, None, w),
               v_bit(B, p, 0, None, w), ALU.add)
        else:  # rz: ST[p0] += B[p0]; ST[p1] -= B[p1]
            tt(v_bit(ST, p, 0, None, w), v_bit(ST, p, 0, None, w),
               v_bit(B, p, 0, None, w), ALU.add)
            tt(v_bit(ST, p, 1, None, w), v_bit(ST, p, 1, None, w),
               v_bit(B, p, 1, None, w), ALU.subtract)

    def crx(pc, pt, j):
        # B[re] = s*ST[im]; B[im] = -s*ST[re]; B2 = (c-1)*ST
        tt(v_full(B, 0), v_full(ST, 1), cview(si, j, 64), ALU.mult)
        tt(v_full(B, 1), v_full(ST, 0), cview(ns, j, 64), ALU.mult)
        tt(v_full(B2, None), v_full(ST, None), cview(cm1, j, 128), ALU.mult)
        # ST[pc=1] += B2[pc=1]   (-> c*ST on the control-1 half)
        tt(v_bit(ST, pc, 1), v_bit(ST, pc, 1), v_bit(B2, pc, 1), ALU.add)
        # ST[pc=1, pt=k] += B[pc=1, pt=1-k]
        if abs(pc - pt) == 1:
            ph, pl = max(pc, pt), min(pc, pt)
            cv = lambda b: 1 if pc == ph else b  # value of bit ph for (pc=1, pt=b)
            tv = lambda b: 1 if pt == ph else b  # helper: not used; explicit below
            for k in (0, 1):
                if pc == ph:
                    o = v_2bit(ST, ph, pl, 1, k)
                    i1 = v_2bit(B, ph, pl, 1, 1 - k)
                else:
                    o = v_2bit(ST, ph, pl, k, 1)
                    i1 = v_2bit(B, ph, pl, 1 - k, 1)
                tt(o, o, i1, ALU.add)
        else:
            # wrap: bits {5, 0}
            for k in (0, 1):
                for ri in (0, 1):
                    if pc == 0:  # control bit0, target bit5
                        o = v_2bit_wrap(ST, k, 1, ri)
                        i1 = v_2bit_wrap(B, 1 - k, 1, ri)
                    else:  # control bit5, target bit0
                        o = v_2bit_wrap(ST, 1, k, ri)
                        i1 = v_2bit_wrap(B, 1, 1 - k, ri)
                    tt(o, o, i1, ALU.add)

    for gi, (kind, loc, j) in enumerate(gates):
        if kind == "crx":
            crx(5 - loc[0], 5 - loc[1], j)
        else:
            w = (loc + 1) if (sparse_first and gi < 3 * NQ) else 6
            rot(kind, 5 - loc, j, w)


# --------------------------------------------- baseline amp_view (tail use)
def amp_view(t, ri, fixed, swap_p=None, split_ps=()):
    """Strided view of a statevector AP t ([P, 128] = [P, (ri, amp6bits)])."""
    part = t.ap[0]
    offset = t.offset
    dims = []
    if ri is None:
        dims.append([DIM, 2])
    else:
        offset += ri * DIM
    run = None
    for p in range(5, -1, -1):
        if p in fixed:
            if run is not None:
                dims.append(run)
                run = None
            offset += fixed[p] << p
        elif swap_p == p:
            if run is not None:
                dims.append(run)
                run = None
            dims.append([-(1 << p), 2])
            offset += 1 << p
        elif p in split_ps:
            if run is not None:
                dims.append(run)
                run = None
            dims.append([1 << p, 2])
        else:
            if run is None:
                run = [1 << p, 2]
            else:
                run = [1 << p, run[1] * 2]
    if run is not None:
        dims.append(run)
    if not dims:
        dims.append([1, 1])
    assert len(dims) <= 2, f"too many free dims: {dims}"
    return bass.AP(tensor=t.tensor, offset=offset, ap=[list(part)] + dims)


def _split_multi_waits(nc):
    """This walrus build allows at most ONE sync-wait per instruction."""
    ctr = [0]
    for f in nc.m.functions:
        for b in f.blocks:
            new = []
            for inst in b.instructions:
                si = inst.sync_info
                if si is not None and len(si.on_wait) > 1:
                    waits = list(si.on_wait)
                    for w in waits[:-1]:
                        ctr[0] += 1
                        nop = mybir.InstNoOp(
                            name=f"wsplit-{ctr[0]}",
                            ins=[],
                            outs=[],
                            engine=inst.engine,
                            sync_info=mybir.SyncInfo(on_wait=[w], on_update=[]),
                        )
                        new.append(nop)
                    inst.sync_info = mybir.SyncInfo(
                        on_wait=[waits[-1]], on_update=list(si.on_update)
                    )
                new.append(inst)
            b.instructions = new


# ---------------------------------------------------------------- program
def build_program(split_waits=True):
    nc = bass.Bass()

    for v in (float(np.pi / 2), 1e-5, -1.0):
        t = nc.alloc_sbuf_tensor(f"const-f32-{v}", [128, 1], F32)
        nc.gpsimd.memset(t.ap(), v)
        nc.const_aps.aps[(F32, v)] = t.ap()
    nc.all_engine_barrier()

    # ---- dram I/O (per core) ----
    xs = nc.declare_dram_parameter("xs", [BPC, C_IN, T], F16, isOutput=False)
    xp = nc.declare_dram_parameter("xp", [BPC, NC, CH * C_IN], F16, isOutput=False)
    wfb = nc.declare_dram_parameter("wfb", [C_IN + 1, 128], F16, isOutput=False)
    aw2 = nc.declare_dram_parameter("aw2", [128, 1], F16, isOutput=False)
    ewb = nc.declare_dram_parameter("ewb", [C_IN + 1, D], F16, isOutput=False)
    pjw = nc.declare_dram_parameter("pjw", [128, 120], F16, isOutput=False)
    pjb = nc.declare_dram_parameter("pjb", [1, 60], F16, isOutput=False)
    cf2 = nc.declare_dram_parameter("cf2", [NC, 2], F16, isOutput=False)
    mqt = nc.declare_dram_parameter("mqt", [STF, STF], F16, isOutput=False)
    owb = nc.declare_dram_parameter("owb", [19, D], F32, isOutput=False)
    lng = nc.declare_dram_parameter("lng", [BPC, D], F32, isOutput=False)
    lnb = nc.declare_dram_parameter("lnb", [BPC, D], F32, isOutput=False)
    cw1 = nc.declare_dram_parameter("cw1", [128, 2 * D], F32, isOutput=False)
    cb1 = nc.declare_dram_parameter("cb1", [1, D], F32, isOutput=False)
    cw2 = nc.declare_dram_parameter("cw2", [128, 4], F32, isOutput=False)
    cb2 = nc.declare_dram_parameter("cb2", [1, 2], F32, isOutput=False)
    idn = nc.declare_dram_parameter("idn", [128, 128], F32, isOutput=False)
    out = nc.declare_dram_parameter("out", [BPC, 2], F32, isOutput=True)

    with tile.TileContext(nc) as tc:
        with (
            tc.tile_pool(name="const", bufs=1) as cp,
            tc.tile_pool(name="xbuf", bufs=2) as xpool,
            tc.tile_pool(name="xpbuf", bufs=2) as xppool,
            tc.tile_pool(name="tanh", bufs=2) as thpool,
            tc.tile_pool(name="small", bufs=4) as sm,
            tc.tile_pool(name="ps_h", bufs=2, space="PSUM") as ps_h,
            tc.tile_pool(name="ps_s", bufs=2, space="PSUM") as ps_s,
            tc.tile_pool(name="ps_m", bufs=2, space="PSUM") as ps_m,
            tc.tile_pool(name="ps_t", bufs=2, space="PSUM") as ps_t,
        ):
            # ---------------- constants into SBUF ----------------
            def cload(name, dram, shape, dt=F32):
                t = cp.tile(shape, dt, tag=name, name=name)
                nc.sync.dma_start(out=t, in_=dram[:, :])
                return t

            wfb_s = cload("wfb", wfb, [C_IN + 1, 128], F16)
            aw2_s = cload("aw2", aw2, [128, 1], F16)
            ewb_s = cload("ewb", ewb, [C_IN + 1, D], F16)
            pjw_s = cload("pjw", pjw, [128, 120], F16)
            pjb_s = cload("pjb", pjb, [1, 60], F16)
            cf2_s = cload("cf2", cf2, [NC, 2], F16)
            mqt_s = cload("mqt", mqt, [STF, STF], F16)
            owb_s = cload("owb", owb, [19, D])
            lng_s = cload("lng", lng, [BPC, D])
            lnb_s = cload("lnb", lnb, [BPC, D])
            cw1_s = cload("cw1", cw1, [128, 2 * D])
            cb1_s = cload("cb1", cb1, [1, D])
            cw2_s = cload("cw2", cw2, [128, 4])
            cb2_s = cload("cb2", cb2, [1, 2])
            idn_s = cload("idn", idn, [128, 128])

            ones = cp.tile([1, 128], F32, tag="ones")
            nc.vector.memset(ones, 1.0)
            ones16 = cp.tile([1, 128], F16, tag="ones16")
            nc.vector.memset(ones16, 1.0)

            # persistent per-group score tiles
            sc_g = [cp.tile([NC, 8 * CH], F32, tag=f"scg{g}", name=f"scg{g}") for g in range(2)]
            esc_g = [cp.tile([NC, 8 * CH], F32, tag=f"escg{g}", name=f"escg{g}") for g in range(2)]
            w_g = [cp.tile([NC, 8 * CH], F16, tag=f"wg{g}", name=f"wg{g}") for g in range(2)]

            # shared fp16 coefficient tiles: free = param_j*16 + b
            co_t = cp.tile([NC, 60 * SB], F16, tag="co", name="co")
            si_t = cp.tile([NC, 60 * SB], F16, tag="si", name="si")
            ns_t = cp.tile([NC, 60 * SB], F16, tag="ns", name="ns")
            cm1_t = cp.tile([NC, 60 * SB], F16, tag="cm1", name="cm1")

            # big state + scratch tiles
            ST = cp.tile([NC, SFREE], F16, tag="ST", name="ST")
            Bt = cp.tile([NC, SFREE], F16, tag="Bt", name="Bt")
            B2t = cp.tile([NC, SFREE], F16, tag="B2t", name="B2t")

            # per-b double buffers
            x_sb = [xpool.tile([C_IN + 1, T], F16, tag="x", name=f"xsb{i}") for i in range(2)]
            xp_sb = [xppool.tile([NC, CH * C_IN], F16, tag="xp", name=f"xpsb{i}") for i in range(2)]
            xwt_sb = [xppool.tile([C_IN + 1, NC], F16, tag="xwt", name=f"xwtsb{i}") for i in range(2)]
            for i in range(2):
                nc.vector.memset(x_sb[i][C_IN : C_IN + 1, :], 1.0)
                nc.vector.memset(xwt_sb[i][C_IN : C_IN + 1, :], 1.0)

            # staged sigmoid inputs: free = param_j*16 + b (for batched ACT)
            theta_all = cp.tile([NC, 60 * SB], F32, tag="theta", name="theta")

            lq_all = cp.tile([BPC, 2 * STF], F32, tag="lqall")
            mix = cp.tile([BPC, STF], F32, tag="mix")
            qfeat = cp.tile([BPC, 19], F32, tag="qfeat")
            nc.vector.memset(qfeat[:, 18:19], 1.0)

            # ================= classical per-b =================
            for b in range(BPC):
                xb = x_sb[b % 2]
                nc.sync.dma_start(out=xb[0:C_IN, :], in_=xs[b, :, :])

                th = thpool.tile([128, T], F16, tag="th")
                for blk in range(4):
                    hp = ps_h.tile([128, 512], F32, tag="hp")
                    nc.tensor.matmul(
                        hp,
                        wfb_s,
                        xb[:, blk * 512 : (blk + 1) * 512],
                        start=True,
                        stop=True,
                    )
                    nc.scalar.activation(
                        th[:, blk * 512 : (blk + 1) * 512], hp, AF.Tanh
                    )
                    sc = ps_s.tile([1, 512], F32, tag="sc")
                    nc.tensor.matmul(
                        sc,
                        aw2_s,
                        th[:, blk * 512 : (blk + 1) * 512],
                        start=True,
                        stop=True,
                    )
                    ssc = sm.tile([1, 512], F32, tag="ssc", name="ssc")
                    if blk % 2 == 0:
                        nc.vector.tensor_copy(ssc, sc)
                    else:
                        nc.scalar.copy(ssc, sc)
                    g, bb = b // 8, b % 8
                    src = ssc.rearrange("p (n k) -> p n k", n=32, k=CH)
                    dst = sc_g[g][blk * 32 : (blk + 1) * 32, bb * CH : (bb + 1) * CH]
                    nc.sync.dma_start(out=dst, in_=src)

                # ---- group softmax + per-b chunk path, after each group of 8
                if b % 8 == 7:
                    g = b // 8
                    nc.scalar.activation(esc_g[g], sc_g[g], AF.Exp)
                    ssum = sm.tile([NC, 8], F32, tag="ssum")
                    nc.vector.tensor_reduce(
                        ssum,
                        esc_g[g].rearrange("p (n k) -> p n k", n=8, k=CH),
                        AX.X,
                        ALU.add,
                    )
                    rsum = sm.tile([NC, 8], F32, tag="rsum")
                    nc.vector.reciprocal(rsum, ssum)
                    for bb in range(8):
                        nc.vector.tensor_scalar_mul(
                            w_g[g][:, bb * CH : (bb + 1) * CH],
                            esc_g[g][:, bb * CH : (bb + 1) * CH],
                            rsum[:, bb : bb + 1],
                        )

                    for bb in range(8):
                        bfull = g * 8 + bb
                        xpb = xp_sb[bfull % 2]
                        nc.sync.dma_start(out=xpb, in_=xp[bfull, :, :])
                        # xw[nc, c] = sum_k w[nc, k] * xpb[nc, c*16+k]
                        xwp = sm.tile([NC, CH * C_IN], F16, tag="xwp")
                        wv = bass.AP(
                            tensor=w_g[g].tensor,
                            offset=w_g[g].offset + bb * CH,
                            ap=[list(w_g[g].ap[0]), [0, C_IN], [1, CH]],
                        )
                        xv = fview(xpb, [[CH, C_IN], [1, CH]], 0)
                        ov = fview(xwp, [[CH, C_IN], [1, CH]], 0)
                        nc.vector.tensor_tensor(ov, xv, wv, ALU.mult)
                        xw = sm.tile([NC, C_IN], F32, tag="xw")
                        nc.vector.tensor_reduce(
                            xw,
                            xwp.rearrange("p (c k) -> p c k", c=C_IN, k=CH),
                            AX.X,
                            ALU.add,
                        )
                        xwt_ps = ps_m.tile([C_IN, NC], F32, tag="m")
                        nc.tensor.transpose(xwt_ps, xw, idn_s)
                        xwt = xwt_sb[bfull % 2]
                        nc.vector.tensor_copy(xwt[0:C_IN, :], xwt_ps)
                        cht = [None, None]
                        for h in range(2):
                            chp = ps_m.tile([128, NC], F32, tag="m")
                            nc.tensor.matmul(
                                chp,
                                ewb_s[:, h * 128 : (h + 1) * 128],
                                xwt,
                                start=True,
                                stop=True,
                            )
                            cht[h] = sm.tile([128, NC], F16, tag=f"cht{h}", name=f"cht{h}")
                            nc.vector.tensor_copy(cht[h], chp)
                        par = ps_t.tile([NC, 60], F32, tag="t")
                        nc.tensor.matmul(
                            par, cht[0], pjw_s[:, 0:60], start=True, stop=False
                        )
                        nc.tensor.matmul(
                            par, cht[1], pjw_s[:, 60:120], start=False, stop=False
                        )
                        nc.tensor.matmul(
                            par, ones16, pjb_s, start=False, stop=True
                        )
                        # stage sigmoid input into strided (j*16+b) slots
                        nc.scalar.copy(fview(theta_all, [[SB, 60]], bfull), par)

            # batched: theta = sigmoid(z); cos/sin/negsin/cos-1 (fp16)
            nc.scalar.activation(theta_all, theta_all, AF.Sigmoid)
            nc.scalar.activation(
                co_t, theta_all, AF.Sin, bias=float(np.pi / 2), scale=0.5
            )
            nc.scalar.activation(si_t, theta_all, AF.Sin, bias=0.0, scale=0.5)
            nc.scalar.activation(ns_t, theta_all, AF.Sin, bias=0.0, scale=-0.5)
            nc.scalar.activation(cm1_t, co_t, AF.Copy, bias=-1.0)

            # ================= quantum stage 1 (b-batched) =================
            nc.vector.memset(ST, 0.0)
            nc.vector.memset(fview(ST, [[1, SB]], 0), 1.0)  # amp0, re, all b

            emit_big_ansatz(
                nc, ST, Bt, B2t, co_t, si_t, ns_t, cm1_t,
                ansatz_gates(2), sparse_first=True,
            )

            # ---- LCU: per-b matmuls over chunk partitions ----
            for b in range(BPC):
                rhs_all = fview(ST, [[SB, STF]], b)
                rhs_re = fview(ST, [[SB, DIM]], b)
                rhs_im = fview(ST, [[SB, DIM]], SAMP + b)
                r0 = ps_t.tile([1, STF], F32, tag="t")
                nc.tensor.matmul(r0, cf2_s[:, 0:1], rhs_all, start=True, stop=True)
                m2a = ps_t.tile([1, DIM], F32, tag="t")
                nc.tensor.matmul(m2a, cf2_s[:, 1:2], rhs_im, start=True, stop=True)
                m2b = ps_t.tile([1, DIM], F32, tag="t")
                nc.tensor.matmul(m2b, cf2_s[:, 1:2], rhs_re, start=True, stop=True)
                lst = sm.tile([1, 2 * STF], F32, tag="lst", name="lst")
                nc.scalar.copy(lst[:, 0:STF], r0)
                nc.scalar.copy(lst[:, STF : STF + DIM], m2a)
                nc.scalar.copy(lst[:, STF + DIM : 2 * STF], m2b)
                nc.sync.dma_start(out=lq_all[b : b + 1, :], in_=lst)

            # mixed = row0 -/+ m2 ; normalize
            nc.vector.tensor_tensor(
                mix[:, 0:DIM], lq_all[:, 0:DIM], lq_all[:, STF : STF + DIM],
                ALU.subtract,
            )
            nc.vector.tensor_tensor(
                mix[:, DIM:STF], lq_all[:, DIM:STF],
                lq_all[:, STF + DIM : 2 * STF], ALU.add,
            )
            sqs = sm.tile([BPC, STF], F32, tag="sqs")
            ss = sm.tile([BPC, 1], F32, tag="ss")
            nc.vector.tensor_tensor(sqs, mix, mix, ALU.mult)
            nc.vector.tensor_reduce(ss, sqs, AX.X, ALU.add)
            sd = sm.tile([BPC, 1], F32, tag="sd")
            nc.scalar.activation(sd, ss, AF.Sqrt)
            rn = sm.tile([BPC, 1], F32, tag="rn")
            nc.vector.reciprocal(rn, sd)
            nc.vector.tensor_scalar_mul(mix, mix, rn)

            # ================= qff ansatz: one 128x128 matmul =================
            mT_ps = ps_m.tile([STF, BPC], F32, tag="m")
            nc.tensor.transpose(mT_ps, mix, idn_s[0:BPC, 0:BPC])
            mT = sm.tile([STF, BPC], F16, tag="mT")
            nc.vector.tensor_copy(mT, mT_ps)
            qT_ps = ps_m.tile([STF, BPC], F32, tag="m")
            nc.tensor.matmul(qT_ps, mqt_s, mT, start=True, stop=True)
            qTs = sm.tile([STF, BPC], F32, tag="qTs")
            nc.vector.tensor_copy(qTs, qT_ps)
            q2_ps = ps_m.tile([BPC, STF], F32, tag="m")
            nc.tensor.transpose(q2_ps, qTs, idn_s)
            nc.vector.tensor_copy(mix, q2_ps)

            # ================= expvals -> qfeat [BPC, 18] =================
            scr = sm.tile([BPC, DIM], F32, tag="scr")
            scr2 = sm.tile([BPC, DIM], F32, tag="scr2")
            tmp1 = sm.tile([BPC, 1], F32, tag="tmp1")
            tmp2 = sm.tile([BPC, 1], F32, tag="tmp2")
            yr2 = sm.tile([BPC, 2], F32, tag="yr2")

            def clike(dst, off, ref):
                counts = [d[1] for d in ref.ap[1:]]
                dims = []
                stride = 1
                for c in reversed(counts):
                    dims.insert(0, [stride, c])
                    stride *= c
                return bass.AP(
                    tensor=dst.tensor, offset=dst.offset + off,
                    ap=[list(dst.ap[0])] + dims,
                )

            def prod(dst, off, a, b):
                nc.vector.tensor_tensor(clike(dst, off, a), a, b, ALU.mult)

            for i in range(NQ):
                p = 5 - i
                v = lambda ri, k: amp_view(mix, ri, {p: k})
                prod(scr, 0, v(0, 0), v(0, 1))
                prod(scr, 32, v(1, 0), v(1, 1))
                nc.vector.tensor_reduce(qfeat[:, i : i + 1], scr, AX.X, ALU.add)
                prod(scr, 0, v(0, 0), v(1, 1))
                prod(scr, 32, v(1, 0), v(0, 1))
                nc.vector.tensor_reduce(
                    yr2,
                    scr.rearrange("p (h q) -> p h q", h=2, q=32),
                    AX.X,
                    ALU.add,
                )
                nc.vector.tensor_tensor(
                    qfeat[:, 6 + i : 7 + i], yr2[:, 0:1], yr2[:, 1:2],
                    ALU.subtract,
                )
                prod(scr, 0, v(0, 0), v(0, 0))
                prod(scr, 32, v(1, 0), v(1, 0))
                prod(scr2, 0, v(0, 1), v(0, 1))
                prod(scr2, 32, v(1, 1), v(1, 1))
                nc.vector.tensor_reduce(tmp1, scr, AX.X, ALU.add)
                nc.vector.tensor_reduce(tmp2, scr2, AX.X, ALU.add)
                nc.vector.tensor_tensor(
                    qfeat[:, 12 + i : 13 + i], tmp1, tmp2, ALU.subtract
                )
            nc.vector.tensor_scalar_mul(qfeat[:, 0:12], qfeat[:, 0:12], 2.0)

            # ================= tail =================
            qfT_ps = ps_m.tile([19, BPC], F32, tag="m")
            nc.tensor.transpose(qfT_ps, qfeat, idn_s[0:BPC, 0:BPC])
            qfT = sm.tile([19, BPC], F32, tag="qfTs")
            nc.vector.tensor_copy(qfT, qfT_ps)
            o1 = ps_t.tile([BPC, D], F32, tag="t")
            nc.tensor.matmul(o1, qfT, owb_s, start=True, stop=True)

            stats = sm.tile([BPC, 6], F32, tag="stats")
            nc.vector.bn_stats(stats, o1)
            mv = sm.tile([BPC, 2], F32, tag="mv")
            nc.vector.bn_aggr(mv, stats)
            sdv = sm.tile([BPC, 1], F32, tag="sdv")
            nc.scalar.activation(sdv, mv[:, 1:2], AF.Sqrt, bias=1e-5)
            rstd = sm.tile([BPC, 1], F32, tag="rstd")
            nc.vector.reciprocal(rstd, sdv)
            ln1 = sm.tile([BPC, D], F32, tag="ln1")
            nc.vector.tensor_scalar(
                ln1, o1, mv[:, 0:1], rstd, ALU.subtract, ALU.mult
            )
            ln2 = sm.tile([BPC, D], F32, tag="ln2")
            nc.vector.tensor_tensor(ln2, ln1, lng_s, ALU.mult)
            nc.vector.tensor_tensor(ln2, ln2, lnb_s, ALU.add)

            # cls layer 1
            lnT = [None, None]
            for h in range(2):
                lnT_ps = ps_m.tile([128, BPC], F32, tag="m")
                nc.tensor.transpose(
                    lnT_ps, ln2[:, h * 128 : (h + 1) * 128], idn_s[0:BPC, 0:BPC]
                )
                lnT[h] = sm.tile([128, BPC], F32, tag=f"lnT{h}", name=f"lnT{h}")
                nc.vector.tensor_copy(lnT[h], lnT_ps)
            h2p = ps_t.tile([BPC, D], F32, tag="t")
            nc.tensor.matmul(h2p, lnT[0], cw1_s[:, 0:D], start=True, stop=False)
            nc.tensor.matmul(
                h2p, lnT[1], cw1_s[:, D : 2 * D], start=False, stop=False
            )
            nc.tensor.matmul(
                h2p, ones[:, 0:BPC], cb1_s, start=False, stop=True
            )
            h2 = sm.tile([BPC, D], F32, tag="h2")
            nc.scalar.activation(h2, h2p, AF.Relu)

            # cls layer 2
            h2T = [None, None]
            for h in range(2):
                h2T_ps = ps_m.tile([128, BPC], F32, tag="m")
                nc.tensor.transpose(
                    h2T_ps, h2[:, h * 128 : (h + 1) * 128], idn_s[0:BPC, 0:BPC]
                )
                h2T[h] = sm.tile([128, BPC], F32, tag=f"h2T{h}", name=f"h2T{h}")
                nc.vector.tensor_copy(h2T[h], h2T_ps)
            lg = ps_t.tile([BPC, 2], F32, tag="t")
            nc.tensor.matmul(lg, h2T[0], cw2_s[:, 0:2], start=True, stop=False)
            nc.tensor.matmul(lg, h2T[1], cw2_s[:, 2:4], start=False, stop=False)
            nc.tensor.matmul(lg, ones[:, 0:BPC], cb2_s, start=False, stop=True)
            lgs = sm.tile([BPC, 2], F32, tag="lgs")
            nc.vector.tensor_copy(lgs, lg)
            nc.sync.dma_start(out=out[:, :], in_=lgs)

    if split_waits:
        _split_multi_waits(nc)
    return nc


_NC_CACHE = {}


def _get_program():
    if "nc" not in _NC_CACHE:
        _NC_CACHE["nc"] = build_program()
    return _NC_CACHE["nc"]


def _qff_matrix(qp):
    """Compose the 30 shared-parameter qff gates into one 64x64 complex matrix."""
    U = np.eye(DIM, dtype=np.complex128)
    for kind, loc, j in ansatz_gates(1):
        th = float(qp[j])
        c, s = np.cos(th / 2), np.sin(th / 2)
        G = np.zeros((DIM, DIM), np.complex128)
        if kind == "crx":
            wc, wt = loc
            bc, bt = 5 - wc, 5 - wt
            for k in range(DIM):
                if (k >> bc) & 1:
                    G[k, k] = c
                    G[k, k ^ (1 << bt)] = -1j * s
                else:
                    G[k, k] = 1.0
        else:
            bq = 5 - loc
            for k in range(DIM):
                kb = (k >> bq) & 1
                if kind == "rx":
                    G[k, k] = c
                    G[k, k ^ (1 << bq)] = -1j * s
                elif kind == "ry":
                    G[k, k] = c
                    G[k, k ^ (1 << bq)] = -s if kb == 0 else s
                else:  # rz
                    G[k, k] = np.exp(-0.5j * th) if kb == 0 else np.exp(0.5j * th)
        U = G @ U
    return U


def host_prep(inputs):
    """Host-side parameter folding -> per-core input maps."""
    f32 = np.float32
    x = np.asarray(inputs["x"], f32)
    emb_w = np.asarray(inputs["emb_w"], np.float64)
    emb_b = np.asarray(inputs["emb_b"], np.float64)
    att_w1 = np.asarray(inputs["att_w1"], np.float64)
    att_b1 = np.asarray(inputs["att_b1"], np.float64)

    f16 = np.float16
    wfold = (emb_w @ att_w1).astype(f16)
    bfold = (emb_b @ att_w1 + att_b1).astype(f16)
    wfb = np.concatenate([wfold, bfold[None, :]], 0)

    ewb = np.concatenate(
        [emb_w.astype(f16), emb_b.astype(f16)[None, :]], 0
    )

    pw = np.asarray(inputs["proj_w"], f16)
    pjw = np.concatenate([pw[0:128, :], pw[128:256, :]], 1)

    cr = np.asarray(inputs["mix_re"], np.float64)
    ci = np.asarray(inputs["mix_im"], np.float64)
    den = np.sqrt(cr * cr + ci * ci).sum() + 1e-8
    cf2 = np.stack([cr / den, ci / den], 1).astype(np.float16)

    qp = np.asarray(inputs["qff_params"], np.float64)
    U = _qff_matrix(qp)
    M = np.block([[U.real, -U.imag], [U.imag, U.real]])
    mqt = np.ascontiguousarray(M.T).astype(np.float16)

    owb = np.concatenate(
        [np.asarray(inputs["out_w"], f32), np.asarray(inputs["out_b"], f32)[None, :]],
        0,
    )
    lng = np.broadcast_to(np.asarray(inputs["ln_g"], f32), (BPC, D)).copy()
    lnb = np.broadcast_to(np.asarray(inputs["ln_b"], f32), (BPC, D)).copy()
    w1 = np.asarray(inputs["cls_w1"], f32)
    cw1 = np.concatenate([w1[0:128, :], w1[128:256, :]], 1)
    cb1 = np.asarray(inputs["cls_b1"], f32)[None, :]
    w2 = np.asarray(inputs["cls_w2"], f32)
    cw2 = np.concatenate([w2[0:128, :], w2[128:256, :]], 1)
    cb2 = np.asarray(inputs["cls_b2"], f32)[None, :]
    idn = np.eye(128, dtype=f32)
    pjb = np.asarray(inputs["proj_b"], f16)[None, :]

    shared = dict(
        wfb=wfb, aw2=np.asarray(inputs["att_w2"], f16), ewb=ewb, pjw=pjw,
        pjb=pjb, cf2=cf2, mqt=mqt, owb=owb, lng=lng,
        lnb=lnb, cw1=cw1, cb1=cb1, cw2=cw2, cb2=cb2, idn=idn,
    )

    x16 = x.astype(f16)
    in_maps = []
    for c in range(N_CORES):
        xc = x16[c * BPC : (c + 1) * BPC]
        # xp[b, nc, c*16+k] = x[b, c, nc*16+k]  (c-major, k inner)
        xp_c = np.ascontiguousarray(
            xc.reshape(BPC, C_IN, NC, CH).transpose(0, 2, 1, 3).reshape(
                BPC, NC, CH * C_IN
            )
        )
        m = dict(shared)
        m["xs"] = np.ascontiguousarray(xc)
        m["xp"] = xp_c
        in_maps.append(m)
    return in_maps


def kernel(**inputs):
    nc = _get_program()
    in_maps = host_prep(inputs)
    res = run_bass_kernel_spmd(nc, in_maps, core_ids=list(range(N_CORES)))
    outs = [res.results[c]["out"] for c in range(N_CORES)]
    return np.concatenate(outs, 0).astype(np.float32)


if __name__ == "__main__":
    nc = build_program()
    print("program built ok")


# revision 30
# speedup vs baseline: 2.8640x; 1.1292x over previous
"""Trainium2 Bass kernel for nn_ClassicalQuantumAttention.

Data-parallel over batch: 128 batch elems -> 16 per NeuronCore x 8 cores.

Per-core pipeline:
  classical   : scores path (PE matmuls + ACT tanh + softmax) and chunk path
                (weighted chunk sums, emb/proj matmuls) as in the baseline;
                circuit params sigmoid+sin/cos written as fp16 into SHARED
                coefficient tiles co/si/ns [128 chunks, 60 params x 16 b].
  quantum     : ALL 16 batch elems' statevectors in ONE fp16 tile
                ST [128 part = chunk, free = ri(2) x amp(64) x b(16)],
                b innermost.  Each gate = ~5 large tensor_tensor ops
                (FD 512-2048, fp16 2x mode) with per-(chunk,b) cos/sin
                applied via stride-0 broadcast views.  Layer-1 rotations
                use sparse (support-restricted) views.
  LCU         : per-b matmuls over chunk partitions (as baseline), then
                normalize on [16, 128].
  qff ansatz  : the 30 shared-parameter gates are ONE host-precomputed
                128x128 real matrix; applied by PE transpose + matmul.
  tail        : expvals (DVE quadratic forms), out head + layernorm +
                classifier (PE + small ops), as baseline.
"""

import numpy as np
import sys

for _p in ("/opt/trn_rl_repo",):
    if _p not in sys.path:
        sys.path.insert(0, _p)

import concourse.bass as bass
import concourse.tile as tile
from concourse import mybir
from concourse.bass_utils import run_bass_kernel_spmd

F32 = mybir.dt.float32
F16 = mybir.dt.float16
ALU = mybir.AluOpType
AF = mybir.ActivationFunctionType
AX = mybir.AxisListType

N_CORES = 8
B_TOT = 128
BPC = B_TOT // N_CORES  # 16 batch elems per core
C_IN = 64
T = 2048
D = 256
CH = 16
NC = T // CH  # 128 chunks
NQ = 6
DIM = 64  # 2**6 amplitudes
STF = 2 * DIM  # 128 floats per state ([64 re | 64 im])

# big-state free layout: idx = ri*1024 + amp*16 + b
SB = BPC          # 16 (b inner)
SAMP = DIM * SB   # 1024 (one ri slab)
SFREE = 2 * SAMP  # 2048


# ---------------------------------------------------------------- gate list
def ansatz_gates(n_layers):
    """[(kind, wire-or-(ctrl,tgt), param_idx)] matching reference _ansatz."""
    gates = []
    idx = 0
    for _ in range(n_layers):
        for i in range(NQ):
            gates.append(("rx", i, idx))
            gates.append(("ry", i, idx + 1))
            gates.append(("rz", i, idx + 2))
            idx += 3
        for i in range(NQ):
            gates.append(("crx", (i, (i + 1) % NQ), idx))
            idx += 1
        for i in range(NQ - 1, -1, -1):
            gates.append(("crx", (i, (i - 1) % NQ), idx))
            idx += 1
    return gates


# ------------------------------------------------------------- AP helpers
def fview(t, dims, off):
    return bass.AP(tensor=t.tensor, offset=t.offset + off, ap=[list(t.ap[0])] + dims)


def v_full(t, ri=None, w=6):
    """All involved amps (support width w: amps {k*2^(6-w)}), b inner.

    ri None: both ri slabs merged into the outer dim."""
    p = 6 - w
    step = (1 << p) * SB
    n = 1 << w
    if ri is None:
        return fview(t, [[step, 2 * n], [1, SB]], 0)
    return fview(t, [[step, n], [1, SB]], ri * SAMP)


def v_bit(t, p, val, ri=None, w=6):
    """Amps with bit p fixed to val; support width w (w<6 implies p == 6-w,
    lower bits all zero)."""
    off = val * (1 << p) * SB + (0 if ri is None else ri * SAMP)
    if w == 6:
        step_hi = (1 << (p + 1)) * SB
        n_hi = 1 << (5 - p)
        inner = (1 << p) * SB
        if ri is None:
            return fview(t, [[step_hi, 2 * n_hi], [1, inner]], off)
        return fview(t, [[step_hi, n_hi], [1, inner]], off)
    assert p == 6 - w
    step = (1 << (p + 1)) * SB
    n = 1 << (w - 1)
    if ri is None:
        return fview(t, [[step, 2 * n], [1, SB]], off)
    return fview(t, [[step, n], [1, SB]], off)


def v_2bit(t, ph, pl, vh, vl):
    """Both-ri view fixing adjacent amp bits ph = pl+1."""
    assert ph == pl + 1
    step_hi = (1 << (ph + 1)) * SB
    n_hi = 1 << (5 - ph)
    inner = (1 << pl) * SB
    off = (vh * (1 << ph) + vl * (1 << pl)) * SB
    return fview(t, [[step_hi, 2 * n_hi], [1, inner]], off)


def v_2bit_wrap(t, v5, v0, ri):
    """Per-ri view fixing amp bits 5 and 0 (the non-adjacent wrap case)."""
    off = ri * SAMP + (v5 * 32 + v0) * SB
    return fview(t, [[2 * SB, 16], [1, SB]], off)


def cview(ct, j, n):
    """Coefficient view for param j: [128, [0,n],[1,16]] (b inner)."""
    return bass.AP(
        tensor=ct.tensor, offset=ct.offset + SB * j,
        ap=[list(ct.ap[0]), [0, n], [1, SB]],
    )


# ------------------------------------------------------------ gate emitters
def v_ctrl(t, pc, ri):
    """Per-ri view of amps with bit pc = 1, when they form a single run
    (pc == 5: contiguous upper half; pc == 0: stride-2 odd amps)."""
    if pc == 5:
        return fview(t, [[SB, 32], [1, SB]], ri * SAMP + 32 * SB)
    assert pc == 0
    return fview(t, [[2 * SB, 32], [1, SB]], ri * SAMP + SB)


def emit_big_ansatz(nc, ST, B, B2, co, si, ns, cm1, ta, nta, gates, sparse_first):
    """Tangent-space rotations: ST here is ST_true / prod(cos of rotations).
    Caller must multiply by the cos product afterwards."""
    tt = nc.vector.tensor_tensor

    def rot(kind, p, j, w):
        n1 = 1 << w        # outer count of per-ri involved view
        n2 = 2 * n1        # both-ri
        if kind == "ry":
            # B = t*ST (no ri swap); ST[p0] -= B[p1]; ST[p1] += B[p0]
            tt(v_full(B, None, w), v_full(ST, None, w), cview(ta, j, n2), ALU.mult)
            tt(v_bit(ST, p, 0, None, w), v_bit(ST, p, 0, None, w),
               v_bit(B, p, 1, None, w), ALU.subtract)
            tt(v_bit(ST, p, 1, None, w), v_bit(ST, p, 1, None, w),
               v_bit(B, p, 0, None, w), ALU.add)
            return
        # rx / rz: B[re] = t*ST[im]; B[im] = -t*ST[re]
        tt(v_full(B, 0, w), v_full(ST, 1, w), cview(ta, j, n1), ALU.mult)
        tt(v_full(B, 1, w), v_full(ST, 0, w), cview(nta, j, n1), ALU.mult)
        if kind == "rx":
            # ST[p0] += B[p1]; ST[p1] += B[p0]
            tt(v_bit(ST, p, 0, None, w), v_bit(ST, p, 0, None, w),
               v_bit(B, p, 1, None, w), ALU.add)
            tt(v_bit(ST, p, 1, None, w), v_bit(ST, p, 1, None, w),
               v_bit(B, p, 0, None, w), ALU.add)
        else:  # rz: ST[p0] += B[p0]; ST[p1] -= B[p1]
            tt(v_bit(ST, p, 0, None, w), v_bit(ST, p, 0, None, w),
               v_bit(B, p, 0, None, w), ALU.add)
            tt(v_bit(ST, p, 1, None, w), v_bit(ST, p, 1, None, w),
               v_bit(B, p, 1, None, w), ALU.subtract)

    def crx_edge(pc, pt, j):
        # pc in {0, 5}: control-1 amps form a single run -> all ops restricted
        tt(v_ctrl(B, pc, 0), v_ctrl(ST, pc, 1), cview(si, j, 32), ALU.mult)
        tt(v_ctrl(B, pc, 1), v_ctrl(ST, pc, 0), cview(ns, j, 32), ALU.mult)
        if pc == 0:
            # both-ri scale merges (stride-2 run spans the ri boundary)
            v = fview(ST, [[2 * SB, 64], [1, SB]], SB)
            tt(v, v, cview(co, j, 64), ALU.mult)
        else:
            for ri in (0, 1):
                tt(v_ctrl(ST, pc, ri), v_ctrl(ST, pc, ri),
                   cview(co, j, 32), ALU.mult)
        if abs(pc - pt) == 1:  # (5,4) or (0,1)
            ph, pl = max(pc, pt), min(pc, pt)
            for k in (0, 1):
                if pc == ph:
                    o, i1 = v_2bit(ST, ph, pl, 1, k), v_2bit(B, ph, pl, 1, 1 - k)
                else:
                    o, i1 = v_2bit(ST, ph, pl, k, 1), v_2bit(B, ph, pl, 1 - k, 1)
                tt(o, o, i1, ALU.add)
        else:  # wrap: (5,0) or (0,5)
            for k in (0, 1):
                for ri in (0, 1):
                    if pc == 0:
                        o, i1 = v_2bit_wrap(ST, k, 1, ri), v_2bit_wrap(B, 1 - k, 1, ri)
                    else:
                        o, i1 = v_2bit_wrap(ST, 1, k, ri), v_2bit_wrap(B, 1, 1 - k, ri)
                    tt(o, o, i1, ALU.add)

    def crx(pc, pt, j):
        if pc in (0, 5):
            crx_edge(pc, pt, j)
            return
        # B[re] = s*ST[im]; B[im] = -s*ST[re]; B2 = (c-1)*ST
        tt(v_full(B, 0), v_full(ST, 1), cview(si, j, 64), ALU.mult)
        tt(v_full(B, 1), v_full(ST, 0), cview(ns, j, 64), ALU.mult)
        tt(v_full(B2, None), v_full(ST, None), cview(cm1, j, 128), ALU.mult)
        # ST[pc=1] += B2[pc=1]   (-> c*ST on the control-1 half)
        tt(v_bit(ST, pc, 1), v_bit(ST, pc, 1), v_bit(B2, pc, 1), ALU.add)
        # ST[pc=1, pt=k] += B[pc=1, pt=1-k]
        ph, pl = max(pc, pt), min(pc, pt)
        assert ph == pl + 1
        for k in (0, 1):
            if pc == ph:
                o, i1 = v_2bit(ST, ph, pl, 1, k), v_2bit(B, ph, pl, 1, 1 - k)
            else:
                o, i1 = v_2bit(ST, ph, pl, k, 1), v_2bit(B, ph, pl, 1 - k, 1)
            tt(o, o, i1, ALU.add)

    for gi, (kind, loc, j) in enumerate(gates):
        if kind == "crx":
            crx(5 - loc[0], 5 - loc[1], j)
        else:
            w = (loc + 1) if (sparse_first and gi < 3 * NQ) else 6
            rot(kind, 5 - loc, j, w)


# --------------------------------------------- baseline amp_view (tail use)
def amp_view(t, ri, fixed, swap_p=None, split_ps=()):
    """Strided view of a statevector AP t ([P, 128] = [P, (ri, amp6bits)])."""
    part = t.ap[0]
    offset = t.offset
    dims = []
    if ri is None:
        dims.append([DIM, 2])
    else:
        offset += ri * DIM
    run = None
    for p in range(5, -1, -1):
        if p in fixed:
            if run is not None:
                dims.append(run)
                run = None
            offset += fixed[p] << p
        elif swap_p == p:
            if run is not None:
                dims.append(run)
                run = None
            dims.append([-(1 << p), 2])
            offset += 1 << p
        elif p in split_ps:
            if run is not None:
                dims.append(run)
                run = None
            dims.append([1 << p, 2])
        else:
            if run is None:
                run = [1 << p, 2]
            else:
                run = [1 << p, run[1] * 2]
    if run is not None:
        dims.append(run)
    if not dims:
        dims.append([1, 1])
    assert len(dims) <= 2, f"too many free dims: {dims}"
    return bass.AP(tensor=t.tensor, offset=offset, ap=[list(part)] + dims)


def _split_multi_waits(nc):
    """This walrus build allows at most ONE sync-wait per instruction."""
    ctr = [0]
    for f in nc.m.functions:
        for b in f.blocks:
            new = []
            for inst in b.instructions:
                si = inst.sync_info
                if si is not None and len(si.on_wait) > 1:
                    waits = list(si.on_wait)
                    for w in waits[:-1]:
                        ctr[0] += 1
                        nop = mybir.InstNoOp(
                            name=f"wsplit-{ctr[0]}",
                            ins=[],
                            outs=[],
                            engine=inst.engine,
                            sync_info=mybir.SyncInfo(on_wait=[w], on_update=[]),
                        )
                        new.append(nop)
                    inst.sync_info = mybir.SyncInfo(
                        on_wait=[waits[-1]], on_update=list(si.on_update)
                    )
                new.append(inst)
            b.instructions = new


# ---------------------------------------------------------------- program
def build_program(split_waits=True):
    nc = bass.Bass()

    for v in (float(np.pi / 2), 1e-5, -1.0):
        t = nc.alloc_sbuf_tensor(f"const-f32-{v}", [128, 1], F32)
        nc.gpsimd.memset(t.ap(), v)
        nc.const_aps.aps[(F32, v)] = t.ap()
    nc.all_engine_barrier()

    # ---- dram I/O (per core) ----
    xs = nc.declare_dram_parameter("xs", [BPC, C_IN, T], F16, isOutput=False)
    xp = nc.declare_dram_parameter("xp", [BPC, NC, CH * C_IN], F16, isOutput=False)
    wfb = nc.declare_dram_parameter("wfb", [C_IN, 128], F16, isOutput=False)
    aw2 = nc.declare_dram_parameter("aw2", [128, 1], F16, isOutput=False)
    ewb = nc.declare_dram_parameter("ewb", [C_IN + 1, D], F16, isOutput=False)
    pjw = nc.declare_dram_parameter("pjw", [128, 120], F16, isOutput=False)
    pjb = nc.declare_dram_parameter("pjb", [1, 60], F16, isOutput=False)
    bfold = nc.declare_dram_parameter("bfold", [128, 1], F32, isOutput=False)
    cf2 = nc.declare_dram_parameter("cf2", [NC, 2], F16, isOutput=False)
    aob = nc.declare_dram_parameter("aob", [STF, 18 * STF], F16, isOutput=False)
    owb = nc.declare_dram_parameter("owb", [19, D], F32, isOutput=False)
    lng = nc.declare_dram_parameter("lng", [BPC, D], F32, isOutput=False)
    lnb = nc.declare_dram_parameter("lnb", [BPC, D], F32, isOutput=False)
    cw1 = nc.declare_dram_parameter("cw1", [128, 2 * D], F32, isOutput=False)
    cb1 = nc.declare_dram_parameter("cb1", [1, D], F32, isOutput=False)
    cw2 = nc.declare_dram_parameter("cw2", [128, 4], F32, isOutput=False)
    cb2 = nc.declare_dram_parameter("cb2", [1, 2], F32, isOutput=False)
    idn = nc.declare_dram_parameter("idn", [128, 128], F32, isOutput=False)
    out = nc.declare_dram_parameter("out", [BPC, 2], F32, isOutput=True)

    with tile.TileContext(nc) as tc:
        with (
            tc.tile_pool(name="const", bufs=1) as cp,
            tc.tile_pool(name="xbuf", bufs=2) as xpool,
            tc.tile_pool(name="xpbuf", bufs=2) as xppool,
            tc.tile_pool(name="tanh", bufs=2) as thpool,
            tc.tile_pool(name="small", bufs=4) as sm,
            tc.tile_pool(name="ps_h", bufs=2, space="PSUM") as ps_h,
            tc.tile_pool(name="ps_s", bufs=2, space="PSUM") as ps_s,
            tc.tile_pool(name="ps_m", bufs=2, space="PSUM") as ps_m,
            tc.tile_pool(name="ps_t", bufs=2, space="PSUM") as ps_t,
        ):
            # ---------------- constants into SBUF ----------------
            def cload(name, dram, shape, dt=F32):
                t = cp.tile(shape, dt, tag=name, name=name)
                nc.sync.dma_start(out=t, in_=dram[:, :])
                return t

            wfb_s = cload("wfb", wfb, [C_IN, 128], F16)
            aw2_s = cload("aw2", aw2, [128, 1], F16)
            ewb_s = cload("ewb", ewb, [C_IN + 1, D], F16)
            pjw_s = cload("pjw", pjw, [128, 120], F16)
            pjb_s = cload("pjb", pjb, [1, 60], F16)
            bfold_s = cload("bfold", bfold, [128, 1])
            cf2_s = cload("cf2", cf2, [NC, 2], F16)
            aob_s = cload("aob", aob, [STF, 18 * STF], F16)
            owb_s = cload("owb", owb, [19, D])
            lng_s = cload("lng", lng, [BPC, D])
            lnb_s = cload("lnb", lnb, [BPC, D])
            cw1_s = cload("cw1", cw1, [128, 2 * D])
            cb1_s = cload("cb1", cb1, [1, D])
            cw2_s = cload("cw2", cw2, [128, 4])
            cb2_s = cload("cb2", cb2, [1, 2])
            idn_s = cload("idn", idn, [128, 128])

            ones = cp.tile([1, 128], F32, tag="ones")
            nc.vector.memset(ones, 1.0)
            ones16 = cp.tile([1, 128], F16, tag="ones16")
            nc.vector.memset(ones16, 1.0)

            # persistent per-group score tiles
            sc_g = [cp.tile([NC, 8 * CH], F32, tag=f"scg{g}", name=f"scg{g}") for g in range(2)]
            esc_g = [cp.tile([NC, 8 * CH], F32, tag=f"escg{g}", name=f"escg{g}") for g in range(2)]
            w_g = [cp.tile([NC, 8 * CH], F16, tag=f"wg{g}", name=f"wg{g}") for g in range(2)]

            # shared fp16 coefficient tiles: free = param_j*16 + b
            co_t = cp.tile([NC, 60 * SB], F16, tag="co", name="co")
            si_t = cp.tile([NC, 60 * SB], F16, tag="si", name="si")
            ns_t = cp.tile([NC, 60 * SB], F16, tag="ns", name="ns")
            cm1_t = cp.tile([NC, 60 * SB], F16, tag="cm1", name="cm1")
            ta_t = cp.tile([NC, 60 * SB], F16, tag="ta", name="ta")
            nta_t = cp.tile([NC, 60 * SB], F16, tag="nta", name="nta")
            ctot = cp.tile([NC, 60 * SB], F32, tag="ctot", name="ctot")

            # big state + scratch tiles
            ST = cp.tile([NC, SFREE], F16, tag="ST", name="ST")
            Bt = cp.tile([NC, SFREE], F16, tag="Bt", name="Bt")
            B2t = cp.tile([NC, SFREE], F16, tag="B2t", name="B2t")

            # per-b double buffers
            x_sb = [xpool.tile([C_IN, T], F16, tag="x", name=f"xsb{i}") for i in range(2)]
            xp_sb = [xppool.tile([NC, CH * C_IN], F16, tag="xp", name=f"xpsb{i}") for i in range(2)]
            xwt_sb = [xppool.tile([C_IN + 1, NC], F16, tag="xwt", name=f"xwtsb{i}") for i in range(2)]
            for i in range(2):
                nc.vector.memset(xwt_sb[i][C_IN : C_IN + 1, :], 1.0)

            # staged sigmoid inputs: free = param_j*16 + b (for batched ACT)
            theta_all = cp.tile([NC, 60 * SB], F32, tag="theta", name="theta")

            lq_all = cp.tile([BPC, 2 * STF], F32, tag="lqall")
            mix = cp.tile([BPC, STF], F32, tag="mix")
            qfeat = cp.tile([BPC, 19], F32, tag="qfeat")
            nc.vector.memset(qfeat[:, 18:19], 1.0)

            # ================= classical per-b =================
            for b in range(BPC):
                xb = x_sb[b % 2]
                nc.sync.dma_start(out=xb, in_=xs[b, :, :])

                th = thpool.tile([128, T], F16, tag="th")
                for blk in range(4):
                    hp = ps_h.tile([128, 512], F32, tag="hp")
                    nc.tensor.matmul(
                        hp,
                        wfb_s,
                        xb[:, blk * 512 : (blk + 1) * 512],
                        start=True,
                        stop=True,
                    )
                    nc.scalar.activation(
                        th[:, blk * 512 : (blk + 1) * 512], hp, AF.Tanh,
                        bias=bfold_s,
                    )
                    sc = ps_s.tile([1, 512], F32, tag="sc")
                    nc.tensor.matmul(
                        sc,
                        aw2_s,
                        th[:, blk * 512 : (blk + 1) * 512],
                        start=True,
                        stop=True,
                    )
                    ssc = sm.tile([1, 512], F32, tag="ssc", name="ssc")
                    if blk % 2 == 0:
                        nc.vector.tensor_copy(ssc, sc)
                    else:
                        nc.scalar.copy(ssc, sc)
                    g, bb = b // 8, b % 8
                    src = ssc.rearrange("p (n k) -> p n k", n=32, k=CH)
                    dst = sc_g[g][blk * 32 : (blk + 1) * 32, bb * CH : (bb + 1) * CH]
                    nc.sync.dma_start(out=dst, in_=src)

                # ---- group softmax + per-b chunk path, after each group of 8
                if b % 8 == 7:
                    g = b // 8
                    nc.scalar.activation(esc_g[g], sc_g[g], AF.Exp)
                    ssum = sm.tile([NC, 8], F32, tag="ssum")
                    nc.vector.tensor_reduce(
                        ssum,
                        esc_g[g].rearrange("p (n k) -> p n k", n=8, k=CH),
                        AX.X,
                        ALU.add,
                    )
                    rsum = sm.tile([NC, 8], F32, tag="rsum")
                    nc.vector.reciprocal(rsum, ssum)
                    for bb in range(8):
                        nc.vector.tensor_scalar_mul(
                            w_g[g][:, bb * CH : (bb + 1) * CH],
                            esc_g[g][:, bb * CH : (bb + 1) * CH],
                            rsum[:, bb : bb + 1],
                        )

                    for bb in range(8):
                        bfull = g * 8 + bb
                        xpb = xp_sb[bfull % 2]
                        nc.sync.dma_start(out=xpb, in_=xp[bfull, :, :])
                        # xw[nc, c] = sum_k w[nc, k] * xpb[nc, c*16+k]
                        xwp = sm.tile([NC, CH * C_IN], F16, tag="xwp")
                        wv = bass.AP(
                            tensor=w_g[g].tensor,
                            offset=w_g[g].offset + bb * CH,
                            ap=[list(w_g[g].ap[0]), [0, C_IN], [1, CH]],
                        )
                        xv = fview(xpb, [[CH, C_IN], [1, CH]], 0)
                        ov = fview(xwp, [[CH, C_IN], [1, CH]], 0)
                        nc.vector.tensor_tensor(ov, xv, wv, ALU.mult)
                        xw = sm.tile([NC, C_IN], F32, tag="xw")
                        nc.vector.tensor_reduce(
                            xw,
                            xwp.rearrange("p (c k) -> p c k", c=C_IN, k=CH),
                            AX.X,
                            ALU.add,
                        )
                        xwt_ps = ps_m.tile([C_IN, NC], F32, tag="m")
                        nc.tensor.transpose(xwt_ps, xw, idn_s)
                        xwt = xwt_sb[bfull % 2]
                        nc.vector.tensor_copy(xwt[0:C_IN, :], xwt_ps)
                        cht = [None, None]
                        for h in range(2):
                            chp = ps_m.tile([128, NC], F32, tag="m")
                            nc.tensor.matmul(
                                chp,
                                ewb_s[:, h * 128 : (h + 1) * 128],
                                xwt,
                                start=True,
                                stop=True,
                            )
                            cht[h] = sm.tile([128, NC], F16, tag=f"cht{h}", name=f"cht{h}")
                            nc.vector.tensor_copy(cht[h], chp)
                        par = ps_t.tile([NC, 60], F32, tag="t")
                        nc.tensor.matmul(
                            par, cht[0], pjw_s[:, 0:60], start=True, stop=False
                        )
                        nc.tensor.matmul(
                            par, cht[1], pjw_s[:, 60:120], start=False, stop=False
                        )
                        nc.tensor.matmul(
                            par, ones16, pjb_s, start=False, stop=True
                        )
                        # stage sigmoid input into strided (j*16+b) slots
                        nc.scalar.copy(fview(theta_all, [[SB, 60]], bfull), par)

            # batched: theta = sigmoid(z); cos/sin/negsin/cos-1 (fp16)
            nc.scalar.activation(theta_all, theta_all, AF.Sigmoid)
            nc.scalar.activation(
                co_t, theta_all, AF.Sin, bias=float(np.pi / 2), scale=0.5
            )
            nc.scalar.activation(si_t, theta_all, AF.Sin, bias=0.0, scale=0.5)
            nc.scalar.activation(ns_t, theta_all, AF.Sin, bias=0.0, scale=-0.5)
            nc.scalar.activation(cm1_t, co_t, AF.Copy, bias=-1.0)

            # tangent coefficients: ta = si/co, nta = -ta  (via fp32 recip)
            t32a = cp.tile([NC, 60 * SB], F32, tag="t32a", name="t32a")
            t32b = cp.tile([NC, 60 * SB], F32, tag="t32b", name="t32b")
            nc.scalar.activation(
                t32a, theta_all, AF.Sin, bias=float(np.pi / 2), scale=0.5
            )  # cos32
            nc.vector.reciprocal(t32b, t32a)
            # cos product tree seed (uses fp32 cos before it is overwritten)
            nc.vector.tensor_tensor(
                ctot[:, 0:288], t32a[:, 0:288], t32a[:, 480:768], ALU.mult
            )
            nc.scalar.activation(t32a, theta_all, AF.Sin, bias=0.0, scale=0.5)
            nc.vector.tensor_tensor(ta_t, t32a, t32b, ALU.mult)
            nc.vector.tensor_scalar_mul(nta_t, ta_t, -1.0)

            # ================= quantum stage 1 (b-batched, tangent space) ===
            nc.vector.memset(ST, 0.0)
            nc.vector.memset(fview(ST, [[1, SB]], 0), 1.0)  # amp0, re, all b

            emit_big_ansatz(
                nc, ST, Bt, B2t, co_t, si_t, ns_t, cm1_t, ta_t, nta_t,
                ansatz_gates(2), sparse_first=True,
            )

            # cos product over the 36 rotation params (seed done above)
            nc.vector.tensor_tensor(
                ctot[:, 0:144], ctot[:, 0:144], ctot[:, 144:288], ALU.mult
            )
            nc.vector.tensor_tensor(
                ctot[:, 0:64], ctot[:, 0:64], ctot[:, 64:128], ALU.mult
            )
            nc.vector.tensor_tensor(
                ctot[:, 0:32], ctot[:, 0:32], ctot[:, 32:64], ALU.mult
            )
            nc.vector.tensor_tensor(
                ctot[:, 0:16], ctot[:, 0:16], ctot[:, 16:32], ALU.mult
            )
            nc.vector.tensor_tensor(
                ctot[:, 0:16], ctot[:, 0:16], ctot[:, 128:144], ALU.mult
            )
            ctot16 = sm.tile([NC, SB], F16, tag="ctot16")
            nc.vector.tensor_copy(ctot16, ctot[:, 0:16])
            nc.vector.tensor_tensor(
                v_full(ST, None, 6), v_full(ST, None, 6),
                cview(ctot16, 0, 128), ALU.mult,
            )

            # ---- LCU: per-b matmuls over chunk partitions ----
            for b in range(BPC):
                rhs_all = fview(ST, [[SB, STF]], b)
                r0 = ps_t.tile([1, STF], F32, tag="t")
                nc.tensor.matmul(r0, cf2_s[:, 0:1], rhs_all, start=True, stop=True)
                r1 = ps_t.tile([1, STF], F32, tag="t")
                nc.tensor.matmul(r1, cf2_s[:, 1:2], rhs_all, start=True, stop=True)
                lst = sm.tile([1, 2 * STF], F32, tag="lst", name="lst")
                nc.scalar.copy(lst[:, 0:STF], r0)
                nc.vector.tensor_copy(lst[:, STF : 2 * STF], r1)
                nc.sync.dma_start(out=lq_all[b : b + 1, :], in_=lst)

            # mixed_re = r0_re - r1_im ; mixed_im = r0_im + r1_re
            nc.vector.tensor_tensor(
                mix[:, 0:DIM], lq_all[:, 0:DIM],
                lq_all[:, STF + DIM : 2 * STF], ALU.subtract,
            )
            nc.vector.tensor_tensor(
                mix[:, DIM:STF], lq_all[:, DIM:STF],
                lq_all[:, STF : STF + DIM], ALU.add,
            )
            # squared norm and 1/n^2 (normalization folded into qfeat scale)
            sqs = sm.tile([BPC, STF], F32, tag="sqs")
            ss = sm.tile([BPC, 1], F32, tag="ss")
            nc.vector.tensor_tensor(sqs, mix, mix, ALU.mult)
            nc.vector.tensor_reduce(ss, sqs, AX.X, ALU.add)
            rn2 = sm.tile([BPC, 1], F32, tag="rn2")
            nc.vector.reciprocal(rn2, ss)

            # ============ expvals via PE: qfeat_o = mix^T (M^T A_o M) mix ====
            mT_ps = ps_m.tile([STF, BPC], F32, tag="m")
            nc.tensor.transpose(mT_ps, mix, idn_s[0:BPC, 0:BPC])
            mixh = sm.tile([STF, BPC], F16, tag="mixh")
            nc.vector.tensor_copy(mixh, mT_ps)
            mixT32 = sm.tile([STF, BPC], F32, tag="mixT32")
            nc.scalar.copy(mixT32, mT_ps)
            pstack = cp.tile([NC, 18 * BPC], F32, tag="pstack")
            for o in range(18):
                T_ps = ps_m.tile([STF, BPC], F32, tag="m")
                nc.tensor.matmul(
                    T_ps, aob_s[:, o * STF : (o + 1) * STF], mixh,
                    start=True, stop=True,
                )
                nc.vector.tensor_tensor(
                    fview(pstack, [[18, BPC]], o), mixT32, T_ps, ALU.mult
                )
            ones_col = cp.tile([128, 1], F32, tag="ones_col")
            nc.vector.memset(ones_col, 1.0)
            qrow_ps = ps_t.tile([1, 18 * BPC], F32, tag="t")
            nc.tensor.matmul(qrow_ps, ones_col, pstack, start=True, stop=True)
            qrow = sm.tile([1, 18 * BPC], F32, tag="qrow")
            nc.scalar.copy(qrow, qrow_ps)
            qf01 = sm.tile([BPC, 18], F32, tag="qf01")
            nc.sync.dma_start(
                out=qf01, in_=qrow.rearrange("p (b o) -> p b o", b=BPC, o=18)
            )
            nc.vector.tensor_scalar_mul(qfeat[:, 0:18], qf01, rn2)

            # ================= tail =================
            qfT_ps = ps_m.tile([19, BPC], F32, tag="m")
            nc.tensor.transpose(qfT_ps, qfeat, idn_s[0:BPC, 0:BPC])
            qfT = sm.tile([19, BPC], F32, tag="qfTs")
            nc.vector.tensor_copy(qfT, qfT_ps)
            o1 = ps_t.tile([BPC, D], F32, tag="t")
            nc.tensor.matmul(o1, qfT, owb_s, start=True, stop=True)

            stats = sm.tile([BPC, 6], F32, tag="stats")
            nc.vector.bn_stats(stats, o1)
            mv = sm.tile([BPC, 2], F32, tag="mv")
            nc.vector.bn_aggr(mv, stats)
            sdv = sm.tile([BPC, 1], F32, tag="sdv")
            nc.scalar.activation(sdv, mv[:, 1:2], AF.Sqrt, bias=1e-5)
            rstd = sm.tile([BPC, 1], F32, tag="rstd")
            nc.vector.reciprocal(rstd, sdv)
            ln1 = sm.tile([BPC, D], F32, tag="ln1")
            nc.vector.tensor_scalar(
                ln1, o1, mv[:, 0:1], rstd, ALU.subtract, ALU.mult
            )
            ln2 = sm.tile([BPC, D], F32, tag="ln2")
            nc.vector.tensor_tensor(ln2, ln1, lng_s, ALU.mult)
            nc.vector.tensor_tensor(ln2, ln2, lnb_s, ALU.add)

            # cls layer 1
            lnT = [None, None]
            for h in range(2):
                lnT_ps = ps_m.tile([128, BPC], F32, tag="m")
                nc.tensor.transpose(
                    lnT_ps, ln2[:, h * 128 : (h + 1) * 128], idn_s[0:BPC, 0:BPC]
                )
                lnT[h] = sm.tile([128, BPC], F32, tag=f"lnT{h}", name=f"lnT{h}")
                nc.vector.tensor_copy(lnT[h], lnT_ps)
            h2p = ps_t.tile([BPC, D], F32, tag="t")
            nc.tensor.matmul(h2p, lnT[0], cw1_s[:, 0:D], start=True, stop=False)
            nc.tensor.matmul(
                h2p, lnT[1], cw1_s[:, D : 2 * D], start=False, stop=False
            )
            nc.tensor.matmul(
                h2p, ones[:, 0:BPC], cb1_s, start=False, stop=True
            )
            h2 = sm.tile([BPC, D], F32, tag="h2")
            nc.scalar.activation(h2, h2p, AF.Relu)

            # cls layer 2
            h2T = [None, None]
            for h in range(2):
                h2T_ps = ps_m.tile([128, BPC], F32, tag="m")
                nc.tensor.transpose(
                    h2T_ps, h2[:, h * 128 : (h + 1) * 128], idn_s[0:BPC, 0:BPC]
                )
                h2T[h] = sm.tile([128, BPC], F32, tag=f"h2T{h}", name=f"h2T{h}")
                nc.vector.tensor_copy(h2T[h], h2T_ps)
            lg = ps_t.tile([BPC, 2], F32, tag="t")
            nc.tensor.matmul(lg, h2T[0], cw2_s[:, 0:2], start=True, stop=False)
            nc.tensor.matmul(lg, h2T[1], cw2_s[:, 2:4], start=False, stop=False)
            nc.tensor.matmul(lg, ones[:, 0:BPC], cb2_s, start=False, stop=True)
            lgs = sm.tile([BPC, 2], F32, tag="lgs")
            nc.vector.tensor_copy(lgs, lg)
            nc.sync.dma_start(out=out[:, :], in_=lgs)

    if split_waits:
        _split_multi_waits(nc)
    return nc


_NC_CACHE = {}


def _get_program():
    if "nc" not in _NC_CACHE:
        _NC_CACHE["nc"] = build_program()
    return _NC_CACHE["nc"]


def _qff_matrix(qp):
    """Compose the 30 shared-parameter qff gates into one 64x64 complex matrix."""
    U = np.eye(DIM, dtype=np.complex128)
    for kind, loc, j in ansatz_gates(1):
        th = float(qp[j])
        c, s = np.cos(th / 2), np.sin(th / 2)
        G = np.zeros((DIM, DIM), np.complex128)
        if kind == "crx":
            wc, wt = loc
            bc, bt = 5 - wc, 5 - wt
            for k in range(DIM):
                if (k >> bc) & 1:
                    G[k, k] = c
                    G[k, k ^ (1 << bt)] = -1j * s
                else:
                    G[k, k] = 1.0
        else:
            bq = 5 - loc
            for k in range(DIM):
                kb = (k >> bq) & 1
                if kind == "rx":
                    G[k, k] = c
                    G[k, k ^ (1 << bq)] = -1j * s
                elif kind == "ry":
                    G[k, k] = c
                    G[k, k ^ (1 << bq)] = -s if kb == 0 else s
                else:  # rz
                    G[k, k] = np.exp(-0.5j * th) if kb == 0 else np.exp(0.5j * th)
        U = G @ U
    return U


def host_prep(inputs):
    """Host-side parameter folding -> per-core input maps."""
    f32 = np.float32
    x = np.asarray(inputs["x"], f32)
    emb_w = np.asarray(inputs["emb_w"], np.float64)
    emb_b = np.asarray(inputs["emb_b"], np.float64)
    att_w1 = np.asarray(inputs["att_w1"], np.float64)
    att_b1 = np.asarray(inputs["att_b1"], np.float64)

    f16 = np.float16
    wfb = (emb_w @ att_w1).astype(f16)
    bfold = (emb_b @ att_w1 + att_b1).astype(f32)[:, None]  # [128, 1]

    ewb = np.concatenate(
        [emb_w.astype(f16), emb_b.astype(f16)[None, :]], 0
    )

    pw = np.asarray(inputs["proj_w"], f16)
    pjw = np.concatenate([pw[0:128, :], pw[128:256, :]], 1)

    cr = np.asarray(inputs["mix_re"], np.float64)
    ci = np.asarray(inputs["mix_im"], np.float64)
    den = np.sqrt(cr * cr + ci * ci).sum() + 1e-8
    cf2 = np.stack([cr / den, ci / den], 1).astype(np.float16)

    qp = np.asarray(inputs["qff_params"], np.float64)
    U = _qff_matrix(qp)
    M = np.block([[U.real, -U.imag], [U.imag, U.real]])
    # folded observables: A~_o = M^T [[Pr, -Pi],[Pi, Pr]] M, o = X0..5,Y0..5,Z0..5
    aobs = np.zeros((DIM * 2, 18 * DIM * 2), np.float64)
    for kind in range(3):
        for i in range(NQ):
            bq = 5 - i
            P = np.zeros((DIM, DIM), np.complex128)
            for k in range(DIM):
                kb = (k >> bq) & 1
                if kind == 0:  # X
                    P[k, k ^ (1 << bq)] = 1.0
                elif kind == 1:  # Y
                    P[k, k ^ (1 << bq)] = 1j if kb else -1j
                else:  # Z
                    P[k, k] = -1.0 if kb else 1.0
            A = np.block([[P.real, -P.imag], [P.imag, P.real]])
            o = kind * NQ + i
            aobs[:, o * 128 : (o + 1) * 128] = M.T @ A @ M
    aob = aobs.astype(np.float16)

    owb = np.concatenate(
        [np.asarray(inputs["out_w"], f32), np.asarray(inputs["out_b"], f32)[None, :]],
        0,
    )
    lng = np.broadcast_to(np.asarray(inputs["ln_g"], f32), (BPC, D)).copy()
    lnb = np.broadcast_to(np.asarray(inputs["ln_b"], f32), (BPC, D)).copy()
    w1 = np.asarray(inputs["cls_w1"], f32)
    cw1 = np.concatenate([w1[0:128, :], w1[128:256, :]], 1)
    cb1 = np.asarray(inputs["cls_b1"], f32)[None, :]
    w2 = np.asarray(inputs["cls_w2"], f32)
    cw2 = np.concatenate([w2[0:128, :], w2[128:256, :]], 1)
    cb2 = np.asarray(inputs["cls_b2"], f32)[None, :]
    idn = np.eye(128, dtype=f32)
    pjb = np.asarray(inputs["proj_b"], f16)[None, :]

    shared = dict(
        wfb=wfb, bfold=bfold, aw2=np.asarray(inputs["att_w2"], f16), ewb=ewb,
        pjw=pjw, pjb=pjb, cf2=cf2, aob=aob, owb=owb, lng=lng,
        lnb=lnb, cw1=cw1, cb1=cb1, cw2=cw2, cb2=cb2, idn=idn,
    )

    x16 = x.astype(f16)
    in_maps = []
    for c in range(N_CORES):
        xc = x16[c * BPC : (c + 1) * BPC]
        # xp[b, nc, c*16+k] = x[b, c, nc*16+k]  (c-major, k inner)
        xp_c = np.ascontiguousarray(
            xc.reshape(BPC, C_IN, NC, CH).transpose(0, 2, 1, 3).reshape(
                BPC, NC, CH * C_IN
            )
        )
        m = dict(shared)
        m["xs"] = np.ascontiguousarray(xc)
        m["xp"] = xp_c
        in_maps.append(m)
    return in_maps


def kernel(**inputs):
    nc = _get_program()
    in_maps = host_prep(inputs)
    res = run_bass_kernel_spmd(nc, in_maps, core_ids=list(range(N_CORES)))
    outs = [res.results[c]["out"] for c in range(N_CORES)]
    return np.concatenate(outs, 0).astype(np.float32)


if __name__ == "__main__":
    nc = build_program()
    print("program built ok")


# revision 45
# speedup vs baseline: 2.9932x; 1.0451x over previous
"""Trainium2 Bass kernel for nn_ClassicalQuantumAttention.

Data-parallel over batch: 128 batch elems -> 16 per NeuronCore x 8 cores.

Per-core pipeline:
  classical   : scores path (PE matmuls + ACT tanh + softmax) and chunk path
                (weighted chunk sums, emb/proj matmuls) as in the baseline;
                circuit params sigmoid+sin/cos written as fp16 into SHARED
                coefficient tiles co/si/ns [128 chunks, 60 params x 16 b].
  quantum     : ALL 16 batch elems' statevectors in ONE fp16 tile
                ST [128 part = chunk, free = ri(2) x amp(64) x b(16)],
                b innermost.  Each gate = ~5 large tensor_tensor ops
                (FD 512-2048, fp16 2x mode) with per-(chunk,b) cos/sin
                applied via stride-0 broadcast views.  Layer-1 rotations
                use sparse (support-restricted) views.
  LCU         : per-b matmuls over chunk partitions (as baseline), then
                normalize on [16, 128].
  qff ansatz  : the 30 shared-parameter gates are ONE host-precomputed
                128x128 real matrix; applied by PE transpose + matmul.
  tail        : expvals (DVE quadratic forms), out head + layernorm +
                classifier (PE + small ops), as baseline.
"""

import numpy as np
import sys

for _p in ("/opt/trn_rl_repo",):
    if _p not in sys.path:
        sys.path.insert(0, _p)

import concourse.bass as bass
import concourse.tile as tile
from concourse import mybir
from concourse.bass_utils import run_bass_kernel_spmd

F32 = mybir.dt.float32
F16 = mybir.dt.float16
F8 = mybir.dt.float8e4
SC8 = True  # fp8 scores path (x, wfold, th, att_w2)
ALU = mybir.AluOpType
AF = mybir.ActivationFunctionType
AX = mybir.AxisListType

N_CORES = 8
B_TOT = 128
BPC = B_TOT // N_CORES  # 16 batch elems per core
C_IN = 64
T = 2048
D = 256
CH = 16
NC = T // CH  # 128 chunks
NQ = 6
DIM = 64  # 2**6 amplitudes
STF = 2 * DIM  # 128 floats per state ([64 re | 64 im])

# big-state free layout: idx = ri*1024 + amp*16 + b
SB = BPC          # 16 (b inner)
SAMP = DIM * SB   # 1024 (one ri slab)
SFREE = 2 * SAMP  # 2048


# ---------------------------------------------------------------- gate list
def ansatz_gates(n_layers):
    """[(kind, wire-or-(ctrl,tgt), param_idx)] matching reference _ansatz."""
    gates = []
    idx = 0
    for _ in range(n_layers):
        for i in range(NQ):
            gates.append(("rx", i, idx))
            gates.append(("ry", i, idx + 1))
            gates.append(("rz", i, idx + 2))
            idx += 3
        for i in range(NQ):
            gates.append(("crx", (i, (i + 1) % NQ), idx))
            idx += 1
        for i in range(NQ - 1, -1, -1):
            gates.append(("crx", (i, (i - 1) % NQ), idx))
            idx += 1
    return gates


# ------------------------------------------------------------- AP helpers
def fview(t, dims, off):
    return bass.AP(tensor=t.tensor, offset=t.offset + off, ap=[list(t.ap[0])] + dims)


def v_full(t, ri=None, w=6):
    """All involved amps (support width w: amps {k*2^(6-w)}), b inner.

    ri None: both ri slabs merged into the outer dim."""
    p = 6 - w
    step = (1 << p) * SB
    n = 1 << w
    if ri is None:
        return fview(t, [[step, 2 * n], [1, SB]], 0)
    return fview(t, [[step, n], [1, SB]], ri * SAMP)


def v_bit(t, p, val, ri=None, w=6):
    """Amps with bit p fixed to val; support width w (w<6 implies p == 6-w,
    lower bits all zero)."""
    off = val * (1 << p) * SB + (0 if ri is None else ri * SAMP)
    if w == 6:
        step_hi = (1 << (p + 1)) * SB
        n_hi = 1 << (5 - p)
        inner = (1 << p) * SB
        if ri is None:
            return fview(t, [[step_hi, 2 * n_hi], [1, inner]], off)
        return fview(t, [[step_hi, n_hi], [1, inner]], off)
    assert p == 6 - w
    step = (1 << (p + 1)) * SB
    n = 1 << (w - 1)
    if ri is None:
        return fview(t, [[step, 2 * n], [1, SB]], off)
    return fview(t, [[step, n], [1, SB]], off)


def v_2bit(t, ph, pl, vh, vl):
    """Both-ri view fixing adjacent amp bits ph = pl+1."""
    assert ph == pl + 1
    step_hi = (1 << (ph + 1)) * SB
    n_hi = 1 << (5 - ph)
    inner = (1 << pl) * SB
    off = (vh * (1 << ph) + vl * (1 << pl)) * SB
    return fview(t, [[step_hi, 2 * n_hi], [1, inner]], off)


def v_2bit_wrap(t, v5, v0, ri):
    """Per-ri view fixing amp bits 5 and 0 (the non-adjacent wrap case)."""
    off = ri * SAMP + (v5 * 32 + v0) * SB
    return fview(t, [[2 * SB, 16], [1, SB]], off)


def cview(ct, j, n):
    """Coefficient view for param j: [128, [0,n],[1,16]] (b inner)."""
    return bass.AP(
        tensor=ct.tensor, offset=ct.offset + SB * j,
        ap=[list(ct.ap[0]), [0, n], [1, SB]],
    )


# ------------------------------------------------------------ gate emitters
def v_ctrl(t, pc, ri):
    """Per-ri view of amps with bit pc = 1, when they form a single run
    (pc == 5: contiguous upper half; pc == 0: stride-2 odd amps)."""
    if pc == 5:
        return fview(t, [[SB, 32], [1, SB]], ri * SAMP + 32 * SB)
    assert pc == 0
    return fview(t, [[2 * SB, 32], [1, SB]], ri * SAMP + SB)


def emit_big_ansatz(nc, ST, B, B2, co, si, ns, cm1, ta, nta, gates, sparse_first):
    """Tangent-space rotations: ST here is ST_true / prod(cos of rotations).
    Caller must multiply by the cos product afterwards."""
    tt = nc.vector.tensor_tensor

    def rot(kind, p, j, w):
        n1 = 1 << w        # outer count of per-ri involved view
        n2 = 2 * n1        # both-ri
        if kind == "ry":
            # B = t*ST (no ri swap); ST[p0] -= B[p1]; ST[p1] += B[p0]
            tt(v_full(B, None, w), v_full(ST, None, w), cview(ta, j, n2), ALU.mult)
            tt(v_bit(ST, p, 0, None, w), v_bit(ST, p, 0, None, w),
               v_bit(B, p, 1, None, w), ALU.subtract)
            tt(v_bit(ST, p, 1, None, w), v_bit(ST, p, 1, None, w),
               v_bit(B, p, 0, None, w), ALU.add)
            return
        # rx / rz: B[re] = t*ST[im]; B[im] = -t*ST[re]
        tt(v_full(B, 0, w), v_full(ST, 1, w), cview(ta, j, n1), ALU.mult)
        tt(v_full(B, 1, w), v_full(ST, 0, w), cview(nta, j, n1), ALU.mult)
        if kind == "rx":
            # ST[p0] += B[p1]; ST[p1] += B[p0]
            tt(v_bit(ST, p, 0, None, w), v_bit(ST, p, 0, None, w),
               v_bit(B, p, 1, None, w), ALU.add)
            tt(v_bit(ST, p, 1, None, w), v_bit(ST, p, 1, None, w),
               v_bit(B, p, 0, None, w), ALU.add)
        else:  # rz: ST[p0] += B[p0]; ST[p1] -= B[p1]
            tt(v_bit(ST, p, 0, None, w), v_bit(ST, p, 0, None, w),
               v_bit(B, p, 0, None, w), ALU.add)
            tt(v_bit(ST, p, 1, None, w), v_bit(ST, p, 1, None, w),
               v_bit(B, p, 1, None, w), ALU.subtract)

    def crx_edge(pc, pt, j):
        # pc in {0, 5}: control-1 amps form a single run -> all ops restricted
        tt(v_ctrl(B, pc, 0), v_ctrl(ST, pc, 1), cview(si, j, 32), ALU.mult)
        tt(v_ctrl(B, pc, 1), v_ctrl(ST, pc, 0), cview(ns, j, 32), ALU.mult)
        if pc == 0:
            # both-ri scale merges (stride-2 run spans the ri boundary)
            v = fview(ST, [[2 * SB, 64], [1, SB]], SB)
            tt(v, v, cview(co, j, 64), ALU.mult)
        else:
            for ri in (0, 1):
                tt(v_ctrl(ST, pc, ri), v_ctrl(ST, pc, ri),
                   cview(co, j, 32), ALU.mult)
        if abs(pc - pt) == 1:  # (5,4) or (0,1)
            ph, pl = max(pc, pt), min(pc, pt)
            for k in (0, 1):
                if pc == ph:
                    o, i1 = v_2bit(ST, ph, pl, 1, k), v_2bit(B, ph, pl, 1, 1 - k)
                else:
                    o, i1 = v_2bit(ST, ph, pl, k, 1), v_2bit(B, ph, pl, 1 - k, 1)
                tt(o, o, i1, ALU.add)
        else:  # wrap: (5,0) or (0,5)
            for k in (0, 1):
                for ri in (0, 1):
                    if pc == 0:
                        o, i1 = v_2bit_wrap(ST, k, 1, ri), v_2bit_wrap(B, 1 - k, 1, ri)
                    else:
                        o, i1 = v_2bit_wrap(ST, 1, k, ri), v_2bit_wrap(B, 1, 1 - k, ri)
                    tt(o, o, i1, ALU.add)

    def crx(pc, pt, j):
        if pc in (0, 5):
            crx_edge(pc, pt, j)
            return
        # B[re] = s*ST[im]; B[im] = -s*ST[re]; B2 = (c-1)*ST
        tt(v_full(B, 0), v_full(ST, 1), cview(si, j, 64), ALU.mult)
        tt(v_full(B, 1), v_full(ST, 0), cview(ns, j, 64), ALU.mult)
        tt(v_full(B2, None), v_full(ST, None), cview(cm1, j, 128), ALU.mult)
        # ST[pc=1] += B2[pc=1]   (-> c*ST on the control-1 half)
        tt(v_bit(ST, pc, 1), v_bit(ST, pc, 1), v_bit(B2, pc, 1), ALU.add)
        # ST[pc=1, pt=k] += B[pc=1, pt=1-k]
        ph, pl = max(pc, pt), min(pc, pt)
        assert ph == pl + 1
        for k in (0, 1):
            if pc == ph:
                o, i1 = v_2bit(ST, ph, pl, 1, k), v_2bit(B, ph, pl, 1, 1 - k)
            else:
                o, i1 = v_2bit(ST, ph, pl, k, 1), v_2bit(B, ph, pl, 1 - k, 1)
            tt(o, o, i1, ALU.add)

    for gi, (kind, loc, j) in enumerate(gates):
        if kind == "crx":
            crx(5 - loc[0], 5 - loc[1], j)
        else:
            w = (loc + 1) if (sparse_first and gi < 3 * NQ) else 6
            rot(kind, 5 - loc, j, w)


# --------------------------------------------- baseline amp_view (tail use)
def amp_view(t, ri, fixed, swap_p=None, split_ps=()):
    """Strided view of a statevector AP t ([P, 128] = [P, (ri, amp6bits)])."""
    part = t.ap[0]
    offset = t.offset
    dims = []
    if ri is None:
        dims.append([DIM, 2])
    else:
        offset += ri * DIM
    run = None
    for p in range(5, -1, -1):
        if p in fixed:
            if run is not None:
                dims.append(run)
                run = None
            offset += fixed[p] << p
        elif swap_p == p:
            if run is not None:
                dims.append(run)
                run = None
            dims.append([-(1 << p), 2])
            offset += 1 << p
        elif p in split_ps:
            if run is not None:
                dims.append(run)
                run = None
            dims.append([1 << p, 2])
        else:
            if run is None:
                run = [1 << p, 2]
            else:
                run = [1 << p, run[1] * 2]
    if run is not None:
        dims.append(run)
    if not dims:
        dims.append([1, 1])
    assert len(dims) <= 2, f"too many free dims: {dims}"
    return bass.AP(tensor=t.tensor, offset=offset, ap=[list(part)] + dims)


def _split_multi_waits(nc):
    """This walrus build allows at most ONE sync-wait per instruction."""
    ctr = [0]
    for f in nc.m.functions:
        for b in f.blocks:
            new = []
            for inst in b.instructions:
                si = inst.sync_info
                if si is not None and len(si.on_wait) > 1:
                    waits = list(si.on_wait)
                    for w in waits[:-1]:
                        ctr[0] += 1
                        nop = mybir.InstNoOp(
                            name=f"wsplit-{ctr[0]}",
                            ins=[],
                            outs=[],
                            engine=inst.engine,
                            sync_info=mybir.SyncInfo(on_wait=[w], on_update=[]),
                        )
                        new.append(nop)
                    inst.sync_info = mybir.SyncInfo(
                        on_wait=[waits[-1]], on_update=list(si.on_update)
                    )
                new.append(inst)
            b.instructions = new


# ---------------------------------------------------------------- program
def build_program(split_waits=True):
    nc = bass.Bass()

    for v in (float(np.pi / 2), 1e-5, -1.0):
        t = nc.alloc_sbuf_tensor(f"const-f32-{v}", [128, 1], F32)
        nc.gpsimd.memset(t.ap(), v)
        nc.const_aps.aps[(F32, v)] = t.ap()
    nc.all_engine_barrier()

    # ---- dram I/O (per core) ----
    SCDT = F8 if SC8 else F16
    xs = nc.declare_dram_parameter("xs", [BPC, C_IN, T], SCDT, isOutput=False)
    xp = nc.declare_dram_parameter("xp", [BPC, NC, CH * C_IN], F16, isOutput=False)
    wfb = nc.declare_dram_parameter("wfb", [C_IN, 128], SCDT, isOutput=False)
    aw2 = nc.declare_dram_parameter("aw2", [128, 1], SCDT, isOutput=False)
    ewb = nc.declare_dram_parameter("ewb", [C_IN + 1, D], F16, isOutput=False)
    pjw = nc.declare_dram_parameter("pjw", [128, 120], F16, isOutput=False)
    pjb = nc.declare_dram_parameter("pjb", [128, 60], F32, isOutput=False)
    bfold = nc.declare_dram_parameter("bfold", [128, 1], F32, isOutput=False)
    cf2 = nc.declare_dram_parameter("cf2", [NC, 2], F16, isOutput=False)
    aob = nc.declare_dram_parameter("aob", [STF, 18 * STF], F16, isOutput=False)
    owb = nc.declare_dram_parameter("owb", [19, D], F32, isOutput=False)
    lng = nc.declare_dram_parameter("lng", [BPC, D], F32, isOutput=False)
    lnb = nc.declare_dram_parameter("lnb", [BPC, D], F32, isOutput=False)
    cw1 = nc.declare_dram_parameter("cw1", [128, 2 * D], F32, isOutput=False)
    cb1 = nc.declare_dram_parameter("cb1", [1, D], F32, isOutput=False)
    cw2 = nc.declare_dram_parameter("cw2", [128, 4], F32, isOutput=False)
    cb2 = nc.declare_dram_parameter("cb2", [1, 2], F32, isOutput=False)
    idn = nc.declare_dram_parameter("idn", [128, 128], F32, isOutput=False)
    out = nc.declare_dram_parameter("out", [BPC, 2], F32, isOutput=True)

    with tile.TileContext(nc) as tc:
        with (
            tc.tile_pool(name="const", bufs=1) as cp,
            tc.tile_pool(name="xbuf", bufs=2) as xpool,
            tc.tile_pool(name="xpbuf", bufs=2) as xppool,
            tc.tile_pool(name="tanh", bufs=2) as thpool,
            tc.tile_pool(name="small", bufs=4) as sm,
            tc.tile_pool(name="ps_h", bufs=2, space="PSUM") as ps_h,
            tc.tile_pool(name="ps_s", bufs=2, space="PSUM") as ps_s,
            tc.tile_pool(name="ps_m", bufs=2, space="PSUM") as ps_m,
            tc.tile_pool(name="ps_t", bufs=2, space="PSUM") as ps_t,
        ):
            # ---------------- constants into SBUF ----------------
            def cload(name, dram, shape, dt=F32):
                t = cp.tile(shape, dt, tag=name, name=name)
                nc.sync.dma_start(out=t, in_=dram[:, :])
                return t

            wfb_s = cload("wfb", wfb, [C_IN, 128], SCDT)
            aw2_s = cload("aw2", aw2, [128, 1], SCDT)
            ewb_s = cload("ewb", ewb, [C_IN + 1, D], F16)
            pjw_s = cload("pjw", pjw, [128, 120], F16)
            pjb_s = cload("pjb", pjb, [128, 60])
            bfold_s = cload("bfold", bfold, [128, 1])
            cf2_s = cload("cf2", cf2, [NC, 2], F16)
            aob_s = cload("aob", aob, [STF, 18 * STF], F16)
            owb_s = cload("owb", owb, [19, D])
            lng_s = cload("lng", lng, [BPC, D])
            lnb_s = cload("lnb", lnb, [BPC, D])
            cw1_s = cload("cw1", cw1, [128, 2 * D])
            cb1_s = cload("cb1", cb1, [1, D])
            cw2_s = cload("cw2", cw2, [128, 4])
            cb2_s = cload("cb2", cb2, [1, 2])
            idn_s = cload("idn", idn, [128, 128])

            ones = cp.tile([1, 128], F32, tag="ones")
            nc.vector.memset(ones, 1.0)

            # persistent per-group score tiles
            sc_g = [cp.tile([NC, 8 * CH], F32, tag=f"scg{g}", name=f"scg{g}") for g in range(2)]
            esc_g = [cp.tile([NC, 8 * CH], F32, tag=f"escg{g}", name=f"escg{g}") for g in range(2)]
            w_g = [cp.tile([NC, 8 * CH], F16, tag=f"wg{g}", name=f"wg{g}") for g in range(2)]

            # shared fp16 coefficient tiles: free = param_j*16 + b
            co_t = cp.tile([NC, 60 * SB], F16, tag="co", name="co")
            si_t = cp.tile([NC, 60 * SB], F16, tag="si", name="si")
            ns_t = cp.tile([NC, 60 * SB], F16, tag="ns", name="ns")
            cm1_t = cp.tile([NC, 60 * SB], F16, tag="cm1", name="cm1")
            ta_t = cp.tile([NC, 60 * SB], F16, tag="ta", name="ta")
            nta_t = cp.tile([NC, 60 * SB], F16, tag="nta", name="nta")
            ctot = cp.tile([NC, 60 * SB], F32, tag="ctot", name="ctot")

            # big state + scratch tiles
            ST = cp.tile([NC, SFREE], F16, tag="ST", name="ST")
            Bt = cp.tile([NC, SFREE], F16, tag="Bt", name="Bt")
            B2t = cp.tile([NC, SFREE], F16, tag="B2t", name="B2t")

            # per-b double buffers
            x_sb = [xpool.tile([C_IN, T], SCDT, tag="x", name=f"xsb{i}") for i in range(2)]
            xp_sb = [xppool.tile([NC, CH * C_IN], F16, tag="xp", name=f"xpsb{i}") for i in range(2)]
            xwt_sb = [xppool.tile([C_IN + 1, NC], F16, tag="xwt", name=f"xwtsb{i}") for i in range(2)]
            for i in range(2):
                nc.vector.memset(xwt_sb[i][C_IN : C_IN + 1, :], 1.0)

            # staged sigmoid inputs: free = param_j*16 + b (for batched ACT)
            theta_all = cp.tile([NC, 60 * SB], F32, tag="theta", name="theta")

            lq_all = cp.tile([BPC, 2 * STF], F32, tag="lqall")
            mix = cp.tile([BPC, STF], F32, tag="mix")
            qfeat = cp.tile([BPC, 19], F32, tag="qfeat")
            nc.vector.memset(qfeat[:, 18:19], 1.0)

            # ================= classical per-b =================
            for b in range(BPC):
                xb = x_sb[b % 2]
                nc.sync.dma_start(out=xb, in_=xs[b, :, :])

                th = thpool.tile([128, T], SCDT, tag="th")
                ssc = sm.tile([1, T], F32, tag="ssc", name="ssc")
                for blk in range(4):
                    hp = ps_h.tile([128, 512], F32, tag="hp")
                    nc.tensor.matmul(
                        hp,
                        wfb_s,
                        xb[:, blk * 512 : (blk + 1) * 512],
                        start=True,
                        stop=True,
                    )
                    nc.scalar.activation(
                        th[:, blk * 512 : (blk + 1) * 512], hp, AF.Tanh,
                        bias=bfold_s,
                    )
                    sc = ps_s.tile([1, 512], F32, tag="sc")
                    nc.tensor.matmul(
                        sc,
                        aw2_s,
                        th[:, blk * 512 : (blk + 1) * 512],
                        start=True,
                        stop=True,
                    )
                    if blk % 2 == 0:
                        nc.vector.tensor_copy(ssc[:, blk * 512 : (blk + 1) * 512], sc)
                    else:
                        nc.scalar.copy(ssc[:, blk * 512 : (blk + 1) * 512], sc)
                g, bb = b // 8, b % 8
                src = ssc.rearrange("p (n k) -> p n k", n=128, k=CH)
                dst = sc_g[g][:, bb * CH : (bb + 1) * CH]
                nc.sync.dma_start(out=dst, in_=src)

                # ---- group softmax + per-b chunk path, after each group of 8
                if b % 8 == 7:
                    g = b // 8
                    nc.scalar.activation(esc_g[g], sc_g[g], AF.Exp)
                    ssum = sm.tile([NC, 8], F32, tag="ssum")
                    nc.vector.tensor_reduce(
                        ssum,
                        esc_g[g].rearrange("p (n k) -> p n k", n=8, k=CH),
                        AX.X,
                        ALU.add,
                    )
                    rsum = sm.tile([NC, 8], F32, tag="rsum")
                    nc.vector.reciprocal(rsum, ssum)
                    for bb in range(8):
                        nc.vector.tensor_scalar_mul(
                            w_g[g][:, bb * CH : (bb + 1) * CH],
                            esc_g[g][:, bb * CH : (bb + 1) * CH],
                            rsum[:, bb : bb + 1],
                        )

                    for bb in range(8):
                        bfull = g * 8 + bb
                        xpb = xp_sb[bfull % 2]
                        nc.sync.dma_start(out=xpb, in_=xp[bfull, :, :])
                        # xw[nc, c] = sum_k w[nc, k] * xpb[nc, c*16+k]
                        xwp = sm.tile([NC, CH * C_IN], F16, tag="xwp")
                        wv = bass.AP(
                            tensor=w_g[g].tensor,
                            offset=w_g[g].offset + bb * CH,
                            ap=[list(w_g[g].ap[0]), [0, C_IN], [1, CH]],
                        )
                        xv = fview(xpb, [[CH, C_IN], [1, CH]], 0)
                        ov = fview(xwp, [[CH, C_IN], [1, CH]], 0)
                        nc.vector.tensor_tensor(ov, xv, wv, ALU.mult)
                        xw = sm.tile([NC, C_IN], F32, tag="xw")
                        nc.vector.tensor_reduce(
                            xw,
                            xwp.rearrange("p (c k) -> p c k", c=C_IN, k=CH),
                            AX.X,
                            ALU.add,
                        )
                        xwt_ps = ps_m.tile([C_IN, NC], F32, tag="m")
                        nc.tensor.transpose(xwt_ps, xw, idn_s)
                        xwt = xwt_sb[bfull % 2]
                        nc.vector.tensor_copy(xwt[0:C_IN, :], xwt_ps)
                        cht = [None, None]
                        for h in range(2):
                            chp = ps_m.tile([128, NC], F32, tag="m")
                            nc.tensor.matmul(
                                chp,
                                ewb_s[:, h * 128 : (h + 1) * 128],
                                xwt,
                                start=True,
                                stop=True,
                            )
                            cht[h] = sm.tile([128, NC], F16, tag=f"cht{h}", name=f"cht{h}")
                            nc.vector.tensor_copy(cht[h], chp)
                        par = ps_t.tile([NC, 60], F32, tag="t")
                        nc.tensor.matmul(
                            par, cht[0], pjw_s[:, 0:60], start=True, stop=False
                        )
                        nc.tensor.matmul(
                            par, cht[1], pjw_s[:, 60:120], start=False, stop=True
                        )
                        # stage sigmoid input (+ proj bias) into (j*16+b) slots
                        nc.vector.tensor_tensor(
                            fview(theta_all, [[SB, 60]], bfull), par, pjb_s,
                            ALU.add,
                        )

            # batched: theta = sigmoid(z); cos/sin/negsin/cos-1 (fp16)
            nc.scalar.activation(theta_all, theta_all, AF.Sigmoid)
            nc.scalar.activation(
                co_t, theta_all, AF.Sin, bias=float(np.pi / 2), scale=0.5
            )
            nc.scalar.activation(si_t, theta_all, AF.Sin, bias=0.0, scale=0.5)
            nc.scalar.activation(ns_t, theta_all, AF.Sin, bias=0.0, scale=-0.5)
            nc.scalar.activation(cm1_t, co_t, AF.Copy, bias=-1.0)

            # tangent coefficients: ta = si/co, nta = -ta  (via fp32 recip)
            t32a = cp.tile([NC, 60 * SB], F32, tag="t32a", name="t32a")
            t32b = cp.tile([NC, 60 * SB], F32, tag="t32b", name="t32b")
            nc.scalar.activation(
                t32a, theta_all, AF.Sin, bias=float(np.pi / 2), scale=0.5
            )  # cos32
            nc.vector.reciprocal(t32b, t32a)
            # cos product tree seed (uses fp32 cos before it is overwritten)
            nc.vector.tensor_tensor(
                ctot[:, 0:288], t32a[:, 0:288], t32a[:, 480:768], ALU.mult
            )
            nc.scalar.activation(t32a, theta_all, AF.Sin, bias=0.0, scale=0.5)
            nc.vector.tensor_tensor(ta_t, t32a, t32b, ALU.mult)
            nc.vector.tensor_scalar_mul(nta_t, ta_t, -1.0)

            # ================= quantum stage 1 (b-batched, tangent space) ===
            nc.vector.memset(ST, 0.0)
            nc.vector.memset(fview(ST, [[1, SB]], 0), 1.0)  # amp0, re, all b

            emit_big_ansatz(
                nc, ST, Bt, B2t, co_t, si_t, ns_t, cm1_t, ta_t, nta_t,
                ansatz_gates(2), sparse_first=True,
            )

            # cos product over the 36 rotation params (seed done above)
            nc.vector.tensor_tensor(
                ctot[:, 0:144], ctot[:, 0:144], ctot[:, 144:288], ALU.mult
            )
            nc.vector.tensor_tensor(
                ctot[:, 0:64], ctot[:, 0:64], ctot[:, 64:128], ALU.mult
            )
            nc.vector.tensor_tensor(
                ctot[:, 0:32], ctot[:, 0:32], ctot[:, 32:64], ALU.mult
            )
            nc.vector.tensor_tensor(
                ctot[:, 0:16], ctot[:, 0:16], ctot[:, 16:32], ALU.mult
            )
            nc.vector.tensor_tensor(
                ctot[:, 0:16], ctot[:, 0:16], ctot[:, 128:144], ALU.mult
            )
            ctot16 = sm.tile([NC, SB], F16, tag="ctot16")
            nc.vector.tensor_copy(ctot16, ctot[:, 0:16])
            nc.vector.tensor_tensor(
                v_full(ST, None, 6), v_full(ST, None, 6),
                cview(ctot16, 0, 128), ALU.mult,
            )

            # ---- LCU: per-b matmuls over chunk partitions ----
            lrow = cp.tile([1, BPC * 2 * STF], F32, tag="lrow", name="lrow")
            for b in range(BPC):
                rhs_all = fview(ST, [[SB, STF]], b)
                r0 = ps_t.tile([1, STF], F32, tag="t")
                nc.tensor.matmul(r0, cf2_s[:, 0:1], rhs_all, start=True, stop=True)
                r1 = ps_s.tile([1, STF], F32, tag="sc", name="r1")
                nc.tensor.matmul(r1, cf2_s[:, 1:2], rhs_all, start=True, stop=True)
                o = b * 2 * STF
                nc.scalar.copy(lrow[:, o : o + STF], r0)
                nc.vector.tensor_copy(lrow[:, o + STF : o + 2 * STF], r1)
            nc.sync.dma_start(
                out=lq_all,
                in_=lrow.rearrange("p (b f) -> p b f", b=BPC, f=2 * STF),
            )

            # mixed_re = r0_re - r1_im ; mixed_im = r0_im + r1_re
            nc.vector.tensor_tensor(
                mix[:, 0:DIM], lq_all[:, 0:DIM],
                lq_all[:, STF + DIM : 2 * STF], ALU.subtract,
            )
            nc.vector.tensor_tensor(
                mix[:, DIM:STF], lq_all[:, DIM:STF],
                lq_all[:, STF : STF + DIM], ALU.add,
            )
            # squared norm and 1/n^2 (normalization folded into qfeat scale)
            sqs = sm.tile([BPC, STF], F32, tag="sqs")
            ss = sm.tile([BPC, 1], F32, tag="ss")
            nc.vector.tensor_tensor(sqs, mix, mix, ALU.mult)
            nc.vector.tensor_reduce(ss, sqs, AX.X, ALU.add)
            rn2 = sm.tile([BPC, 1], F32, tag="rn2")
            nc.vector.reciprocal(rn2, ss)

            # ============ expvals via PE: qfeat_o = mix^T (M^T A_o M) mix ====
            # E = mix^T @ Astack  ->  [16, 18*128];  qfeat_o[b] = sum_p E*mix
            mT_ps = ps_m.tile([STF, BPC], F32, tag="m")
            nc.tensor.transpose(mT_ps, mix, idn_s[0:BPC, 0:BPC])
            mixh = sm.tile([STF, BPC], F16, tag="mixh")
            nc.vector.tensor_copy(mixh, mT_ps)
            Et = cp.tile([BPC, 18 * STF], F32, tag="Et", name="Et")
            for c5 in range(5):
                n = min(512, 18 * STF - c5 * 512)
                E_ps = ps_h.tile([BPC, 512], F32, tag="hp")
                nc.tensor.matmul(
                    E_ps[:, 0:n], mixh, aob_s[:, c5 * 512 : c5 * 512 + n],
                    start=True, stop=True,
                )
                if c5 % 2 == 0:
                    nc.scalar.copy(Et[:, c5 * 512 : c5 * 512 + n], E_ps[:, 0:n])
                else:
                    nc.vector.tensor_copy(Et[:, c5 * 512 : c5 * 512 + n], E_ps[:, 0:n])
            mixv = bass.AP(
                tensor=mix.tensor, offset=mix.offset,
                ap=[list(mix.ap[0]), [0, 18], [1, STF]],
            )
            nc.vector.tensor_tensor(
                Et.rearrange("p (o f) -> p o f", o=18, f=STF), Et.rearrange(
                    "p (o f) -> p o f", o=18, f=STF), mixv, ALU.mult,
            )
            qf01 = sm.tile([BPC, 18], F32, tag="qf01")
            nc.vector.tensor_reduce(
                qf01, Et.rearrange("p (o f) -> p o f", o=18, f=STF), AX.X, ALU.add
            )
            nc.vector.tensor_scalar_mul(qfeat[:, 0:18], qf01, rn2)

            # ================= tail =================
            qfT_ps = ps_m.tile([19, BPC], F32, tag="m")
            nc.tensor.transpose(qfT_ps, qfeat, idn_s[0:BPC, 0:BPC])
            qfT = sm.tile([19, BPC], F32, tag="qfTs")
            nc.vector.tensor_copy(qfT, qfT_ps)
            o1 = ps_t.tile([BPC, D], F32, tag="t")
            nc.tensor.matmul(o1, qfT, owb_s, start=True, stop=True)

            stats = sm.tile([BPC, 6], F32, tag="stats")
            nc.vector.bn_stats(stats, o1)
            mv = sm.tile([BPC, 2], F32, tag="mv")
            nc.vector.bn_aggr(mv, stats)
            sdv = sm.tile([BPC, 1], F32, tag="sdv")
            nc.scalar.activation(sdv, mv[:, 1:2], AF.Sqrt, bias=1e-5)
            rstd = sm.tile([BPC, 1], F32, tag="rstd")
            nc.vector.reciprocal(rstd, sdv)
            ln1 = sm.tile([BPC, D], F32, tag="ln1")
            nc.vector.tensor_scalar(
                ln1, o1, mv[:, 0:1], rstd, ALU.subtract, ALU.mult
            )
            ln2 = sm.tile([BPC, D], F32, tag="ln2")
            nc.vector.tensor_tensor(ln2, ln1, lng_s, ALU.mult)
            nc.vector.tensor_tensor(ln2, ln2, lnb_s, ALU.add)

            # cls layer 1
            lnT = [None, None]
            for h in range(2):
                lnT_ps = ps_m.tile([128, BPC], F32, tag="m")
                nc.tensor.transpose(
                    lnT_ps, ln2[:, h * 128 : (h + 1) * 128], idn_s[0:BPC, 0:BPC]
                )
                lnT[h] = sm.tile([128, BPC], F32, tag=f"lnT{h}", name=f"lnT{h}")
                nc.vector.tensor_copy(lnT[h], lnT_ps)
            h2p = ps_t.tile([BPC, D], F32, tag="t")
            nc.tensor.matmul(h2p, lnT[0], cw1_s[:, 0:D], start=True, stop=False)
            nc.tensor.matmul(
                h2p, lnT[1], cw1_s[:, D : 2 * D], start=False, stop=False
            )
            nc.tensor.matmul(
                h2p, ones[:, 0:BPC], cb1_s, start=False, stop=True
            )
            h2 = sm.tile([BPC, D], F32, tag="h2")
            nc.scalar.activation(h2, h2p, AF.Relu)

            # cls layer 2
            h2T = [None, None]
            for h in range(2):
                h2T_ps = ps_m.tile([128, BPC], F32, tag="m")
                nc.tensor.transpose(
                    h2T_ps, h2[:, h * 128 : (h + 1) * 128], idn_s[0:BPC, 0:BPC]
                )
                h2T[h] = sm.tile([128, BPC], F32, tag=f"h2T{h}", name=f"h2T{h}")
                nc.vector.tensor_copy(h2T[h], h2T_ps)
            lg = ps_t.tile([BPC, 2], F32, tag="t")
            nc.tensor.matmul(lg, h2T[0], cw2_s[:, 0:2], start=True, stop=False)
            nc.tensor.matmul(lg, h2T[1], cw2_s[:, 2:4], start=False, stop=False)
            nc.tensor.matmul(lg, ones[:, 0:BPC], cb2_s, start=False, stop=True)
            lgs = sm.tile([BPC, 2], F32, tag="lgs")
            nc.vector.tensor_copy(lgs, lg)
            nc.sync.dma_start(out=out[:, :], in_=lgs)

    if split_waits:
        _split_multi_waits(nc)
    return nc


_NC_CACHE = {}


def _get_program():
    if "nc" not in _NC_CACHE:
        _NC_CACHE["nc"] = build_program()
    return _NC_CACHE["nc"]


def _qff_matrix(qp):
    """Compose the 30 shared-parameter qff gates into one 64x64 complex matrix."""
    U = np.eye(DIM, dtype=np.complex128)
    for kind, loc, j in ansatz_gates(1):
        th = float(qp[j])
        c, s = np.cos(th / 2), np.sin(th / 2)
        G = np.zeros((DIM, DIM), np.complex128)
        if kind == "crx":
            wc, wt = loc
            bc, bt = 5 - wc, 5 - wt
            for k in range(DIM):
                if (k >> bc) & 1:
                    G[k, k] = c
                    G[k, k ^ (1 << bt)] = -1j * s
                else:
                    G[k, k] = 1.0
        else:
            bq = 5 - loc
            for k in range(DIM):
                kb = (k >> bq) & 1
                if kind == "rx":
                    G[k, k] = c
                    G[k, k ^ (1 << bq)] = -1j * s
                elif kind == "ry":
                    G[k, k] = c
                    G[k, k ^ (1 << bq)] = -s if kb == 0 else s
                else:  # rz
                    G[k, k] = np.exp(-0.5j * th) if kb == 0 else np.exp(0.5j * th)
        U = G @ U
    return U


def host_prep(inputs):
    """Host-side parameter folding -> per-core input maps."""
    f32 = np.float32
    x = np.asarray(inputs["x"], f32)
    emb_w = np.asarray(inputs["emb_w"], np.float64)
    emb_b = np.asarray(inputs["emb_b"], np.float64)
    att_w1 = np.asarray(inputs["att_w1"], np.float64)
    att_b1 = np.asarray(inputs["att_b1"], np.float64)

    f16 = np.float16
    import ml_dtypes
    scdt = ml_dtypes.float8_e4m3 if SC8 else f16
    wfb = (emb_w @ att_w1).astype(scdt)
    bfold = (emb_b @ att_w1 + att_b1).astype(f32)[:, None]  # [128, 1]

    ewb = np.concatenate(
        [emb_w.astype(f16), emb_b.astype(f16)[None, :]], 0
    )

    pw = np.asarray(inputs["proj_w"], f16)
    pjw = np.concatenate([pw[0:128, :], pw[128:256, :]], 1)

    cr = np.asarray(inputs["mix_re"], np.float64)
    ci = np.asarray(inputs["mix_im"], np.float64)
    den = np.sqrt(cr * cr + ci * ci).sum() + 1e-8
    cf2 = np.stack([cr / den, ci / den], 1).astype(np.float16)

    qp = np.asarray(inputs["qff_params"], np.float64)
    U = _qff_matrix(qp)
    M = np.block([[U.real, -U.imag], [U.imag, U.real]])
    # folded observables: A~_o = M^T [[Pr, -Pi],[Pi, Pr]] M, o = X0..5,Y0..5,Z0..5
    aobs = np.zeros((DIM * 2, 18 * DIM * 2), np.float64)
    for kind in range(3):
        for i in range(NQ):
            bq = 5 - i
            P = np.zeros((DIM, DIM), np.complex128)
            for k in range(DIM):
                kb = (k >> bq) & 1
                if kind == 0:  # X
                    P[k, k ^ (1 << bq)] = 1.0
                elif kind == 1:  # Y
                    P[k, k ^ (1 << bq)] = 1j if kb else -1j
                else:  # Z
                    P[k, k] = -1.0 if kb else 1.0
            A = np.block([[P.real, -P.imag], [P.imag, P.real]])
            o = kind * NQ + i
            aobs[:, o * 128 : (o + 1) * 128] = M.T @ A @ M
    aob = aobs.astype(np.float16)

    owb = np.concatenate(
        [np.asarray(inputs["out_w"], f32), np.asarray(inputs["out_b"], f32)[None, :]],
        0,
    )
    lng = np.broadcast_to(np.asarray(inputs["ln_g"], f32), (BPC, D)).copy()
    lnb = np.broadcast_to(np.asarray(inputs["ln_b"], f32), (BPC, D)).copy()
    w1 = np.asarray(inputs["cls_w1"], f32)
    cw1 = np.concatenate([w1[0:128, :], w1[128:256, :]], 1)
    cb1 = np.asarray(inputs["cls_b1"], f32)[None, :]
    w2 = np.asarray(inputs["cls_w2"], f32)
    cw2 = np.concatenate([w2[0:128, :], w2[128:256, :]], 1)
    cb2 = np.asarray(inputs["cls_b2"], f32)[None, :]
    idn = np.eye(128, dtype=f32)
    pjb = np.broadcast_to(
        np.asarray(inputs["proj_b"], f32), (NC, 60)
    ).copy()

    shared = dict(
        wfb=wfb, bfold=bfold, aw2=np.asarray(inputs["att_w2"], scdt), ewb=ewb,
        pjw=pjw, pjb=pjb, cf2=cf2, aob=aob, owb=owb, lng=lng,
        lnb=lnb, cw1=cw1, cb1=cb1, cw2=cw2, cb2=cb2, idn=idn,
    )

    x16 = x.astype(f16)
    xsc = x.astype(scdt)
    in_maps = []
    for c in range(N_CORES):
        xc = x16[c * BPC : (c + 1) * BPC]
        # xp[b, nc, c*16+k] = x[b, c, nc*16+k]  (c-major, k inner)
        xp_c = np.ascontiguousarray(
            xc.reshape(BPC, C_IN, NC, CH).transpose(0, 2, 1, 3).reshape(
                BPC, NC, CH * C_IN
            )
        )
        m = dict(shared)
        m["xs"] = np.ascontiguousarray(xsc[c * BPC : (c + 1) * BPC])
        m["xp"] = xp_c
        in_maps.append(m)
    return in_maps


def kernel(**inputs):
    nc = _get_program()
    in_maps = host_prep(inputs)
    res = run_bass_kernel_spmd(nc, in_maps, core_ids=list(range(N_CORES)))
    outs = [res.results[c]["out"] for c in range(N_CORES)]
    return np.concatenate(outs, 0).astype(np.float32)


if __name__ == "__main__":
    nc = build_program()
    print("program built ok")


# revision 51
# speedup vs baseline: 3.0104x; 1.0058x over previous
"""Trainium2 Bass kernel for nn_ClassicalQuantumAttention.

Data-parallel over batch: 128 batch elems -> 16 per NeuronCore x 8 cores.

Per-core pipeline:
  classical   : scores path (PE matmuls + ACT tanh + softmax) and chunk path
                (weighted chunk sums, emb/proj matmuls) as in the baseline;
                circuit params sigmoid+sin/cos written as fp16 into SHARED
                coefficient tiles co/si/ns [128 chunks, 60 params x 16 b].
  quantum     : ALL 16 batch elems' statevectors in ONE fp16 tile
                ST [128 part = chunk, free = ri(2) x amp(64) x b(16)],
                b innermost.  Each gate = ~5 large tensor_tensor ops
                (FD 512-2048, fp16 2x mode) with per-(chunk,b) cos/sin
                applied via stride-0 broadcast views.  Layer-1 rotations
                use sparse (support-restricted) views.
  LCU         : per-b matmuls over chunk partitions (as baseline), then
                normalize on [16, 128].
  qff ansatz  : the 30 shared-parameter gates are ONE host-precomputed
                128x128 real matrix; applied by PE transpose + matmul.
  tail        : expvals (DVE quadratic forms), out head + layernorm +
                classifier (PE + small ops), as baseline.
"""

import numpy as np
import sys

for _p in ("/opt/trn_rl_repo",):
    if _p not in sys.path:
        sys.path.insert(0, _p)

import concourse.bass as bass
import concourse.tile as tile
from concourse import mybir
from concourse.bass_utils import run_bass_kernel_spmd

F32 = mybir.dt.float32
F16 = mybir.dt.float16
F8 = mybir.dt.float8e4
SC8 = True  # fp8 scores path (x, wfold, th, att_w2)
ALU = mybir.AluOpType
AF = mybir.ActivationFunctionType
AX = mybir.AxisListType

N_CORES = 8
B_TOT = 128
BPC = B_TOT // N_CORES  # 16 batch elems per core
C_IN = 64
T = 2048
D = 256
CH = 16
NC = T // CH  # 128 chunks
NQ = 6
DIM = 64  # 2**6 amplitudes
STF = 2 * DIM  # 128 floats per state ([64 re | 64 im])

# big-state free layout: idx = ri*1024 + amp*16 + b
SB = BPC          # 16 (b inner)
SAMP = DIM * SB   # 1024 (one ri slab)
SFREE = 2 * SAMP  # 2048


# ---------------------------------------------------------------- gate list
def ansatz_gates(n_layers):
    """[(kind, wire-or-(ctrl,tgt), param_idx)] matching reference _ansatz."""
    gates = []
    idx = 0
    for _ in range(n_layers):
        for i in range(NQ):
            gates.append(("rx", i, idx))
            gates.append(("ry", i, idx + 1))
            gates.append(("rz", i, idx + 2))
            idx += 3
        for i in range(NQ):
            gates.append(("crx", (i, (i + 1) % NQ), idx))
            idx += 1
        for i in range(NQ - 1, -1, -1):
            gates.append(("crx", (i, (i - 1) % NQ), idx))
            idx += 1
    return gates


# ------------------------------------------------------------- AP helpers
def fview(t, dims, off):
    return bass.AP(tensor=t.tensor, offset=t.offset + off, ap=[list(t.ap[0])] + dims)


def v_full(t, ri=None, w=6):
    """All involved amps (support width w: amps {k*2^(6-w)}), b inner.

    ri None: both ri slabs merged into the outer dim."""
    p = 6 - w
    step = (1 << p) * SB
    n = 1 << w
    if ri is None:
        return fview(t, [[step, 2 * n], [1, SB]], 0)
    return fview(t, [[step, n], [1, SB]], ri * SAMP)


def v_bit(t, p, val, ri=None, w=6):
    """Amps with bit p fixed to val; support width w (w<6 implies p == 6-w,
    lower bits all zero)."""
    off = val * (1 << p) * SB + (0 if ri is None else ri * SAMP)
    if w == 6:
        step_hi = (1 << (p + 1)) * SB
        n_hi = 1 << (5 - p)
        inner = (1 << p) * SB
        if ri is None:
            return fview(t, [[step_hi, 2 * n_hi], [1, inner]], off)
        return fview(t, [[step_hi, n_hi], [1, inner]], off)
    assert p == 6 - w
    step = (1 << (p + 1)) * SB
    n = 1 << (w - 1)
    if ri is None:
        return fview(t, [[step, 2 * n], [1, SB]], off)
    return fview(t, [[step, n], [1, SB]], off)


def v_2bit(t, ph, pl, vh, vl):
    """Both-ri view fixing adjacent amp bits ph = pl+1."""
    assert ph == pl + 1
    step_hi = (1 << (ph + 1)) * SB
    n_hi = 1 << (5 - ph)
    inner = (1 << pl) * SB
    off = (vh * (1 << ph) + vl * (1 << pl)) * SB
    return fview(t, [[step_hi, 2 * n_hi], [1, inner]], off)


def v_2bit_wrap(t, v5, v0, ri):
    """Per-ri view fixing amp bits 5 and 0 (the non-adjacent wrap case)."""
    off = ri * SAMP + (v5 * 32 + v0) * SB
    return fview(t, [[2 * SB, 16], [1, SB]], off)


def cview(ct, j, n):
    """Coefficient view for param j: [128, [0,n],[1,16]] (b inner)."""
    return bass.AP(
        tensor=ct.tensor, offset=ct.offset + SB * j,
        ap=[list(ct.ap[0]), [0, n], [1, SB]],
    )


# ------------------------------------------------------------ gate emitters
def v_ctrl(t, pc, ri):
    """Per-ri view of amps with bit pc = 1, when they form a single run
    (pc == 5: contiguous upper half; pc == 0: stride-2 odd amps)."""
    if pc == 5:
        return fview(t, [[SB, 32], [1, SB]], ri * SAMP + 32 * SB)
    assert pc == 0
    return fview(t, [[2 * SB, 32], [1, SB]], ri * SAMP + SB)


def emit_big_ansatz(nc, ST, B, B2, co, si, ns, cm1, ta, nta, gates, sparse_first):
    """Tangent-space rotations: ST here is ST_true / prod(cos of rotations).
    Caller must multiply by the cos product afterwards."""
    tt = nc.vector.tensor_tensor

    def rot(kind, p, j, w):
        n1 = 1 << w        # outer count of per-ri involved view
        n2 = 2 * n1        # both-ri
        if kind == "ry":
            # B = t*ST (no ri swap); ST[p0] -= B[p1]; ST[p1] += B[p0]
            tt(v_full(B, None, w), v_full(ST, None, w), cview(ta, j, n2), ALU.mult)
            tt(v_bit(ST, p, 0, None, w), v_bit(ST, p, 0, None, w),
               v_bit(B, p, 1, None, w), ALU.subtract)
            tt(v_bit(ST, p, 1, None, w), v_bit(ST, p, 1, None, w),
               v_bit(B, p, 0, None, w), ALU.add)
            return
        # rx / rz: B[re] = t*ST[im]; B[im] = -t*ST[re]
        tt(v_full(B, 0, w), v_full(ST, 1, w), cview(ta, j, n1), ALU.mult)
        tt(v_full(B, 1, w), v_full(ST, 0, w), cview(nta, j, n1), ALU.mult)
        if kind == "rx":
            # ST[p0] += B[p1]; ST[p1] += B[p0]
            tt(v_bit(ST, p, 0, None, w), v_bit(ST, p, 0, None, w),
               v_bit(B, p, 1, None, w), ALU.add)
            tt(v_bit(ST, p, 1, None, w), v_bit(ST, p, 1, None, w),
               v_bit(B, p, 0, None, w), ALU.add)
        else:  # rz: ST[p0] += B[p0]; ST[p1] -= B[p1]
            tt(v_bit(ST, p, 0, None, w), v_bit(ST, p, 0, None, w),
               v_bit(B, p, 0, None, w), ALU.add)
            tt(v_bit(ST, p, 1, None, w), v_bit(ST, p, 1, None, w),
               v_bit(B, p, 1, None, w), ALU.subtract)

    def crx_edge(pc, pt, j):
        # pc in {0, 5}: control-1 amps form a single run -> all ops restricted
        tt(v_ctrl(B, pc, 0), v_ctrl(ST, pc, 1), cview(si, j, 32), ALU.mult)
        tt(v_ctrl(B, pc, 1), v_ctrl(ST, pc, 0), cview(ns, j, 32), ALU.mult)
        if pc == 0:
            # both-ri scale merges (stride-2 run spans the ri boundary)
            v = fview(ST, [[2 * SB, 64], [1, SB]], SB)
            tt(v, v, cview(co, j, 64), ALU.mult)
        else:
            for ri in (0, 1):
                tt(v_ctrl(ST, pc, ri), v_ctrl(ST, pc, ri),
                   cview(co, j, 32), ALU.mult)
        if abs(pc - pt) == 1:  # (5,4) or (0,1)
            ph, pl = max(pc, pt), min(pc, pt)
            for k in (0, 1):
                if pc == ph:
                    o, i1 = v_2bit(ST, ph, pl, 1, k), v_2bit(B, ph, pl, 1, 1 - k)
                else:
                    o, i1 = v_2bit(ST, ph, pl, k, 1), v_2bit(B, ph, pl, 1 - k, 1)
                tt(o, o, i1, ALU.add)
        else:  # wrap: (5,0) or (0,5)
            for k in (0, 1):
                for ri in (0, 1):
                    if pc == 0:
                        o, i1 = v_2bit_wrap(ST, k, 1, ri), v_2bit_wrap(B, 1 - k, 1, ri)
                    else:
                        o, i1 = v_2bit_wrap(ST, 1, k, ri), v_2bit_wrap(B, 1, 1 - k, ri)
                    tt(o, o, i1, ALU.add)

    def crx(pc, pt, j):
        if pc in (0, 5):
            crx_edge(pc, pt, j)
            return
        # B[re] = s*ST[im]; B[im] = -s*ST[re]; B2 = (c-1)*ST
        tt(v_full(B, 0), v_full(ST, 1), cview(si, j, 64), ALU.mult)
        tt(v_full(B, 1), v_full(ST, 0), cview(ns, j, 64), ALU.mult)
        tt(v_full(B2, None), v_full(ST, None), cview(cm1, j, 128), ALU.mult)
        # ST[pc=1] += B2[pc=1]   (-> c*ST on the control-1 half)
        tt(v_bit(ST, pc, 1), v_bit(ST, pc, 1), v_bit(B2, pc, 1), ALU.add)
        # ST[pc=1, pt=k] += B[pc=1, pt=1-k]
        ph, pl = max(pc, pt), min(pc, pt)
        assert ph == pl + 1
        for k in (0, 1):
            if pc == ph:
                o, i1 = v_2bit(ST, ph, pl, 1, k), v_2bit(B, ph, pl, 1, 1 - k)
            else:
                o, i1 = v_2bit(ST, ph, pl, k, 1), v_2bit(B, ph, pl, 1 - k, 1)
            tt(o, o, i1, ALU.add)

    for gi, (kind, loc, j) in enumerate(gates):
        if kind == "crx":
            crx(5 - loc[0], 5 - loc[1], j)
        else:
            w = (loc + 1) if (sparse_first and gi < 3 * NQ) else 6
            rot(kind, 5 - loc, j, w)


# --------------------------------------------- baseline amp_view (tail use)
def amp_view(t, ri, fixed, swap_p=None, split_ps=()):
    """Strided view of a statevector AP t ([P, 128] = [P, (ri, amp6bits)])."""
    part = t.ap[0]
    offset = t.offset
    dims = []
    if ri is None:
        dims.append([DIM, 2])
    else:
        offset += ri * DIM
    run = None
    for p in range(5, -1, -1):
        if p in fixed:
            if run is not None:
                dims.append(run)
                run = None
            offset += fixed[p] << p
        elif swap_p == p:
            if run is not None:
                dims.append(run)
                run = None
            dims.append([-(1 << p), 2])
            offset += 1 << p
        elif p in split_ps:
            if run is not None:
                dims.append(run)
                run = None
            dims.append([1 << p, 2])
        else:
            if run is None:
                run = [1 << p, 2]
            else:
                run = [1 << p, run[1] * 2]
    if run is not None:
        dims.append(run)
    if not dims:
        dims.append([1, 1])
    assert len(dims) <= 2, f"too many free dims: {dims}"
    return bass.AP(tensor=t.tensor, offset=offset, ap=[list(part)] + dims)


def _split_multi_waits(nc):
    """This walrus build allows at most ONE sync-wait per instruction."""
    ctr = [0]
    for f in nc.m.functions:
        for b in f.blocks:
            new = []
            for inst in b.instructions:
                si = inst.sync_info
                if si is not None and len(si.on_wait) > 1:
                    waits = list(si.on_wait)
                    for w in waits[:-1]:
                        ctr[0] += 1
                        nop = mybir.InstNoOp(
                            name=f"wsplit-{ctr[0]}",
                            ins=[],
                            outs=[],
                            engine=inst.engine,
                            sync_info=mybir.SyncInfo(on_wait=[w], on_update=[]),
                        )
                        new.append(nop)
                    inst.sync_info = mybir.SyncInfo(
                        on_wait=[waits[-1]], on_update=list(si.on_update)
                    )
                new.append(inst)
            b.instructions = new


# ---------------------------------------------------------------- program
def build_program(split_waits=True):
    nc = bass.Bass()

    for v in (float(np.pi / 2), 1e-5, -1.0):
        t = nc.alloc_sbuf_tensor(f"const-f32-{v}", [128, 1], F32)
        nc.gpsimd.memset(t.ap(), v)
        nc.const_aps.aps[(F32, v)] = t.ap()
    nc.all_engine_barrier()

    # ---- dram I/O (per core) ----
    SCDT = F8 if SC8 else F16
    xs = nc.declare_dram_parameter("xs", [BPC, C_IN, T], SCDT, isOutput=False)
    xp = nc.declare_dram_parameter("xp", [BPC, NC, CH * C_IN], F16, isOutput=False)
    wfb = nc.declare_dram_parameter("wfb", [C_IN, 128], SCDT, isOutput=False)
    aw2 = nc.declare_dram_parameter("aw2", [128, 1], SCDT, isOutput=False)
    ewb = nc.declare_dram_parameter("ewb", [C_IN + 1, D], F16, isOutput=False)
    pjw = nc.declare_dram_parameter("pjw", [128, 120], F16, isOutput=False)
    pjb = nc.declare_dram_parameter("pjb", [128, 60], F32, isOutput=False)
    bfold = nc.declare_dram_parameter("bfold", [128, 1], F32, isOutput=False)
    cf2 = nc.declare_dram_parameter("cf2", [NC, 2], F16, isOutput=False)
    aob = nc.declare_dram_parameter("aob", [STF, 18 * STF], F16, isOutput=False)
    owb = nc.declare_dram_parameter("owb", [19, D], F32, isOutput=False)
    lng = nc.declare_dram_parameter("lng", [BPC, D], F32, isOutput=False)
    lnb = nc.declare_dram_parameter("lnb", [BPC, D], F32, isOutput=False)
    cw1 = nc.declare_dram_parameter("cw1", [128, 2 * D], F32, isOutput=False)
    cb1 = nc.declare_dram_parameter("cb1", [1, D], F32, isOutput=False)
    cw2 = nc.declare_dram_parameter("cw2", [128, 4], F32, isOutput=False)
    cb2 = nc.declare_dram_parameter("cb2", [1, 2], F32, isOutput=False)
    idn = nc.declare_dram_parameter("idn", [128, 128], F32, isOutput=False)
    out = nc.declare_dram_parameter("out", [BPC, 2], F32, isOutput=True)

    with tile.TileContext(nc) as tc:
        with (
            tc.tile_pool(name="const", bufs=1) as cp,
            tc.tile_pool(name="xbuf", bufs=2) as xpool,
            tc.tile_pool(name="xpbuf", bufs=2) as xppool,
            tc.tile_pool(name="tanh", bufs=2) as thpool,
            tc.tile_pool(name="small", bufs=4) as sm,
            tc.tile_pool(name="ps_h", bufs=2, space="PSUM") as ps_h,
            tc.tile_pool(name="ps_s", bufs=2, space="PSUM") as ps_s,
            tc.tile_pool(name="ps_m", bufs=2, space="PSUM") as ps_m,
            tc.tile_pool(name="ps_t", bufs=2, space="PSUM") as ps_t,
        ):
            # ---------------- constants into SBUF ----------------
            def cload(name, dram, shape, dt=F32):
                t = cp.tile(shape, dt, tag=name, name=name)
                nc.sync.dma_start(out=t, in_=dram[:, :])
                return t

            # classical-path constants first (DMA issue order matters:
            # the first hpre matmul waits on wfb + xs[0])
            wfb_s = cload("wfb", wfb, [C_IN, 128], SCDT)
            bfold_s = cload("bfold", bfold, [128, 1])
            aw2_s = cload("aw2", aw2, [128, 1], SCDT)
            ewb_s = cload("ewb", ewb, [C_IN + 1, D], F16)
            pjw_s = cload("pjw", pjw, [128, 120], F16)
            pjb_s = cload("pjb", pjb, [128, 60])
            idn_s = cload("idn", idn, [128, 128])

            ones = cp.tile([1, 128], F32, tag="ones")
            nc.vector.memset(ones, 1.0)

            # persistent per-group score tiles
            sc_g = [cp.tile([NC, 8 * CH], F32, tag=f"scg{g}", name=f"scg{g}") for g in range(2)]
            esc_g = [cp.tile([NC, 8 * CH], F32, tag=f"escg{g}", name=f"escg{g}") for g in range(2)]
            w_g = [cp.tile([NC, 8 * CH], F16, tag=f"wg{g}", name=f"wg{g}") for g in range(2)]

            # shared fp16 coefficient tiles: free = param_j*16 + b
            co_t = cp.tile([NC, 60 * SB], F16, tag="co", name="co")
            si_t = cp.tile([NC, 60 * SB], F16, tag="si", name="si")
            ns_t = cp.tile([NC, 60 * SB], F16, tag="ns", name="ns")
            cm1_t = cp.tile([NC, 60 * SB], F16, tag="cm1", name="cm1")
            ta_t = cp.tile([NC, 60 * SB], F16, tag="ta", name="ta")
            nta_t = cp.tile([NC, 60 * SB], F16, tag="nta", name="nta")
            ctot = cp.tile([NC, 60 * SB], F32, tag="ctot", name="ctot")

            # big state + scratch tiles
            ST = cp.tile([NC, SFREE], F16, tag="ST", name="ST")
            Bt = cp.tile([NC, SFREE], F16, tag="Bt", name="Bt")
            B2t = cp.tile([NC, SFREE], F16, tag="B2t", name="B2t")

            # per-b double buffers
            x_sb = [xpool.tile([C_IN, T], SCDT, tag="x", name=f"xsb{i}") for i in range(2)]
            xp_sb = [xppool.tile([NC, CH * C_IN], F16, tag="xp", name=f"xpsb{i}") for i in range(2)]
            xwt_sb = [xppool.tile([C_IN + 1, NC], F16, tag="xwt", name=f"xwtsb{i}") for i in range(2)]
            for i in range(2):
                nc.vector.memset(xwt_sb[i][C_IN : C_IN + 1, :], 1.0)

            # staged sigmoid inputs: free = param_j*16 + b (for batched ACT)
            theta_all = cp.tile([NC, 60 * SB], F32, tag="theta", name="theta")

            lq_all = cp.tile([BPC, 2 * STF], F32, tag="lqall")
            mix = cp.tile([BPC, STF], F32, tag="mix")
            qfeat = cp.tile([BPC, 19], F32, tag="qfeat")
            nc.vector.memset(qfeat[:, 18:19], 1.0)

            # PE warm-up burst: ~5us of dense matmuls to release the HAM
            # cold-throttle (K=4/8 -> 8/8) before the scores phase
            for wi in range(16):
                wup = ps_h.tile([128, 128], F32, tag="hp")
                nc.tensor.matmul(wup, idn_s, idn_s, start=True, stop=True)

            # tail-only constants (issued after the classical ones)
            cf2_s = cload("cf2", cf2, [NC, 2], F16)
            aob_s = cload("aob", aob, [STF, 18 * STF], F16)
            owb_s = cload("owb", owb, [19, D])
            lng_s = cload("lng", lng, [BPC, D])
            lnb_s = cload("lnb", lnb, [BPC, D])
            cw1_s = cload("cw1", cw1, [128, 2 * D])
            cb1_s = cload("cb1", cb1, [1, D])
            cw2_s = cload("cw2", cw2, [128, 4])
            cb2_s = cload("cb2", cb2, [1, 2])

            # ================= classical per-b =================
            for b in range(BPC):
                xb = x_sb[b % 2]
                nc.sync.dma_start(out=xb, in_=xs[b, :, :])

                th = thpool.tile([128, T], SCDT, tag="th")
                ssc = sm.tile([1, T], F32, tag="ssc", name="ssc")
                for blk in range(4):
                    hp = ps_h.tile([128, 512], F32, tag="hp")
                    nc.tensor.matmul(
                        hp,
                        wfb_s,
                        xb[:, blk * 512 : (blk + 1) * 512],
                        start=True,
                        stop=True,
                    )
                    nc.scalar.activation(
                        th[:, blk * 512 : (blk + 1) * 512], hp, AF.Tanh,
                        bias=bfold_s,
                    )
                    sc = ps_s.tile([1, 512], F32, tag="sc")
                    nc.tensor.matmul(
                        sc,
                        aw2_s,
                        th[:, blk * 512 : (blk + 1) * 512],
                        start=True,
                        stop=True,
                    )
                    if blk % 4 == 3:
                        nc.scalar.copy(ssc[:, blk * 512 : (blk + 1) * 512], sc)
                    else:
                        nc.vector.tensor_copy(ssc[:, blk * 512 : (blk + 1) * 512], sc)
                g, bb = b // 8, b % 8
                src = ssc.rearrange("p (n k) -> p n k", n=128, k=CH)
                dst = sc_g[g][:, bb * CH : (bb + 1) * CH]
                nc.sync.dma_start(out=dst, in_=src)

                # ---- group softmax + per-b chunk path, after each group of 8
                if b % 8 == 7:
                    g = b // 8
                    nc.scalar.activation(esc_g[g], sc_g[g], AF.Exp)
                    ssum = sm.tile([NC, 8], F32, tag="ssum")
                    nc.vector.tensor_reduce(
                        ssum,
                        esc_g[g].rearrange("p (n k) -> p n k", n=8, k=CH),
                        AX.X,
                        ALU.add,
                    )
                    rsum = sm.tile([NC, 8], F32, tag="rsum")
                    nc.vector.reciprocal(rsum, ssum)
                    for bb in range(8):
                        nc.vector.tensor_scalar_mul(
                            w_g[g][:, bb * CH : (bb + 1) * CH],
                            esc_g[g][:, bb * CH : (bb + 1) * CH],
                            rsum[:, bb : bb + 1],
                        )

                    for bb in range(8):
                        bfull = g * 8 + bb
                        xpb = xp_sb[bfull % 2]
                        nc.sync.dma_start(out=xpb, in_=xp[bfull, :, :])
                        # xw[nc, c] = sum_k w[nc, k] * xpb[nc, c*16+k]
                        xwp = sm.tile([NC, CH * C_IN], F16, tag="xwp")
                        wv = bass.AP(
                            tensor=w_g[g].tensor,
                            offset=w_g[g].offset + bb * CH,
                            ap=[list(w_g[g].ap[0]), [0, C_IN], [1, CH]],
                        )
                        xv = fview(xpb, [[CH, C_IN], [1, CH]], 0)
                        ov = fview(xwp, [[CH, C_IN], [1, CH]], 0)
                        nc.vector.tensor_tensor(ov, xv, wv, ALU.mult)
                        xw = sm.tile([NC, C_IN], F32, tag="xw")
                        nc.vector.tensor_reduce(
                            xw,
                            xwp.rearrange("p (c k) -> p c k", c=C_IN, k=CH),
                            AX.X,
                            ALU.add,
                        )
                        xwt_ps = ps_m.tile([C_IN, NC], F32, tag="m")
                        nc.tensor.transpose(xwt_ps, xw, idn_s)
                        xwt = xwt_sb[bfull % 2]
                        nc.vector.tensor_copy(xwt[0:C_IN, :], xwt_ps)
                        cht = [None, None]
                        for h in range(2):
                            chp = ps_m.tile([128, NC], F32, tag="m")
                            nc.tensor.matmul(
                                chp,
                                ewb_s[:, h * 128 : (h + 1) * 128],
                                xwt,
                                start=True,
                                stop=True,
                            )
                            cht[h] = sm.tile([128, NC], F16, tag=f"cht{h}", name=f"cht{h}")
                            nc.vector.tensor_copy(cht[h], chp)
                        par = ps_t.tile([NC, 60], F32, tag="t")
                        nc.tensor.matmul(
                            par, cht[0], pjw_s[:, 0:60], start=True, stop=False
                        )
                        nc.tensor.matmul(
                            par, cht[1], pjw_s[:, 60:120], start=False, stop=True
                        )
                        # stage sigmoid input (+ proj bias) into (j*16+b) slots
                        nc.vector.tensor_tensor(
                            fview(theta_all, [[SB, 60]], bfull), par, pjb_s,
                            ALU.add,
                        )

            # batched: theta = sigmoid(z); cos/sin/negsin/cos-1 (fp16)
            nc.scalar.activation(theta_all, theta_all, AF.Sigmoid)
            nc.scalar.activation(
                co_t, theta_all, AF.Sin, bias=float(np.pi / 2), scale=0.5
            )
            nc.scalar.activation(si_t, theta_all, AF.Sin, bias=0.0, scale=0.5)
            nc.scalar.activation(ns_t, theta_all, AF.Sin, bias=0.0, scale=-0.5)
            nc.scalar.activation(cm1_t, co_t, AF.Copy, bias=-1.0)

            # tangent coefficients: ta = si/co, nta = -ta  (via fp32 recip)
            t32a = cp.tile([NC, 60 * SB], F32, tag="t32a", name="t32a")
            t32b = cp.tile([NC, 60 * SB], F32, tag="t32b", name="t32b")
            nc.scalar.activation(
                t32a, theta_all, AF.Sin, bias=float(np.pi / 2), scale=0.5
            )  # cos32
            nc.vector.reciprocal(t32b, t32a)
            # cos product tree seed (uses fp32 cos before it is overwritten)
            nc.vector.tensor_tensor(
                ctot[:, 0:288], t32a[:, 0:288], t32a[:, 480:768], ALU.mult
            )
            nc.scalar.activation(t32a, theta_all, AF.Sin, bias=0.0, scale=0.5)
            nc.vector.tensor_tensor(ta_t, t32a, t32b, ALU.mult)
            nc.vector.tensor_scalar_mul(nta_t, ta_t, -1.0)

            # ================= quantum stage 1 (b-batched, tangent space) ===
            nc.vector.memset(ST, 0.0)
            nc.vector.memset(fview(ST, [[1, SB]], 0), 1.0)  # amp0, re, all b

            emit_big_ansatz(
                nc, ST, Bt, B2t, co_t, si_t, ns_t, cm1_t, ta_t, nta_t,
                ansatz_gates(2), sparse_first=True,
            )

            # cos product over the 36 rotation params (seed done above)
            nc.vector.tensor_tensor(
                ctot[:, 0:144], ctot[:, 0:144], ctot[:, 144:288], ALU.mult
            )
            nc.vector.tensor_tensor(
                ctot[:, 0:64], ctot[:, 0:64], ctot[:, 64:128], ALU.mult
            )
            nc.vector.tensor_tensor(
                ctot[:, 0:32], ctot[:, 0:32], ctot[:, 32:64], ALU.mult
            )
            nc.vector.tensor_tensor(
                ctot[:, 0:16], ctot[:, 0:16], ctot[:, 16:32], ALU.mult
            )
            nc.vector.tensor_tensor(
                ctot[:, 0:16], ctot[:, 0:16], ctot[:, 128:144], ALU.mult
            )
            ctot16 = sm.tile([NC, SB], F16, tag="ctot16")
            nc.vector.tensor_copy(ctot16, ctot[:, 0:16])
            nc.vector.tensor_tensor(
                v_full(ST, None, 6), v_full(ST, None, 6),
                cview(ctot16, 0, 128), ALU.mult,
            )

            # ---- LCU: per-b matmuls over chunk partitions ----
            lrow = cp.tile([1, BPC * 2 * STF], F32, tag="lrow", name="lrow")
            for b in range(BPC):
                rhs_all = fview(ST, [[SB, STF]], b)
                r0 = ps_t.tile([1, STF], F32, tag="t")
                nc.tensor.matmul(r0, cf2_s[:, 0:1], rhs_all, start=True, stop=True)
                r1 = ps_s.tile([1, STF], F32, tag="sc", name="r1")
                nc.tensor.matmul(r1, cf2_s[:, 1:2], rhs_all, start=True, stop=True)
                o = b * 2 * STF
                nc.scalar.copy(lrow[:, o : o + STF], r0)
                nc.vector.tensor_copy(lrow[:, o + STF : o + 2 * STF], r1)
            nc.sync.dma_start(
                out=lq_all,
                in_=lrow.rearrange("p (b f) -> p b f", b=BPC, f=2 * STF),
            )

            # mixed_re = r0_re - r1_im ; mixed_im = r0_im + r1_re
            nc.vector.tensor_tensor(
                mix[:, 0:DIM], lq_all[:, 0:DIM],
                lq_all[:, STF + DIM : 2 * STF], ALU.subtract,
            )
            nc.vector.tensor_tensor(
                mix[:, DIM:STF], lq_all[:, DIM:STF],
                lq_all[:, STF : STF + DIM], ALU.add,
            )
            # squared norm and 1/n^2 (normalization folded into qfeat scale)
            sqs = sm.tile([BPC, STF], F32, tag="sqs")
            ss = sm.tile([BPC, 1], F32, tag="ss")
            nc.vector.tensor_tensor(sqs, mix, mix, ALU.mult)
            nc.vector.tensor_reduce(ss, sqs, AX.X, ALU.add)
            rn2 = sm.tile([BPC, 1], F32, tag="rn2")
            nc.vector.reciprocal(rn2, ss)

            # ============ expvals via PE: qfeat_o = mix^T (M^T A_o M) mix ====
            # E = mix^T @ Astack  ->  [16, 18*128];  qfeat_o[b] = sum_p E*mix
            mT_ps = ps_m.tile([STF, BPC], F32, tag="m")
            nc.tensor.transpose(mT_ps, mix, idn_s[0:BPC, 0:BPC])
            mixh = sm.tile([STF, BPC], F16, tag="mixh")
            nc.vector.tensor_copy(mixh, mT_ps)
            Et = cp.tile([BPC, 18 * STF], F32, tag="Et", name="Et")
            for c5 in range(5):
                n = min(512, 18 * STF - c5 * 512)
                E_ps = ps_h.tile([BPC, 512], F32, tag="hp")
                nc.tensor.matmul(
                    E_ps[:, 0:n], mixh, aob_s[:, c5 * 512 : c5 * 512 + n],
                    start=True, stop=True,
                )
                if c5 % 2 == 0:
                    nc.scalar.copy(Et[:, c5 * 512 : c5 * 512 + n], E_ps[:, 0:n])
                else:
                    nc.vector.tensor_copy(Et[:, c5 * 512 : c5 * 512 + n], E_ps[:, 0:n])
            mixv = bass.AP(
                tensor=mix.tensor, offset=mix.offset,
                ap=[list(mix.ap[0]), [0, 18], [1, STF]],
            )
            nc.vector.tensor_tensor(
                Et.rearrange("p (o f) -> p o f", o=18, f=STF), Et.rearrange(
                    "p (o f) -> p o f", o=18, f=STF), mixv, ALU.mult,
            )
            qf01 = sm.tile([BPC, 18], F32, tag="qf01")
            nc.vector.tensor_reduce(
                qf01, Et.rearrange("p (o f) -> p o f", o=18, f=STF), AX.X, ALU.add
            )
            nc.vector.tensor_scalar_mul(qfeat[:, 0:18], qf01, rn2)

            # ================= tail =================
            qfT_ps = ps_m.tile([19, BPC], F32, tag="m")
            nc.tensor.transpose(qfT_ps, qfeat, idn_s[0:BPC, 0:BPC])
            qfT = sm.tile([19, BPC], F32, tag="qfTs")
            nc.vector.tensor_copy(qfT, qfT_ps)
            o1 = ps_t.tile([BPC, D], F32, tag="t")
            nc.tensor.matmul(o1, qfT, owb_s, start=True, stop=True)

            stats = sm.tile([BPC, 6], F32, tag="stats")
            nc.vector.bn_stats(stats, o1)
            mv = sm.tile([BPC, 2], F32, tag="mv")
            nc.vector.bn_aggr(mv, stats)
            sdv = sm.tile([BPC, 1], F32, tag="sdv")
            nc.scalar.activation(sdv, mv[:, 1:2], AF.Sqrt, bias=1e-5)
            rstd = sm.tile([BPC, 1], F32, tag="rstd")
            nc.vector.reciprocal(rstd, sdv)
            ln1 = sm.tile([BPC, D], F32, tag="ln1")
            nc.vector.tensor_scalar(
                ln1, o1, mv[:, 0:1], rstd, ALU.subtract, ALU.mult
            )
            ln2 = sm.tile([BPC, D], F32, tag="ln2")
            nc.vector.tensor_tensor(ln2, ln1, lng_s, ALU.mult)
            nc.vector.tensor_tensor(ln2, ln2, lnb_s, ALU.add)

            # cls layer 1
            lnT = [None, None]
            for h in range(2):
                lnT_ps = ps_m.tile([128, BPC], F32, tag="m")
                nc.tensor.transpose(
                    lnT_ps, ln2[:, h * 128 : (h + 1) * 128], idn_s[0:BPC, 0:BPC]
                )
                lnT[h] = sm.tile([128, BPC], F32, tag=f"lnT{h}", name=f"lnT{h}")
                nc.vector.tensor_copy(lnT[h], lnT_ps)
            h2p = ps_t.tile([BPC, D], F32, tag="t")
            nc.tensor.matmul(h2p, lnT[0], cw1_s[:, 0:D], start=True, stop=False)
            nc.tensor.matmul(
                h2p, lnT[1], cw1_s[:, D : 2 * D], start=False, stop=False
            )
            nc.tensor.matmul(
                h2p, ones[:, 0:BPC], cb1_s, start=False, stop=True
            )
            h2 = sm.tile([BPC, D], F32, tag="h2")
            nc.scalar.activation(h2, h2p, AF.Relu)

            # cls layer 2
            h2T = [None, None]
            for h in range(2):
                h2T_ps = ps_m.tile([128, BPC], F32, tag="m")
                nc.tensor.transpose(
                    h2T_ps, h2[:, h * 128 : (h + 1) * 128], idn_s[0:BPC, 0:BPC]
                )
                h2T[h] = sm.tile([128, BPC], F32, tag=f"h2T{h}", name=f"h2T{h}")
                nc.vector.tensor_copy(h2T[h], h2T_ps)
            lg = ps_t.tile([BPC, 2], F32, tag="t")
            nc.tensor.matmul(lg, h2T[0], cw2_s[:, 0:2], start=True, stop=False)
            nc.tensor.matmul(lg, h2T[1], cw2_s[:, 2:4], start=False, stop=False)
            nc.tensor.matmul(lg, ones[:, 0:BPC], cb2_s, start=False, stop=True)
            lgs = sm.tile([BPC, 2], F32, tag="lgs")
            nc.vector.tensor_copy(lgs, lg)
            nc.sync.dma_start(out=out[:, :], in_=lgs)

    if split_waits:
        _split_multi_waits(nc)
    return nc


_NC_CACHE = {}


def _get_program():
    if "nc" not in _NC_CACHE:
        _NC_CACHE["nc"] = build_program()
    return _NC_CACHE["nc"]


def _qff_matrix(qp):
    """Compose the 30 shared-parameter qff gates into one 64x64 complex matrix."""
    U = np.eye(DIM, dtype=np.complex128)
    for kind, loc, j in ansatz_gates(1):
        th = float(qp[j])
        c, s = np.cos(th / 2), np.sin(th / 2)
        G = np.zeros((DIM, DIM), np.complex128)
        if kind == "crx":
            wc, wt = loc
            bc, bt = 5 - wc, 5 - wt
            for k in range(DIM):
                if (k >> bc) & 1:
                    G[k, k] = c
                    G[k, k ^ (1 << bt)] = -1j * s
                else:
                    G[k, k] = 1.0
        else:
            bq = 5 - loc
            for k in range(DIM):
                kb = (k >> bq) & 1
                if kind == "rx":
                    G[k, k] = c
                    G[k, k ^ (1 << bq)] = -1j * s
                elif kind == "ry":
                    G[k, k] = c
                    G[k, k ^ (1 << bq)] = -s if kb == 0 else s
                else:  # rz
                    G[k, k] = np.exp(-0.5j * th) if kb == 0 else np.exp(0.5j * th)
        U = G @ U
    return U


def host_prep(inputs):
    """Host-side parameter folding -> per-core input maps."""
    f32 = np.float32
    x = np.asarray(inputs["x"], f32)
    emb_w = np.asarray(inputs["emb_w"], np.float64)
    emb_b = np.asarray(inputs["emb_b"], np.float64)
    att_w1 = np.asarray(inputs["att_w1"], np.float64)
    att_b1 = np.asarray(inputs["att_b1"], np.float64)

    f16 = np.float16
    import ml_dtypes
    scdt = ml_dtypes.float8_e4m3 if SC8 else f16
    wfb = (emb_w @ att_w1).astype(scdt)
    bfold = (emb_b @ att_w1 + att_b1).astype(f32)[:, None]  # [128, 1]

    ewb = np.concatenate(
        [emb_w.astype(f16), emb_b.astype(f16)[None, :]], 0
    )

    pw = np.asarray(inputs["proj_w"], f16)
    pjw = np.concatenate([pw[0:128, :], pw[128:256, :]], 1)

    cr = np.asarray(inputs["mix_re"], np.float64)
    ci = np.asarray(inputs["mix_im"], np.float64)
    den = np.sqrt(cr * cr + ci * ci).sum() + 1e-8
    cf2 = np.stack([cr / den, ci / den], 1).astype(np.float16)

    qp = np.asarray(inputs["qff_params"], np.float64)
    U = _qff_matrix(qp)
    M = np.block([[U.real, -U.imag], [U.imag, U.real]])
    # folded observables: A~_o = M^T [[Pr, -Pi],[Pi, Pr]] M, o = X0..5,Y0..5,Z0..5
    aobs = np.zeros((DIM * 2, 18 * DIM * 2), np.float64)
    for kind in range(3):
        for i in range(NQ):
            bq = 5 - i
            P = np.zeros((DIM, DIM), np.complex128)
            for k in range(DIM):
                kb = (k >> bq) & 1
                if kind == 0:  # X
                    P[k, k ^ (1 << bq)] = 1.0
                elif kind == 1:  # Y
                    P[k, k ^ (1 << bq)] = 1j if kb else -1j
                else:  # Z
                    P[k, k] = -1.0 if kb else 1.0
            A = np.block([[P.real, -P.imag], [P.imag, P.real]])
            o = kind * NQ + i
            aobs[:, o * 128 : (o + 1) * 128] = M.T @ A @ M
    aob = aobs.astype(np.float16)

    owb = np.concatenate(
        [np.asarray(inputs["out_w"], f32), np.asarray(inputs["out_b"], f32)[None, :]],
        0,
    )
    lng = np.broadcast_to(np.asarray(inputs["ln_g"], f32), (BPC, D)).copy()
    lnb = np.broadcast_to(np.asarray(inputs["ln_b"], f32), (BPC, D)).copy()
    w1 = np.asarray(inputs["cls_w1"], f32)
    cw1 = np.concatenate([w1[0:128, :], w1[128:256, :]], 1)
    cb1 = np.asarray(inputs["cls_b1"], f32)[None, :]
    w2 = np.asarray(inputs["cls_w2"], f32)
    cw2 = np.concatenate([w2[0:128, :], w2[128:256, :]], 1)
    cb2 = np.asarray(inputs["cls_b2"], f32)[None, :]
    idn = np.eye(128, dtype=f32)
    pjb = np.broadcast_to(
        np.asarray(inputs["proj_b"], f32), (NC, 60)
    ).copy()

    shared = dict(
        wfb=wfb, bfold=bfold, aw2=np.asarray(inputs["att_w2"], scdt), ewb=ewb,
        pjw=pjw, pjb=pjb, cf2=cf2, aob=aob, owb=owb, lng=lng,
        lnb=lnb, cw1=cw1, cb1=cb1, cw2=cw2, cb2=cb2, idn=idn,
    )

    x16 = x.astype(f16)
    xsc = x.astype(scdt)
    in_maps = []
    for c in range(N_CORES):
        xc = x16[c * BPC : (c + 1) * BPC]
        # xp[b, nc, c*16+k] = x[b, c, nc*16+k]  (c-major, k inner)
        xp_c = np.ascontiguousarray(
            xc.reshape(BPC, C_IN, NC, CH).transpose(0, 2, 1, 3).reshape(
                BPC, NC, CH * C_IN
            )
        )
        m = dict(shared)
        m["xs"] = np.ascontiguousarray(xsc[c * BPC : (c + 1) * BPC])
        m["xp"] = xp_c
        in_maps.append(m)
    return in_maps


def kernel(**inputs):
    nc = _get_program()
    in_maps = host_prep(inputs)
    res = run_bass_kernel_spmd(nc, in_maps, core_ids=list(range(N_CORES)))
    outs = [res.results[c]["out"] for c in range(N_CORES)]
    return np.concatenate(outs, 0).astype(np.float32)


if __name__ == "__main__":
    nc = build_program()
    print("program built ok")
